# revision 46
# baseline (speedup 1.0000x reference)
"""Trainium2 Bass kernel for batched 7-DOF RNEA inverse dynamics.

Layout: pure data-parallel over 8 NeuronCores (32768 batch elements each).
Per core, every per-element scalar lives as an SBUF "plane" [128, 256] f32
(batch element e = partition*256 + free). All per-link parameters (trans,
rot_fix, mass, com, inertia, damping) are baked into the instruction stream
as immediates at build time.

The math is emitted through a small expression compiler:
  Expr = sum(coef * plane) + const
Linear combinations are free (term concatenation); they materialize as
chains of fused scalar_tensor_tensor ops ((in0*imm) + in1, in-place
accumulation) only when a product or an output needs a raw plane. Products
are DVE tensor_tensor ops. sin/cos and the final affine outputs go to the
scalar engine (ACT).

Algebraic structure used:
  - Rinv @ x with Rinv = Rz(q)^T @ rot_fix^T: constant matvec folded into
    stt-chain immediates, then one complex rotation (4 products).
  - cross(t_inv, Rinv x) = -Rinv (t x x): constant cross matrices fold into
    linear immediates, killing 12 products per forward joint.
  - forward state for joint 6 is never computed (the reference's backward
    recursion never reads it).
  - ACT Sin has no range reduction (accurate only on [-pi, pi]) and
    q ~ N(0,1) exceeds pi; half-angle identities avoid wrapping:
    cos q = 1-2sin^2(q/2), sin q = 2 sin(q/2)(1-2sin^2(q/4)).

Measured (8 cores, trn2): HW exec ~309 us, max abs err ~8e-5 on torque
absmax ~139 (fp32). Vector engine is the bottleneck (~650 DVE ops after
packing, from ~950); measured DVE op costs: 359 ns/[128,256] STT or TT
(267 ns data + ~92 ns SBUF-access/issue overhead), tensor_scalar 225 ns
(2x_2p), ACT 420 ns and fully parallel with DVE. Wide-op packing exploits
the per-op overhead plus term-count reduction:
  - forward quad-rot (K_QPACK): the 4 per-joint Rz^T rotations run as 4
    wide [P,4F] products against ACT-replicated trig tiles (slot scales
    absorb the mats' lead coefs; consts folded into slot contents via
    ACT-seeded exact chains) + 4 wide combines; outputs are pure planes.
  - backward child rotation (K_BROT): lf/af xy carried as RAW exprs,
    materialized into one [lf0|lf1|af0|af1] tile at the consuming joint,
    rotated with 4+2 wide [P,2F] ops.
  - backward crosses (K_XPACK): w, v, IcV_a, IcV_l land exactly in
    cyclically-duplicated [x0|x1|x2|x0|x1] tiles; each cross is 2 wide
    [P,3F] products + 1 wide subtract.
  - base joint computes only ang_f.z (everything else is dead there).
  - trig chains emitted lazily (joint j+2 with joint j's body) so the
    in-order ACT queue reaches each joint's replicated-trig copies just
    before the DVE needs them; front-loading all six chains costs ~5 us.
Rejected with on-HW measurements: TensorE accumulate (475 ns/[128,256]
matmul term, only ~50% overlap with DVE traffic), GPSIMD (SBUF ports
physically shared with DVE: 2-input ops serialize ~100%), bf16/fp16
(abs-err budget ~3e-3 at near-zero outputs vs ~1e-2+ rounding of O(30)
intermediates), and state-major layouts for PE matvecs (DVE free-dim
cycle cost explodes at <128 active partitions).
"""

import os
import sys

for _p in ("/opt/trn_rl_repo",):
    if os.path.isdir(_p) and _p not in sys.path:
        sys.path.append(_p)

import numpy as np

import concourse.bass as bass
import concourse.bacc as bacc
import concourse.mybir as mybir
from concourse import tile as tile_mod
from concourse import bass_utils

N_CORES = 8
ND = 7
B_TOTAL = 262144
BC = B_TOTAL // N_CORES  # 32768 per core
P = 128
F = BC // P  # 256
GRAV = 9.81
DT = mybir.dt.float32
DT_C = mybir.dt.bfloat16 if os.environ.get("K_BF16", "0") == "1" else mybir.dt.float32
OP = mybir.AluOpType
AF = mybir.ActivationFunctionType


class Expr:
    """value = sum(coef * plane_ap) + const"""

    __slots__ = ("terms", "const", "_mat")

    def __init__(self, terms=(), const=0.0):
        self.terms = list(terms)
        self.const = float(const)
        self._mat = None  # cached (coef, ap) of materialized sum-of-terms


ZERO = Expr()


def _nonzero(e):
    return bool(e.terms) or e.const != 0.0


class Builder:
    def __init__(self, nc, pool, ring_size=64, pool_frac=0.0,
                 pspool=None, wtile=None, wmax=0):
        self.nc = nc
        self.pool = pool
        self.n_stt = 0
        self.n_tt = 0
        self.n_act = 0
        self.n_copy = 0
        self.n_mm = 0
        # engine load balancing between DVE and GPSIMD (2-input ops)
        self.eng_busy = [0.0, 0.0]  # ns accumulated: [vector, gpsimd]
        self.eng_cost = [445.0, 980.0]
        self.pool_frac = pool_frac
        self.ring_size = ring_size
        self.ring_idx = 0
        self.joint_allocs = 0
        self.max_joint_allocs = 0
        self.pers_idx = 0
        self.free_tags = []       # recycled persistent tags
        self.free_wide = {}       # recycled wide tags by slot count
        self.pers_ids = set()     # id(ap) of planes safe to reference long-term
        self.ap_tag = {}          # id(ap) -> tag (for freeing)
        # ---- PE (TensorEngine) chain offload ----
        # Constant-coefficient linear combinations accumulate in PSUM via
        # diag-weight fp32 matmuls (exact: HW does the 4-pass H/L split).
        # Measured: ~490ns/term sustained incl. per-MM LDWEIGHTS, fully
        # parallel with DVE; DVE reads PSUM operands at SBUF cost.
        self.use_pe = wtile is not None and os.environ.get("K_PE", "1") == "1"
        self.pspool = pspool
        self.wtile = wtile
        self.wmax = wmax
        self.wave = 0
        self.wave_coefs = [[]]    # per wave: slot i holds diag(coefs[i])
        self.wslot = {}
        self.w_d = None           # wconst dram tensor (for wave refills)
        self.ps_ring = 0
        self.ps_ntags = int(os.environ.get("K_PSTAGS", "8"))
        # projected engine busy (ns) for greedy makespan routing
        self.busy = {"dve": 0.0, "act": 0.0, "pe": 0.0}
        self.pe_cost = float(os.environ.get("K_PE_COST", "500"))
        self.dve_cost = 424.0
        self.act_evac = 480.0
        self.dve_evac = 258.0

    def dve_track(self, n_ops, width=1):
        self.busy["dve"] += n_ops * (width * 267.0 + 157.0)

    @property
    def whalf(self):
        return self.wmax // 2

    def _wslot_room(self, coefs):
        cur = self.wave_coefs[self.wave]
        new = {c for c in coefs if c not in self.wslot}
        return len(cur) + len(new) <= self.whalf

    def _wslot_ap(self, c):
        s = self.wslot.get(c)
        if s is None:
            cur = self.wave_coefs[self.wave]
            s = len(cur)
            cur.append(c)
            self.wslot[c] = s
        base = (self.wave % 2) * self.whalf
        return self.wtile[:, (base + s) * P:(base + s + 1) * P]

    def _wave_dma(self):
        """DMA this wave's coefficient table into its half of the weight
        tile. Waves ping-pong between halves, so wave k+1's DMA overlaps
        wave k's LDWEIGHTS reads instead of stalling on them."""
        dram_base = self.wave * self.whalf * P
        sb_base = (self.wave % 2) * self.whalf * P
        wq = [self.nc.sync, self.nc.gpsimd] * 3
        nchunk = len(wq)
        bounds = [round(i * self.whalf / nchunk) for i in range(nchunk + 1)]
        for ci in range(nchunk):
            a, bnd = bounds[ci] * P, bounds[ci + 1] * P
            if a < bnd:
                wq[ci].dma_start(self.wtile[:, sb_base + a:sb_base + bnd],
                                 self.w_d.ap()[:, dram_base + a:dram_base + bnd])

    def new_wave(self):
        """Advance to a fresh coefficient table half."""
        if not self.use_pe or self.wave + 1 >= self.n_waves:
            return
        self.wave += 1
        self.wave_coefs.append([])
        self.wslot = {}
        self._wave_dma()

    def _psum(self):
        """PSUM chain slot. Banks are the allocation granularity (8), so two
        chain slots share each bank tile (has_written bits are per-element,
        so independent accumulation groups coexist in one bank)."""
        if os.environ.get("K_PSHALF", "0") == "1":
            n = self.ps_ring
            self.ps_ring += 1
            half = n % 2
            if half == 0:
                tag = f"ps{(n // 2) % self.ps_ntags}"
                self._ps_cur = self.pspool.tile([P, 2 * F], DT, tag=tag,
                                                name=tag)
            return self._ps_cur[:, half * F:(half + 1) * F]
        tag = f"ps{self.ps_ring % self.ps_ntags}"
        self.ps_ring += 1
        t = self.pspool.tile([P, 2 * F], DT, tag=tag, name=tag)
        return t[:, 0:F]

    def _pe_accum(self, terms):
        """PSUM <- sum(c*ap) over SBUF-resident terms via diag-weight MMs."""
        ps = self._psum()
        n = len(terms)
        for i, (c, ap) in enumerate(terms):
            w = self._wslot_ap(float(c))
            self.nc.tensor.matmul(ps, w, ap, start=(i == 0), stop=(i == n - 1))
        self.n_mm += n
        self.busy["pe"] += n * self.pe_cost
        return ps

    def _pe_split(self, terms):
        """(pe_terms, dve_terms, sigma) under space/slot constraints, or None.

        sigma flips the whole accumulation's sign when -c slots are a better
        match for the live weight table (the flip is undone at evac / in the
        returned coefficient), halving slot burn for +-c coefficient pairs.
        """
        if not self.use_pe:
            return None
        pe, dve = [], []
        for c, ap in terms:
            if ap.space == bass.MemorySpace.SBUF:
                pe.append((float(c), ap))
            else:
                dve.append((c, ap))
        if len(pe) < 2:
            return None
        hits_p = sum(1 for c, _ in pe if c in self.wslot)
        hits_n = sum(1 for c, _ in pe if -c in self.wslot)
        sigma = -1.0 if hits_n > hits_p else 1.0
        pe = [(sigma * c, ap) for c, ap in pe]
        if not self._wslot_room([c for c, _ in pe]):
            return None
        return pe, dve, sigma

    def _evac(self, dst, ps, scale, bias, dve_pref=False):
        ca = self.busy["act"] + self.act_evac
        cd = self.busy["dve"] + self.dve_evac
        mode = os.environ.get("K_EVAC", "auto")
        if mode == "act":
            dve_pref = False
            cd = ca + 1.0
        elif mode == "dve":
            dve_pref = True
        if dve_pref or cd < ca:
            self.nc.vector.tensor_scalar(dst, ps, float(scale), float(bias),
                                         OP.mult, OP.add)
            self.busy["dve"] = cd
            self.n_stt += 1
        else:
            self.nc.scalar.activation(dst, ps, AF.Copy, bias=float(bias),
                                      scale=float(scale))
            self.busy["act"] = ca
            self.n_act += 1

    def _pe_chain(self, terms, dst, scale=1.0, bias=0.0, dve_seed=False,
                  seed_act=True):
        """Try PE route for sum(c*ap)*scale(+bias). Returns result ap or None.

        dst None: result stays in PSUM (requires scale==1, bias==0 handled by
        caller convention). dst given: evacuated into dst (SBUF).
        Split chains merge the DVE-side terms into dst with the PSUM partial
        as a free STT src1 operand.
        """
        limit = int(os.environ.get("K_PE_LIMIT", "100000"))
        if getattr(self, "pe_chains", 0) >= limit:
            return None
        split = self._pe_split(terms)
        if split is None:
            return None
        pe, dve, sigma = split
        n_pe, n_dve = len(pe), len(dve)
        # status quo: whole chain on DVE (+ACT seed when it would use one)
        sq_d = self.busy["dve"] + (len(terms) - 1) * self.dve_cost
        sq_a = self.busy["act"] + (self.act_evac if seed_act else 0.0)
        mk_sq = max(sq_d, sq_a, self.busy["pe"])
        ev = 0.0 if (dst is None and n_dve == 0) else \
            (0.0 if n_dve else min(self.act_evac, self.dve_evac))
        pe_d = self.busy["dve"] + n_dve * self.dve_cost
        pe_p = self.busy["pe"] + n_pe * self.pe_cost
        mk_pe = max(pe_d, self.busy["act"], pe_p) + ev
        if mk_pe >= mk_sq:
            return None
        self.pe_chains = getattr(self, "pe_chains", 0) + 1
        if dst is None and n_dve:
            # merge would need an SBUF dst anyway; use scratch
            dst_eff = self.scratch()
        else:
            dst_eff = dst
        ps = self._pe_accum(pe)  # holds sigma * sum(c_i x_i) over pe terms
        if n_dve == 0:
            if dst_eff is None:
                return ps, sigma
            self._evac(dst_eff, ps, sigma * scale, bias, dve_pref=dve_seed)
            return dst_eff, 1.0
        # DVE merge. The leftover (non-SBUF) terms are PSUM-resident, so the
        # partial must leave PSUM first (one PSUM operand per DVE op).
        self.nc.vector.tensor_scalar(dst_eff, ps, float(sigma * scale),
                                     float(bias), OP.mult, OP.add)
        self.n_stt += 1
        self.busy["dve"] += self.dve_evac
        for ck, xk in dve:
            self.nc.vector.scalar_tensor_tensor(
                dst_eff, xk, float(ck * scale), dst_eff, OP.mult, OP.add)
            self.n_stt += 1
            self.busy["dve"] += self.dve_cost
        return dst_eff, 1.0

    def pick_engine(self, n_ops=1):
        """Pick vector or gpsimd for a chain of n_ops 2-input ops."""
        self.busy["dve"] += n_ops * self.dve_cost
        if self.pool_frac <= 0.0:
            self.eng_busy[0] += n_ops * self.eng_cost[0]
            return self.nc.vector
        c0 = self.eng_busy[0] + n_ops * self.eng_cost[0]
        c1 = self.eng_busy[1] + n_ops * self.eng_cost[1]
        if c1 < c0:
            self.eng_busy[1] = c1
            return self.nc.gpsimd
        self.eng_busy[0] = c0
        return self.nc.vector

    # ---- allocation ----
    def scratch(self, dtype=None):
        tag = f"s{self.ring_idx % self.ring_size}"
        t = self.pool.tile([P, F], dtype or DT_C, tag=tag, name=tag)
        self.ring_idx += 1
        self.joint_allocs += 1
        return t[:, :]

    def persistent(self, label=""):
        if self.free_tags:
            tag = self.free_tags.pop()
        else:
            tag = f"p{self.pers_idx}"
            self.pers_idx += 1
        t = self.pool.tile([P, F], DT_C, tag=tag, name=f"{tag}_{label}")
        ap = t[:, :]
        self.pers_ids.add(id(ap))
        self.ap_tag[id(ap)] = ("p", tag)
        return ap

    def wscratch(self, slots):
        """Wide scratch tile [P, slots*F] from a per-width ring."""
        if not hasattr(self, "wring"):
            self.wring = {}
        idx = self.wring.get(slots, 0)
        self.wring[slots] = idx + 1
        mod = {2: 2, 3: 5, 4: 8}.get(slots, 10)
        tag = f"w{slots}_{idx % mod}"
        t = self.pool.tile([P, slots * F], DT_C, tag=tag, name=tag)
        return t[:, :]

    def persistent_wide(self, slots, label=""):
        if not hasattr(self, "pwide_idx"):
            self.pwide_idx = 0
        fl = self.free_wide.setdefault(slots, [])
        if fl:
            tag = fl.pop()
        else:
            tag = f"pw{slots}_{self.pwide_idx}"
            self.pwide_idx += 1
        t = self.pool.tile([P, slots * F], DT_C, tag=tag, name=f"{tag}_{label}")
        return t[:, :], tag

    def packed_pair_rot(self, Y0, Y1, t4, sign, dst0, dst1, pers,
                        A_ready=None):
        """Rotate one (y0, y1) pair by the z-angle whose 4-slot trig tile is
        t4 = [cos | sin/2 | -sin/2 | cos].

        sign=+1: z0 = c y0 + s y1, z1 = -s y0 + c y1   (Rz^T)
        sign=-1: z0 = c y0 - s y1, z1 = +s y0 + c y1   (Rz)
        dst0/dst1: [P, F] planes receiving z0/a0, z1/a1.
        A_ready: optional (A_region[P,2F], a0, a1, k0, k1) when the pair is
        already materialized adjacently (coefs a*, deferred consts k*).
        Returns (z0_expr, z1_expr) incl. rotated deferred-const terms.
        """
        if A_ready is None:
            W = self.wscratch(2)
            a0, _ = self.mat(Y0, dst=W[:, 0:F])
            a1, _ = self.mat(Y1, dst=W[:, F:2 * F])
            k0, k1 = Y0.const, Y1.const
            A = W
        else:
            A, a0, a1, k0, k1 = A_ready
        c_slot = t4[:, 0:F]
        s_slot = t4[:, F:2 * F]
        if sign > 0:
            B1 = t4[:, 0:2 * F]            # [c | s/2]
            B2 = t4[:, 2 * F:4 * F]        # [-s/2 | c]
            # P1 = [c*y0r | (s/2)*y1r]; P2 = [(-s/2)*y0r | c*y1r]
            # z0 = a0*P1s0 + 2 a1*P1s1 ; z1 = 2 a0*P2s0 + a1*P2s1
        else:
            t4r = t4.rearrange("p (a c b) -> p a c b", a=2, c=2, b=F)
            B1 = t4r[:, :, 0, :]           # [c | -s/2]
            B2 = t4r[:, :, 1, :]           # [s/2 | c]
            # P1 = [c*y0r | (-s/2)*y1r]: z0 = a0 P1s0 + 2 a1 P1s1
            # P2 = [(s/2)*y0r | c*y1r]:  z1 = 2 a0 P2s0 + a1 P2s1
        P1 = self.wscratch(2)
        P2 = self.wscratch(2)
        self.nc.vector.tensor_tensor(P1, A, B1, OP.mult)
        self.nc.vector.tensor_tensor(P2, A, B2, OP.mult)
        self.n_tt += 2
        self.nc.vector.scalar_tensor_tensor(
            dst0, P1[:, F:2 * F], 2.0 * a1 / a0, P1[:, 0:F], OP.mult, OP.add)
        self.nc.vector.scalar_tensor_tensor(
            dst1, P2[:, 0:F], 2.0 * a0 / a1, P2[:, F:2 * F], OP.mult, OP.add)
        self.n_stt += 2
        if pers:
            self.pers_ids.add(id(dst0))
            self.pers_ids.add(id(dst1))
        t0 = [(a0, dst0)]
        t1 = [(a1, dst1)]
        if k0 != 0.0:
            t0.append((k0, c_slot))
            t1.append((-sign * 2.0 * k0, s_slot))
        if k1 != 0.0:
            t0.append((sign * 2.0 * k1, s_slot))
            t1.append((k1, c_slot))
        e0 = Expr(t0)
        e1 = Expr(t1)
        if len(t0) == 1:
            e0._mat = (a0, dst0)
        if len(t1) == 1:
            e1._mat = (a1, dst1)
        return e0, e1

    def act_copy(self, dst, src, scale=1.0, bias=0.0):
        self.nc.scalar.activation(dst, src, AF.Copy, bias=float(bias),
                                  scale=float(scale))
        self.n_act += 1
        self.busy["act"] += 480.0

    def mat_exact(self, e, dst, dve_seed=False):
        """Materialize the EXACT value of e into dst (coef 1, const folded).

        Unlike mat(), the result plane holds sum(coef*plane)+const verbatim,
        so packed slot-aligned products across different slots stay
        coefficient-consistent. Costs the same n-1 STT as mat(); a leading
        ACT copy (idle engine) absorbs the lead coef and the constant when
        no unit-coefficient lead term exists.
        """
        terms = sorted(e.terms, key=lambda t: -abs(t[0]))
        if not terms:
            self.nc.vector.memset(dst, float(e.const))
            return
        const = e.const
        unit = next((i for i, (ck, _) in enumerate(terms)
                     if ck == 1.0), None)
        if len(terms) >= 2 and self._pe_chain(
                terms, dst, 1.0, const, dve_seed=dve_seed,
                seed_act=(unit is None or const != 0.0) and not dve_seed,
                ) is not None:
            return
        if len(terms) == 1:
            c0, x0 = terms[0]
            if dve_seed:
                self.nc.vector.tensor_scalar(
                    dst, x0, float(c0), const, OP.mult, OP.add)
                self.n_stt += 1
            else:
                self.act_copy(dst, x0, scale=c0, bias=const)
            return
        if unit is not None and const == 0.0:
            c0, x0 = terms.pop(unit)
            ck, xk = terms.pop(0)
            self.nc.vector.scalar_tensor_tensor(
                dst, xk, float(ck), x0, OP.mult, OP.add)
            self.n_stt += 1
        else:
            c0, x0 = terms.pop(0)
            if dve_seed:
                self.nc.vector.tensor_scalar(
                    dst, x0, float(c0), const, OP.mult, OP.add)
                self.n_stt += 1
            else:
                self.act_copy(dst, x0, scale=c0, bias=const)
        for ck, xk in terms:
            self.nc.vector.scalar_tensor_tensor(
                dst, xk, float(ck), dst, OP.mult, OP.add)
            self.n_stt += 1

    def cyc_tile(self, vec, label, dve_seed=False):
        """[P,5F] tile holding [x0|x1|x2|x0|x1] of a 3-vector of Exprs."""
        t, tag = self.persistent_wide(5, label)
        for k in range(3):
            self.mat_exact(vec[k], t[:, k * F:(k + 1) * F], dve_seed=dve_seed)
        if dve_seed:
            self.nc.vector.tensor_scalar(
                t[:, 3 * F:5 * F], t[:, 0:2 * F], 1.0, None, OP.mult)
            self.n_stt += 1
        else:
            self.act_copy(t[:, 3 * F:4 * F], t[:, 0:F])
            self.act_copy(t[:, 4 * F:5 * F], t[:, F:2 * F])
        return t, tag

    def mat_rel(self, e, dst, dve_seed=False):
        """Materialize (value incl. const)/c0 into dst; returns c0.

        Like mat() but the constant is folded into the plane (via an ACT
        seed copy on the idle scalar engine), so rotating the plane rotates
        the full affine value and downstream exprs stay pure. dve_seed=True
        seeds with a DVE tensor_scalar instead: at the backward tail the
        ACT queue latency sits on the critical path (DVE has nothing left
        to overlap), so cross-engine seeding there costs ~0.5us per chain.
        """
        assert e.terms
        terms = sorted(e.terms, key=lambda t: -abs(t[0]))
        c0, x0 = terms[0]
        if len(terms) >= 2 and self._pe_chain(
                terms, dst, 1.0 / c0, e.const / c0, dve_seed=dve_seed,
                seed_act=(e.const != 0.0) and not dve_seed) is not None:
            return c0
        if e.const != 0.0 or len(terms) == 1:
            if dve_seed:
                self.nc.vector.tensor_scalar(
                    dst, x0, 1.0, e.const / c0, OP.mult, OP.add)
                self.n_stt += 1
            else:
                self.act_copy(dst, x0, scale=1.0, bias=e.const / c0)
            rest = terms[1:]
        else:
            c1, x1 = terms[1]
            self.nc.vector.scalar_tensor_tensor(
                dst, x1, c1 / c0, x0, OP.mult, OP.add)
            self.n_stt += 1
            rest = terms[2:]
        for ck, xk in rest:
            self.nc.vector.scalar_tensor_tensor(
                dst, xk, ck / c0, dst, OP.mult, OP.add)
            self.n_stt += 1
        return c0

    def quad_rot(self, pairs, c_ap, s_ap, label):
        """Rz^T-rotate W (Y0,Y1) expr pairs sharing one joint angle.

        z0 = c Y0 + s Y1 ; z1 = -s Y0 + c Y1. s_ap holds sin/2 (the 2x is
        folded into the replicated-trig scales). Y consts fold into slot
        contents (mat_rel), so outputs are pure planes. Slot coefficients
        fold into the per-slot scales of ACT-replicated trig tiles, letting
        each product group and each combine run as ONE wide DVE op.
        Returns (z0_exprs, z1_exprs, persistent_tags): z0/z1 of pairs [0,1]
        land in persistent [P,2F] tiles (joint states w/v); pairs [2,3] in
        scratch (consumed by the same joint's dw/dv).
        """
        W = len(pairs)
        assert W == 4
        A = self.wscratch(W)
        Bt = self.wscratch(W)
        CA = self.wscratch(W)
        SB = self.wscratch(W)
        SmA = self.wscratch(W)
        CB = self.wscratch(W)
        aA, aB = [], []
        for k, (y0, y1) in enumerate(pairs):
            sl = slice(k * F, (k + 1) * F)
            aA.append(self.mat_rel(y0, A[:, sl]))
            aB.append(self.mat_rel(y1, Bt[:, sl]))
        for k in range(W):
            sl = slice(k * F, (k + 1) * F)
            self.act_copy(CA[:, sl], c_ap, scale=aA[k])
            self.act_copy(SB[:, sl], s_ap, scale=2.0 * aB[k])
            self.act_copy(SmA[:, sl], s_ap, scale=-2.0 * aA[k])
            self.act_copy(CB[:, sl], c_ap, scale=aB[k])
        v = self.nc.vector
        P1 = self.wscratch(W)
        P2 = self.wscratch(W)
        v.tensor_tensor(P1, A, CA, OP.mult)
        v.tensor_tensor(P2, Bt, SB, OP.mult)
        ZA1, t1 = self.persistent_wide(2, f"{label}a1")
        ZB1 = self.wscratch(2)
        v.tensor_tensor(ZA1, P1[:, 0:2 * F], P2[:, 0:2 * F], OP.add)
        v.tensor_tensor(ZB1, P1[:, 2 * F:4 * F], P2[:, 2 * F:4 * F], OP.add)
        v.tensor_tensor(P1, A, SmA, OP.mult)
        v.tensor_tensor(P2, Bt, CB, OP.mult)
        ZA2, t2 = self.persistent_wide(2, f"{label}a2")
        ZB2 = self.wscratch(2)
        v.tensor_tensor(ZA2, P1[:, 0:2 * F], P2[:, 0:2 * F], OP.add)
        v.tensor_tensor(ZB2, P1[:, 2 * F:4 * F], P2[:, 2 * F:4 * F], OP.add)
        self.n_tt += 8
        self.dve_track(4, 4)
        self.dve_track(4, 2)

        def _mk(region, k):
            ap = region[:, k * F:(k + 1) * F]
            self.pers_ids.add(id(ap))
            e = Expr([(1.0, ap)])
            e._mat = (1.0, ap)
            return e

        z0s = [_mk(ZA1, 0), _mk(ZA1, 1), _mk(ZB1, 0), _mk(ZB1, 1)]
        z1s = [_mk(ZA2, 0), _mk(ZA2, 1), _mk(ZB2, 0), _mk(ZB2, 1)]
        return z0s, z1s, (t1, t2)

    def pair_rot_bwd(self, lf01, af01, c_ap, s_ap, label, dve_seed=False):
        """Rz-rotate (lf0,lf1) and (af0,af1): z0 = c y0 - s y1; z1 = s y0 + c y1.

        Raw force exprs land exactly (consts folded, lead coefs in the
        ACT-replicated trig scales) in one [lf0|lf1|af0|af1] tile; the two
        rotations then cost 4 wide products + 2 wide combines on [P,2F]
        instead of 8 narrow products + 4 chain materializations.
        Returns ((RzLf0e, RzLf1e), (RzAf0e, RzAf1e)) as pure plane exprs.
        """
        T = self.wscratch(4)
        coefs = []
        for k, e in enumerate([lf01[0], lf01[1], af01[0], af01[1]]):
            coefs.append(self.mat_rel(e, T[:, k * F:(k + 1) * F],
                                      dve_seed=dve_seed))
        aL0, aL1, aA0, aA1 = coefs
        Tr = T.rearrange("p (a b f) -> p a b f", a=2, b=2, f=F)
        Av = Tr[:, :, 0, :]   # [lf0 | af0]
        Bv = Tr[:, :, 1, :]   # [lf1 | af1]
        R1 = self.wscratch(4)  # [c*aL0 | c*aA0 | -s*aL1 | -s*aA1]
        self.act_copy(R1[:, 0:F], c_ap, scale=aL0)
        self.act_copy(R1[:, F:2 * F], c_ap, scale=aA0)
        self.act_copy(R1[:, 2 * F:3 * F], s_ap, scale=-2.0 * aL1)
        self.act_copy(R1[:, 3 * F:4 * F], s_ap, scale=-2.0 * aA1)
        R2 = self.wscratch(4)  # [s*aL0 | s*aA0 | c*aL1 | c*aA1]
        self.act_copy(R2[:, 0:F], s_ap, scale=2.0 * aL0)
        self.act_copy(R2[:, F:2 * F], s_ap, scale=2.0 * aA0)
        self.act_copy(R2[:, 2 * F:3 * F], c_ap, scale=aL1)
        self.act_copy(R2[:, 3 * F:4 * F], c_ap, scale=aA1)
        v = self.nc.vector
        Pt = self.wscratch(4)
        Z = self.wscratch(4)
        v.tensor_tensor(Pt[:, 0:2 * F], Av, R1[:, 0:2 * F], OP.mult)
        v.tensor_tensor(Pt[:, 2 * F:4 * F], Bv, R1[:, 2 * F:4 * F], OP.mult)
        v.tensor_tensor(Z[:, 0:2 * F], Pt[:, 0:2 * F], Pt[:, 2 * F:4 * F],
                        OP.add)
        v.tensor_tensor(Pt[:, 0:2 * F], Av, R2[:, 0:2 * F], OP.mult)
        v.tensor_tensor(Pt[:, 2 * F:4 * F], Bv, R2[:, 2 * F:4 * F], OP.mult)
        v.tensor_tensor(Z[:, 2 * F:4 * F], Pt[:, 0:2 * F], Pt[:, 2 * F:4 * F],
                        OP.add)
        self.n_tt += 6
        self.dve_track(6, 2)

        def _e(sl):
            ap = Z[:, sl * F:(sl + 1) * F]
            self.pers_ids.add(id(ap))
            e = Expr([(1.0, ap)])
            e._mat = (1.0, ap)
            return e

        return (_e(0), _e(2)), (_e(1), _e(3))

    def cross_packed(self, cycA, cycB):
        """cross(A, B) of two cyc tiles -> [P,3F] tile [c0|c1|c2]."""
        Pa = self.wscratch(3)
        Pb = self.wscratch(3)
        self.nc.vector.tensor_tensor(
            Pa, cycA[:, F:4 * F], cycB[:, 2 * F:5 * F], OP.mult)
        self.nc.vector.tensor_tensor(
            Pb, cycA[:, 2 * F:5 * F], cycB[:, F:4 * F], OP.mult)
        self.nc.vector.tensor_tensor(Pa, Pa, Pb, OP.subtract)
        self.n_tt += 3
        self.dve_track(3, 3)
        return Pa

    def free_expr_vec(self, vec):
        for e in vec:
            for _, ap in e.terms:
                ent = self.ap_tag.pop(id(ap), None)
                if ent is None:
                    continue
                self.pers_ids.discard(id(ap))
                if ent[0] == "p":
                    self.free_tags.append(ent[1])
                else:
                    self.free_wide.setdefault(ent[1], []).append(ent[2])

    def joint_boundary(self):
        self.max_joint_allocs = max(self.max_joint_allocs, self.joint_allocs)
        self.joint_allocs = 0

    # ---- expression ops ----
    def lin(self, *pairs, const=0.0):
        acc = {}
        aps = {}
        c_acc = float(const)
        for coef, e in pairs:
            if coef == 0.0 or e is None or e is ZERO and e.const == 0.0:
                if e is not None:
                    c_acc += coef * e.const
                continue
            c_acc += coef * e.const
            for tc, ap in e.terms:
                k = id(ap)
                acc[k] = acc.get(k, 0.0) + coef * tc
                aps[k] = ap
        terms = [(c, aps[k]) for k, c in acc.items() if c != 0.0]
        return Expr(terms, c_acc)

    def mat(self, e, dst=None):
        """Materialize sum-of-terms: e == coef*ap + e.const -> (coef, ap)."""
        assert e.terms, "cannot materialize empty expr"
        if e._mat is not None and dst is None:
            return e._mat
        terms = sorted(e.terms, key=lambda t: -abs(t[0]))
        if len(terms) == 1 and dst is None:
            e._mat = (terms[0][0], terms[0][1])
            return e._mat
        c0, x0 = terms[0]
        if len(terms) == 1:
            self.nc.vector.tensor_scalar(dst, x0, 1.0, None, OP.mult)
            self.n_copy += 1
            e._mat = (c0, dst)
            return e._mat
        if not hasattr(self, "mat_hist"):
            self.mat_hist = {}
        self.mat_hist[len(terms)] = self.mat_hist.get(len(terms), 0) + 1
        r = self._pe_chain(terms, dst, 1.0, 0.0, seed_act=False)
        if r is not None:
            e._mat = (r[1], r[0])
            return e._mat
        t = dst if dst is not None else self.scratch()
        c1, x1 = terms[1]
        eng = self.pick_engine(len(terms) - 1)
        eng.scalar_tensor_tensor(t, x1, c1 / c0, x0, OP.mult, OP.add)
        self.n_stt += 1
        for ck, xk in terms[2:]:
            eng.scalar_tensor_tensor(t, xk, ck / c0, t, OP.mult, OP.add)
            self.n_stt += 1
        e._mat = (c0, t)
        return e._mat

    def mul(self, x, y):
        if not _nonzero(x) or not _nonzero(y):
            return ZERO
        if not x.terms:  # pure const
            return Expr([(x.const * c, ap) for c, ap in y.terms],
                        x.const * y.const)
        if not y.terms:
            return Expr([(y.const * c, ap) for c, ap in x.terms],
                        x.const * y.const)
        cx, ax = self.mat(x)
        cy, ay = self.mat(y)
        if ax.space == bass.MemorySpace.PSUM \
                and ay.space == bass.MemorySpace.PSUM:
            tmp = self.scratch()
            self.nc.vector.tensor_scalar(tmp, ay, 1.0, None, OP.mult)
            self.n_stt += 1
            self.busy["dve"] += self.dve_evac
            ay = tmp
        prod = self.scratch()
        self.pick_engine(1).tensor_tensor(prod, ax, ay, OP.mult)
        self.n_tt += 1
        terms = [(cx * cy, prod)]
        if y.const != 0.0:
            terms.append((cx * y.const, ax))
        if x.const != 0.0:
            terms.append((cy * x.const, ay))
        return Expr(terms, x.const * y.const)

    def snap(self, e, label="", scratch_ok=False):
        """Materialize into a stable plane; returns single-term Expr."""
        if not e.terms:
            return e
        if len(e.terms) == 1 and e._mat is None \
                and id(e.terms[0][1]) in self.pers_ids and not scratch_ok:
            out = Expr(list(e.terms), e.const)
            out._mat = e.terms[0]
            return out
        if e._mat is not None:
            c, src = e._mat
            if id(src) in self.pers_ids or scratch_ok:
                out = Expr([(c, src)], e.const)
                out._mat = (c, src)
                return out
            dst = self.persistent(label)
            self.nc.scalar.activation(dst, src, AF.Copy, bias=0.0, scale=1.0)
            self.n_copy += 1
            out = Expr([(c, dst)], e.const)
            out._mat = (c, dst)
            return out
        dst = self.scratch() if scratch_ok else self.persistent(label)
        c, ap = self.mat(e, dst=dst)
        out = Expr([(c, ap)], e.const)
        out._mat = (c, ap)
        return out

    def snap_to(self, e, dst_ap):
        """Materialize into the given plane; returns single-term Expr."""
        assert e.terms
        c, ap = self.mat(e, dst=dst_ap)
        self.pers_ids.add(id(ap))
        out = Expr([(c, ap)], e.const)
        out._mat = (c, ap)
        return out

    def snap_vec(self, vec, label="", scratch_ok=False):
        return [self.snap(e, f"{label}{i}", scratch_ok) for i, e in enumerate(vec)]

    # ---- 3-vector helpers ----
    def vadd(self, *vecs):
        return [self.lin(*[(1.0, v[i]) for v in vecs]) for i in range(3)]

    def vsub(self, a, b):
        return [self.lin((1.0, a[i]), (-1.0, b[i])) for i in range(3)]

    def cross_const(self, t, X):
        return [
            self.lin((-t[2], X[1]), (t[1], X[2])),
            self.lin((t[2], X[0]), (-t[0], X[2])),
            self.lin((-t[1], X[0]), (t[0], X[1])),
        ]

    def cross_ee(self, A, B):
        return [
            self.lin((1.0, self.mul(A[1], B[2])), (-1.0, self.mul(A[2], B[1]))),
            self.lin((1.0, self.mul(A[2], B[0])), (-1.0, self.mul(A[0], B[2]))),
            self.lin((1.0, self.mul(A[0], B[1])), (-1.0, self.mul(A[1], B[0]))),
        ]

    def matvec_const(self, M, X):
        return [
            self.lin((M[i, 0], X[0]), (M[i, 1], X[1]), (M[i, 2], X[2]))
            for i in range(3)
        ]


def build_module(params):
    trans = np.asarray(params["trans"], np.float64)
    rot_fix = np.asarray(params["rot_fix"], np.float64)
    mass = np.asarray(params["mass"], np.float64)
    com = np.asarray(params["com"], np.float64)
    inertia = np.asarray(params["inertia"], np.float64)
    damping = np.asarray(params["damping"], np.float64)

    nc = bacc.Bacc("TRN2", target_bir_lowering=False, debug=False,
                   num_devices=N_CORES)
    q_d = nc.dram_tensor("q", (BC, ND), DT, kind="ExternalInput")
    qd_d = nc.dram_tensor("qd", (BC, ND), DT, kind="ExternalInput")
    qdd_d = nc.dram_tensor("qdd_des", (BC, ND), DT, kind="ExternalInput")
    use_pe = os.environ.get("K_PE", "1") == "1"
    wmax = int(os.environ.get("K_WMAX", "46"))
    n_waves = int(os.environ.get("K_NWAVES", "9"))
    if use_pe:
        w_d = nc.dram_tensor("wconst", (P, n_waves * (wmax // 2) * P), DT,
                             kind="ExternalInput")
    tq_d = nc.dram_tensor("torque", (BC, ND), DT, kind="ExternalOutput")

    with tile_mod.TileContext(nc) as tc:
        with tc.tile_pool(name="main", bufs=1) as pool, \
             tc.tile_pool(name="io", bufs=1) as io_pool, \
             tc.psum_pool(name="psp", bufs=1) as pspool:
            wtile_ap = None
            if use_pe:
                wtile = io_pool.tile([P, wmax * P], DT, tag="wconst",
                                     name="wconst_sb")
                wtile_ap = wtile[:, :]
            b = Builder(nc, pool,
                        ring_size=int(os.environ.get("K_RING", "11")),
                        pool_frac=float(os.environ.get("K_POOL_FRAC", "0")),
                        pspool=pspool, wtile=wtile_ap, wmax=wmax)
            b.n_waves = n_waves
            if use_pe:
                b.w_d = w_d

            q_t = io_pool.tile([P, F * ND], DT, tag="q", name="q_sb")
            qd_t = io_pool.tile([P, F * ND], DT, tag="qd", name="qd_sb")
            qdd_t = io_pool.tile([P, F * ND], DT, tag="qdd", name="qdd_sb")
            # out aliases q's buffer: q is fully consumed by the trig
            # prologue long before the first backward write_out.
            out_t = io_pool.tile([P, F * ND], DT, tag="q", name="out_sb")
            dram_view = lambda t: t.ap().rearrange("(p f) d -> p (f d)", p=P)
            # q gates trig (the whole critical path): give it the sync queue
            # alone; qd/qdd ride the idle gpsimd queue in parallel.
            nc.sync.dma_start(q_t[:, :], dram_view(q_d))
            nc.gpsimd.dma_start(qd_t[:, :], dram_view(qd_d))
            nc.gpsimd.dma_start(qdd_t[:, :], dram_view(qdd_d))
            if use_pe:
                # weight slots are allocated in first-use (= emission) order,
                # so chunked DMA in slot order arrives before consumers.
                b._wave_dma()

            q3 = q_t[:, :].rearrange("p (f d) -> p f d", d=ND)
            qd3 = qd_t[:, :].rearrange("p (f d) -> p f d", d=ND)
            qdd3 = qdd_t[:, :].rearrange("p (f d) -> p f d", d=ND)
            out3 = out_t[:, :].rearrange("p (f d) -> p f d", d=ND)

            def as_pers_expr(ap):
                b.pers_ids.add(id(ap))
                e = Expr([(1.0, ap)])
                e._mat = (1.0, ap)
                return e

            def const_col(tag, val):
                t = io_pool.tile([P, 1], DT, tag=tag, name=tag)
                nc.vector.memset(t[:, :], float(val))
                return t

            zero_t = const_col("zconst", 0.0)
            # dummy Sin on an immediately-ready tile: hoists the ~2.7us ACT
            # table load to t~0, overlapping the input DMA instead of
            # serializing after it.
            warm_t = io_pool.tile([P, 1], DT, tag="warm", name="warm")
            nc.scalar.activation(warm_t[:, :], zero_t[:, :], AF.Sin,
                                 bias=zero_t[:, :], scale=1.0)

            # ACT Sin has no range reduction (accurate only on [-pi, pi]).
            # q ~ N(0,1) so |q| <= ~5.5: one conditional wrap of 2*pi covers
            # sin(q) and cos(q) = sin((q + pi/2) wrapped).
            PI = float(np.pi)
            TWO_PI = float(2 * np.pi)
            trig = {}
            trig_raw = {}
            _kp = os.environ.get("K_PACK", "0")
            use_pack_fwd = _kp in ("1", "fwd")
            use_pack_bwd = _kp in ("1", "bwd")
            use_xpack = os.environ.get("K_XPACK", "1") == "1"
            def emit_trig(j):
                # ACT Sin is only accurate on [-pi, pi] (no range reduction)
                # and q ~ N(0,1) exceeds pi. Half-angle identities avoid any
                # wrapping: |q/2| <= ~2.9 < pi and |q/4| < pi/2 always.
                #   cos q = 1 - 2 sin^2(q/2)
                #   sin q = 2 sin(q/2) cos(q/2),  cos(q/2) = 1 - 2 sin^2(q/4)
                # Everything runs on the otherwise-idle ACT engine except one
                # DVE product for sin. Emitted lazily (joint j+1's chain goes
                # out with joint j's body) so the in-order ACT queue reaches
                # each joint's replicated-trig copies just before the DVE
                # needs them, instead of front-loading all six chains.
                if j in trig:
                    return
                qj = q3[:, :, j]
                t4w = 4 if (use_pack_fwd or use_pack_bwd) else 2
                t4, _t4tag = b.persistent_wide(t4w, f"t4_{j}")
                c_ap = t4[:, 0:F]
                s_ap = t4[:, F:2 * F]
                b.pers_ids.add(id(c_ap))
                b.pers_ids.add(id(s_ap))
                sh = b.scratch(DT)   # sin(q/2)
                nc.scalar.activation(sh, qj, AF.Sin, bias=zero_t[:, :],
                                     scale=0.5)
                sq = b.scratch(DT)   # sin(q/4)
                nc.scalar.activation(sq, qj, AF.Sin, bias=zero_t[:, :],
                                     scale=0.25)
                sq2 = b.scratch(DT)  # sin^2(q/4)
                nc.scalar.activation(sq2, sq, AF.Square)
                ch = b.scratch(DT)   # cos(q/2)
                nc.scalar.activation(ch, sq2, AF.Copy, bias=1.0, scale=-2.0)
                sh2 = b.scratch(DT)  # sin^2(q/2)
                nc.scalar.activation(sh2, sh, AF.Square)
                nc.scalar.activation(c_ap, sh2, AF.Copy, bias=1.0, scale=-2.0)
                nc.vector.tensor_tensor(s_ap, sh, ch, OP.mult)  # sin q / 2
                if use_pack_fwd or use_pack_bwd:
                    # slots 2,3: [-sin/2 | cos] for packed-rotation patterns
                    nc.scalar.activation(t4[:, 2 * F:3 * F], s_ap, AF.Copy,
                                         bias=0.0, scale=-1.0)
                    nc.scalar.activation(t4[:, 3 * F:4 * F], c_ap, AF.Copy,
                                         bias=0.0, scale=1.0)
                    b.n_act += 2
                b.n_tt += 1
                b.n_act += 6
                se = Expr([(2.0, s_ap)])
                se._mat = (2.0, s_ap)
                trig[j] = (as_pers_expr(c_ap), se)
                trig_raw[j] = t4

            emit_trig(0)
            emit_trig(1)
            b.joint_boundary()



            # qd repack on DVE, after the sin products in the in-order DVE
            # stream (qd's DMA rides the slower gpsimd queue; putting these
            # first would block the trig products behind that DMA).
            qd_pl = []
            for j in range(ND - 1):  # qd_6 is only read once (tau_6)
                d_ap = b.persistent(f"qd{j}")
                nc.vector.tensor_scalar(d_ap, qd3[:, :, j], 1.0, None, OP.mult)
                qd_pl.append(as_pers_expr(d_ap))

            def qdd_expr(j):
                # strided read (28B stride) is cheapest, measured against all
                # repack routes: ACT copy queues behind the trig prologue
                # (+5us); DVE copy costs more than the ~190ns/op penalty;
                # SBUF->SBUF de-stride DMA on the sync queue is ~19x derated
                # and its sem deps stall the forward chains (+23us).
                ap = qdd3[:, :, j]
                e = Expr([(1.0, ap)])
                e._mat = (1.0, ap)
                b.pers_ids.add(id(ap))
                return e

            def rot_inv(j, X):
                """Rz(q_j)^T @ (rot_fix_j^T @ X)"""
                if not any(_nonzero(e) for e in X):
                    return [ZERO, ZERO, ZERO]
                Fm = rot_fix[j].T
                Y = b.matvec_const(Fm, X)
                c, s = trig[j]
                z0 = b.lin((1.0, b.mul(c, Y[0])), (1.0, b.mul(s, Y[1])))
                z1 = b.lin((-1.0, b.mul(s, Y[0])), (1.0, b.mul(c, Y[1])))
                return [z0, z1, Y[2]]

            def write_out(j, e):
                dst = out3[:, :, j]
                if not e.terms:
                    b.nc.vector.memset(dst, float(e.const))
                    return
                c, ap = b.mat(e)
                nc.scalar.activation(dst, ap, AF.Copy, bias=float(e.const),
                                     scale=float(c))
                b.n_act += 1

            # ---------------- forward (joints 0..5) ----------------
            w_p = [ZERO, ZERO, ZERO]
            v_p = [ZERO, ZERO, ZERO]
            a_p = [ZERO, ZERO, ZERO]
            la_p = [ZERO, ZERO, Expr(const=GRAV)]
            states = []
            use_qpack = os.environ.get("K_QPACK", "1") == "1"
            fwd_ztags = {}
            for j in range(ND - 1):
                t_j = trans[j]
                Uv = b.vsub(v_p, b.cross_const(t_j, w_p))
                Ua = b.vsub(la_p, b.cross_const(t_j, a_p))
                Uv = [b.snap(e, f"Uv{j}", scratch_ok=True)
                      if len(e.terms) > 1 else e for e in Uv]
                Ua = [b.snap(e, f"Ua{j}", scratch_ok=True)
                      if len(e.terms) > 1 else e for e in Ua]
                packed_done = False
                if use_qpack and j > 0:
                    Fm = rot_fix[j].T
                    Yw = b.matvec_const(Fm, w_p)
                    Yv = b.matvec_const(Fm, Uv)
                    Ya = b.matvec_const(Fm, a_p)
                    Yu = b.matvec_const(Fm, Ua)
                    pairs = [(Yw[0], Yw[1]), (Yv[0], Yv[1]),
                             (Ya[0], Ya[1]), (Yu[0], Yu[1])]
                    if all(e.terms for pr in pairs for e in pr):
                        t4 = trig_raw[j]
                        z0s, z1s, ztags = b.quad_rot(
                            pairs, t4[:, 0:F], t4[:, F:2 * F], f"q{j}")
                        Rw = [z0s[0], z1s[0], Yw[2]]
                        Rv = [z0s[1], z1s[1], Yv[2]]
                        Ra = [z0s[2], z1s[2], Ya[2]]
                        Rla = [z0s[3], z1s[3], Yu[2]]
                        fwd_ztags[j] = ztags
                        packed_done = True
                _kpj = int(os.environ.get("K_PACK_J", "-1"))
                if not packed_done and use_pack_fwd and j > 0 \
                        and (_kpj < 0 or j == _kpj):
                    Fm = rot_fix[j].T

                    def _rows(X):
                        return [b.lin((Fm[i, 0], X[0]), (Fm[i, 1], X[1]),
                                      (Fm[i, 2], X[2])) for i in range(3)]

                    Yw, Yv, Ya, Yu = _rows(w_p), _rows(Uv), _rows(a_p), _rows(Ua)
                    heads = [Yw[0], Yw[1], Yv[0], Yv[1],
                             Ya[0], Ya[1], Yu[0], Yu[1]]
                    if all(e.terms for e in heads):
                        t4 = trig_raw[j]
                        zw = b.packed_pair_rot(Yw[0], Yw[1], t4, +1,
                                               b.persistent(f"w{j}0"),
                                               b.persistent(f"w{j}1"), True)
                        zv = b.packed_pair_rot(Yv[0], Yv[1], t4, +1,
                                               b.persistent(f"v{j}0"),
                                               b.persistent(f"v{j}1"), True)
                        za = b.packed_pair_rot(Ya[0], Ya[1], t4, +1,
                                               b.scratch(), b.scratch(), False)
                        zu = b.packed_pair_rot(Yu[0], Yu[1], t4, +1,
                                               b.scratch(), b.scratch(), False)
                        Rw = [zw[0], zw[1], Yw[2]]
                        Rv = [zv[0], zv[1], Yv[2]]
                        Ra = [za[0], za[1], Ya[2]]
                        Rla = [zu[0], zu[1], Yu[2]]
                        packed_done = True
                if not packed_done:
                    Rw = rot_inv(j, w_p)
                    Rv = rot_inv(j, Uv)
                    Ra = rot_inv(j, a_p)
                    Rla = rot_inv(j, Ua)
                qdj = qd_pl[j]
                qddj = qdd_expr(j)
                w = [Rw[0], Rw[1], b.lin((1.0, Rw[2]), (1.0, qdj))]
                w = b.snap_vec(w, f"w{j}_")
                v = b.snap_vec(Rv, f"v{j}_")
                dw = [
                    b.lin((1.0, Ra[0]), (1.0, b.mul(w[1], qdj))),
                    b.lin((1.0, Ra[1]), (-1.0, b.mul(w[0], qdj))),
                    b.lin((1.0, Ra[2]), (1.0, qddj)),
                ]
                dv = [
                    b.lin((1.0, Rla[0]), (1.0, b.mul(v[1], qdj))),
                    b.lin((1.0, Rla[1]), (-1.0, b.mul(v[0], qdj))),
                    Rla[2],
                ]
                dw = b.snap_vec(dw, f"dw{j}_")
                dv = b.snap_vec(dv, f"dv{j}_")
                states.append((w, v, dw, dv))
                w_p, v_p, a_p, la_p = w, v, dw, dv
                if j + 2 < ND - 1:
                    emit_trig(j + 2)
                if j in (1, 3):
                    b.new_wave()
                b.joint_boundary()

            # ---------------- backward (j = 5..0) ----------------
            # tau_6 = damping_6 * qd_6 is qd_6's only read: use the strided
            # column directly instead of a dense repack copy.
            qd6 = Expr([(1.0, qd3[:, :, ND - 1])])
            qd6._mat = (1.0, qd3[:, :, ND - 1])
            b.pers_ids.add(id(qd6.terms[0][1]))
            write_out(ND - 1, b.lin((damping[ND - 1], qd6)))

            lin_f = [ZERO, ZERO, ZERO]
            ang_f = [ZERO, ZERO, ZERO]
            bw_pack = None
            use_brot = os.environ.get("K_BROT", "1") == "1"
            use_dveseed = os.environ.get("K_DVESEED", "0") == "1"
            for j in range(ND - 2, -1, -1):
                if j in (5, 4, 3, 2, 1):
                    b.new_wave()
                have_child = any(_nonzero(e) for e in lin_f + ang_f)
                if have_child and use_pack_bwd and bw_pack is not None:
                    bw_tile_p, lfc, afc, prev_tag = bw_pack
                    t4 = trig_raw[j + 1]
                    Rf = rot_fix[j + 1]
                    zlf = b.packed_pair_rot(
                        None, None, t4, -1, b.scratch(), b.scratch(), False,
                        A_ready=(bw_tile_p[:, 0:2 * F],) + lfc)
                    zaf = b.packed_pair_rot(
                        None, None, t4, -1, b.scratch(), b.scratch(), False,
                        A_ready=(bw_tile_p[:, 2 * F:4 * F],) + afc)
                    b.free_wide.setdefault(4, []).append(prev_tag)
                    RzLf = [zlf[0], zlf[1], lin_f[2]]
                    RzAf = [zaf[0], zaf[1], ang_f[2]]
                    Rc_lf = b.matvec_const(Rf, RzLf)
                    Rc_lf = [b.snap(e, f"rclf{j}", scratch_ok=True)
                             if len(e.terms) > 2 and (j > 0 or i < 2) else e
                             for i, e in enumerate(Rc_lf)]
                    Rc_af = b.matvec_const(Rf, RzAf)
                    child_ang = b.vadd(b.cross_const(trans[j + 1], Rc_lf), Rc_af)
                    child_lin = Rc_lf
                elif have_child and use_brot:
                    Rf = rot_fix[j + 1]
                    t4c = trig_raw[j + 1]
                    lfp, afp = b.pair_rot_bwd(
                        (lin_f[0], lin_f[1]), (ang_f[0], ang_f[1]),
                        t4c[:, 0:F], t4c[:, F:2 * F], f"br{j}",
                        dve_seed=(j <= 1 and use_dveseed))
                    RzLf = [lfp[0], lfp[1], lin_f[2]]
                    RzAf = [afp[0], afp[1], ang_f[2]]
                    Rc_lf = b.matvec_const(Rf, RzLf)
                    Rc_lf = [b.snap(e, f"rclf{j}", scratch_ok=True)
                             if len(e.terms) > 2 and (j > 0 or i < 2) else e
                             for i, e in enumerate(Rc_lf)]
                    Rc_af = b.matvec_const(Rf, RzAf)
                    child_ang = b.vadd(b.cross_const(trans[j + 1], Rc_lf), Rc_af)
                    child_lin = Rc_lf
                elif have_child:
                    cs, ss = trig[j + 1]
                    Rf = rot_fix[j + 1]
                    # xy-rotation outputs fan out 3x through the rot_fix
                    # matvec: snapping them saves (t-1)(f-1) chain slots.
                    RzLf = [
                        b.snap(b.lin((1.0, b.mul(cs, lin_f[0])),
                                     (-1.0, b.mul(ss, lin_f[1]))),
                               f"rzlf{j}0", scratch_ok=True),
                        b.snap(b.lin((1.0, b.mul(ss, lin_f[0])),
                                     (1.0, b.mul(cs, lin_f[1]))),
                               f"rzlf{j}1", scratch_ok=True),
                        lin_f[2],
                    ]
                    Rc_lf = b.matvec_const(Rf, RzLf)
                    Rc_lf = [b.snap(e, f"rclf{j}", scratch_ok=True)
                             if len(e.terms) > 2 and (j > 0 or i < 2) else e
                             for i, e in enumerate(Rc_lf)]
                    RzAf = [
                        b.snap(b.lin((1.0, b.mul(cs, ang_f[0])),
                                     (-1.0, b.mul(ss, ang_f[1]))),
                               f"rzaf{j}0", scratch_ok=True),
                        b.snap(b.lin((1.0, b.mul(ss, ang_f[0])),
                                     (1.0, b.mul(cs, ang_f[1]))),
                               f"rzaf{j}1", scratch_ok=True),
                        ang_f[2],
                    ]
                    Rc_af = b.matvec_const(Rf, RzAf)
                    child_ang = b.vadd(b.cross_const(trans[j + 1], Rc_lf), Rc_af)
                    child_lin = Rc_lf
                else:
                    child_ang = [ZERO, ZERO, ZERO]
                    child_lin = [ZERO, ZERO, ZERO]

                # free previous joint's planes only after the child rotation
                # has consumed the raw lf/af term planes
                if j != ND - 2:
                    for vec in states[j + 1]:
                        b.free_expr_vec(vec)
                    if j + 1 in fwd_ztags:
                        for tag in fwd_ztags.pop(j + 1):
                            b.free_wide.setdefault(2, []).append(tag)

                w, v, dw, dv = states[j]
                m = float(mass[j])
                mc = m * com[j]
                cxm = np.array([
                    [0.0, -com[j][2], com[j][1]],
                    [com[j][2], 0.0, -com[j][0]],
                    [-com[j][1], com[j][0], 0.0],
                ])
                Isp = inertia[j] + m * (cxm @ cxm.T)

                if j == 0 and not _nonzero(w[0]) and not _nonzero(w[1]) \
                        and not any(_nonzero(e) for e in v):
                    # Base joint: only ang_f.z feeds torque_0 and nothing
                    # consumes lin_f_0/ang_f_0 further down. With w=(0,0,qd0)
                    # and v=0, tmp_a.z = (w x IcV_a).z + (v x IcV_l).z = 0,
                    # so af_z = (Isp dw + mc x dv).z + child_ang.z only.
                    af2 = b.lin(
                        (Isp[2, 0], dw[0]), (Isp[2, 1], dw[1]),
                        (Isp[2, 2], dw[2]),
                        (mc[0], dv[1]), (-mc[1], dv[0]),
                        (1.0, child_ang[2]))
                    write_out(0, b.lin((1.0, af2), (damping[0], qd_pl[0])))
                    b.joint_boundary()
                    continue

                IcA_l = b.vsub([b.lin((m, dv[i])) for i in range(3)],
                               b.cross_const(mc, dw))
                IcA_a = b.vadd(b.matvec_const(Isp, dw), b.cross_const(mc, dv))
                IcV_l = b.vsub([b.lin((m, v[i])) for i in range(3)],
                               b.cross_const(mc, w))
                IcV_a = b.vadd(b.matvec_const(Isp, w), b.cross_const(mc, v))
                if use_xpack:
                    # Packed crosses: operands land EXACTLY (coef 1, const
                    # folded) in cyclically-duplicated [x0|x1|x2|x0|x1]
                    # tiles; each cross is then 2 wide products + 1 wide
                    # subtract instead of 6 narrow products + per-component
                    # chain slots.
                    tail = j <= 1 and use_dveseed
                    cw, cw_tag = b.cyc_tile(w, f"cw{j}", dve_seed=tail)
                    cv, cv_tag = b.cyc_tile(v, f"cv{j}", dve_seed=tail)
                    cva, cva_tag = b.cyc_tile(IcV_a, f"cva{j}", dve_seed=tail)
                    cvl, cvl_tag = b.cyc_tile(IcV_l, f"cvl{j}", dve_seed=tail)
                    C1 = b.cross_packed(cw, cva)
                    C2 = b.cross_packed(cv, cvl)
                    C3 = b.cross_packed(cw, cvl)
                    nc.vector.tensor_tensor(C1, C1, C2, OP.add)
                    b.n_tt += 1

                    def _slices(t):
                        out = []
                        for k in range(3):
                            ap = t[:, k * F:(k + 1) * F]
                            b.pers_ids.add(id(ap))
                            e = Expr([(1.0, ap)])
                            e._mat = (1.0, ap)
                            out.append(e)
                        return out

                    tmp_a = _slices(C1)
                    tmp_l = _slices(C3)
                    for tag in (cw_tag, cv_tag, cva_tag, cvl_tag):
                        b.free_wide.setdefault(5, []).append(tag)
                else:
                    tmp_a = b.vadd(b.cross_ee(w, IcV_a), b.cross_ee(v, IcV_l))
                    tmp_l = b.cross_ee(w, IcV_l)
                lf_new = b.vadd(IcA_l, tmp_l, child_lin)
                af_new = b.vadd(IcA_a, tmp_a, child_ang)
                if use_pack_bwd and j > 0:
                    bw_tile, bw_tag = b.persistent_wide(4, f"bw{j}")
                    lf0 = b.snap_to(lf_new[0], bw_tile[:, 0:F])
                    lf1 = b.snap_to(lf_new[1], bw_tile[:, F:2 * F])
                    af0 = b.snap_to(af_new[0], bw_tile[:, 2 * F:3 * F])
                    af1 = b.snap_to(af_new[1], bw_tile[:, 3 * F:4 * F])
                    lf2 = b.snap(lf_new[2], f"lf{j}2")
                    af2 = b.snap(af_new[2], f"af{j}2")
                    lin_f = [lf0, lf1, lf2]
                    ang_f = [af0, af1, af2]
                    bw_pack = (bw_tile,
                               (lf0._mat[0], lf1._mat[0],
                                lf0.const, lf1.const),
                               (af0._mat[0], af1._mat[0],
                                af0.const, af1.const), bw_tag)
                elif use_brot:
                    # carry lf/af xy raw: the next joint's packed rotation
                    # materializes them directly into its operand tile
                    lin_f = [lf_new[0], lf_new[1],
                             b.snap(lf_new[2], f"lf{j}2")]
                    ang_f = [af_new[0], af_new[1],
                             b.snap(af_new[2], f"af{j}2")]
                    bw_pack = None
                else:
                    lin_f = b.snap_vec(lf_new, f"lf{j}_")
                    ang_f = b.snap_vec(af_new, f"af{j}_")
                    bw_pack = None
                write_out(j, b.lin((1.0, ang_f[2]), (damping[j], qd_pl[j])))
                b.joint_boundary()

            nc.sync.dma_start(dram_view(tq_d), out_t[:, :])

            stats = dict(stt=b.n_stt, tt=b.n_tt, act=b.n_act, copy=b.n_copy,
                         mm=b.n_mm, nw=[len(w) for w in b.wave_coefs],
                         busy={k: round(v / 1000.0, 1)
                               for k, v in b.busy.items()},
                         pers=b.pers_idx, max_joint_allocs=b.max_joint_allocs,
                         ring=b.ring_size,
                         mat_hist=dict(sorted(getattr(b, "mat_hist", {}).items())))
            stats["wcoefs"] = [list(w) for w in b.wave_coefs]
            stats["wmax"] = wmax if use_pe else 0
            stats["n_waves"] = n_waves

    nc.compile()
    return nc, stats


_CACHE = {}


def _get_module(params):
    import hashlib
    key = b"".join(np.ascontiguousarray(np.asarray(params[k], np.float32)).tobytes()
                   for k in ("trans", "rot_fix", "mass", "com", "inertia",
                             "damping"))
    h = hashlib.sha1(key).hexdigest()
    if h not in _CACHE:
        _CACHE[h] = build_module(params)
    return _CACHE[h]


def run(q, qd, qdd_des, trans, rot_fix, mass, com, inertia, damping,
        trace=False):
    q = np.asarray(q)
    qd = np.asarray(qd)
    qdd_des = np.asarray(qdd_des)
    assert q.shape == (B_TOTAL, ND), f"unexpected q shape {q.shape}"
    assert qd.shape == (B_TOTAL, ND) and qdd_des.shape == (B_TOTAL, ND)
    params = dict(trans=trans, rot_fix=rot_fix, mass=mass, com=com,
                  inertia=inertia, damping=damping)
    nc, stats = _get_module(params)
    wconst = None
    if stats.get("wmax"):
        whalf = stats["wmax"] // 2
        nw = stats["n_waves"]
        wconst = np.zeros((P, nw * whalf * P), np.float32)
        eye = np.eye(P, dtype=np.float32)
        for wv, coefs in enumerate(stats["wcoefs"]):
            for s, cval in enumerate(coefs):
                a = (wv * whalf + s) * P
                wconst[:, a:a + P] = np.float32(cval) * eye
    in_maps = []
    for c in range(N_CORES):
        sl = slice(c * BC, (c + 1) * BC)
        m = {
            "q": np.ascontiguousarray(q[sl], np.float32),
            "qd": np.ascontiguousarray(qd[sl], np.float32),
            "qdd_des": np.ascontiguousarray(qdd_des[sl], np.float32),
        }
        if wconst is not None:
            m["wconst"] = wconst
        in_maps.append(m)
    res = bass_utils.run_bass_kernel_spmd(
        nc, in_maps, core_ids=list(range(N_CORES)), trace=trace)
    out = np.concatenate([res.results[c]["torque"] for c in range(N_CORES)],
                         axis=0)
    return out.astype(np.float32), res, stats


def kernel(q, qd, qdd_des, trans, rot_fix, mass, com, inertia, damping):
    out, _, _ = run(q, qd, qdd_des, trans, rot_fix, mass, com, inertia,
                    damping, trace=False)
    return out



# revision 49
# speedup vs baseline: 1.0033x; 1.0033x over previous
"""Trainium2 Bass kernel for batched 7-DOF RNEA inverse dynamics.

Layout: pure data-parallel over 8 NeuronCores (32768 batch elements each).
Per core, every per-element scalar lives as an SBUF "plane" [128, 256] f32
(batch element e = partition*256 + free). All per-link parameters (trans,
rot_fix, mass, com, inertia, damping) are baked into the instruction stream
as immediates at build time.

The math is emitted through a small expression compiler:
  Expr = sum(coef * plane) + const
Linear combinations are free (term concatenation); they materialize as
chains of fused scalar_tensor_tensor ops ((in0*imm) + in1, in-place
accumulation) only when a product or an output needs a raw plane. Products
are DVE tensor_tensor ops. sin/cos and the final affine outputs go to the
scalar engine (ACT).

Algebraic structure used:
  - Rinv @ x with Rinv = Rz(q)^T @ rot_fix^T: constant matvec folded into
    stt-chain immediates, then one complex rotation (4 products).
  - cross(t_inv, Rinv x) = -Rinv (t x x): constant cross matrices fold into
    linear immediates, killing 12 products per forward joint.
  - forward state for joint 6 is never computed (the reference's backward
    recursion never reads it).
  - ACT Sin has no range reduction (accurate only on [-pi, pi]) and
    q ~ N(0,1) exceeds pi; half-angle identities avoid wrapping:
    cos q = 1-2sin^2(q/2), sin q = 2 sin(q/2)(1-2sin^2(q/4)).

Measured (8 cores, trn2): HW exec ~309 us, max abs err ~8e-5 on torque
absmax ~139 (fp32). Vector engine is the bottleneck (~650 DVE ops after
packing, from ~950); measured DVE op costs: 359 ns/[128,256] STT or TT
(267 ns data + ~92 ns SBUF-access/issue overhead), tensor_scalar 225 ns
(2x_2p), ACT 420 ns and fully parallel with DVE. Wide-op packing exploits
the per-op overhead plus term-count reduction:
  - forward quad-rot (K_QPACK): the 4 per-joint Rz^T rotations run as 4
    wide [P,4F] products against ACT-replicated trig tiles (slot scales
    absorb the mats' lead coefs; consts folded into slot contents via
    ACT-seeded exact chains) + 4 wide combines; outputs are pure planes.
  - backward child rotation (K_BROT): lf/af xy carried as RAW exprs,
    materialized into one [lf0|lf1|af0|af1] tile at the consuming joint,
    rotated with 4+2 wide [P,2F] ops.
  - backward crosses (K_XPACK): w, v, IcV_a, IcV_l land exactly in
    cyclically-duplicated [x0|x1|x2|x0|x1] tiles; each cross is 2 wide
    [P,3F] products + 1 wide subtract.
  - base joint computes only ang_f.z (everything else is dead there).
  - trig chains emitted lazily (joint j+2 with joint j's body) so the
    in-order ACT queue reaches each joint's replicated-trig copies just
    before the DVE needs them; front-loading all six chains costs ~5 us.
Rejected with on-HW measurements: TensorE accumulate (475 ns/[128,256]
matmul term, only ~50% overlap with DVE traffic), GPSIMD (SBUF ports
physically shared with DVE: 2-input ops serialize ~100%), bf16/fp16
(abs-err budget ~3e-3 at near-zero outputs vs ~1e-2+ rounding of O(30)
intermediates), and state-major layouts for PE matvecs (DVE free-dim
cycle cost explodes at <128 active partitions).
"""

import os
import sys

for _p in ("/opt/trn_rl_repo",):
    if os.path.isdir(_p) and _p not in sys.path:
        sys.path.append(_p)

import numpy as np

import concourse.bass as bass
import concourse.bacc as bacc
import concourse.mybir as mybir
from concourse import tile as tile_mod
from concourse import bass_utils

N_CORES = 8
ND = 7
B_TOTAL = 262144
BC = B_TOTAL // N_CORES  # 32768 per core
P = 128
F = BC // P  # 256
GRAV = 9.81
DT = mybir.dt.float32
DT_C = mybir.dt.bfloat16 if os.environ.get("K_BF16", "0") == "1" else mybir.dt.float32
OP = mybir.AluOpType
AF = mybir.ActivationFunctionType


class Expr:
    """value = sum(coef * plane_ap) + const"""

    __slots__ = ("terms", "const", "_mat")

    def __init__(self, terms=(), const=0.0):
        self.terms = list(terms)
        self.const = float(const)
        self._mat = None  # cached (coef, ap) of materialized sum-of-terms


ZERO = Expr()


def _nonzero(e):
    return bool(e.terms) or e.const != 0.0


class Builder:
    def __init__(self, nc, pool, ring_size=64, pool_frac=0.0,
                 pspool=None, wtile=None, wmax=0):
        self.nc = nc
        self.pool = pool
        self.n_stt = 0
        self.n_tt = 0
        self.n_act = 0
        self.n_copy = 0
        self.n_mm = 0
        # engine load balancing between DVE and GPSIMD (2-input ops)
        self.eng_busy = [0.0, 0.0]  # ns accumulated: [vector, gpsimd]
        self.eng_cost = [445.0, 980.0]
        self.pool_frac = pool_frac
        self.ring_size = ring_size
        self.ring_idx = 0
        self.joint_allocs = 0
        self.max_joint_allocs = 0
        self.pers_idx = 0
        self.free_tags = []       # recycled persistent tags
        self.free_wide = {}       # recycled wide tags by slot count
        self.pers_ids = set()     # id(ap) of planes safe to reference long-term
        self.ap_tag = {}          # id(ap) -> tag (for freeing)
        # ---- PE (TensorEngine) chain offload ----
        # Constant-coefficient linear combinations accumulate in PSUM via
        # diag-weight fp32 matmuls (exact: HW does the 4-pass H/L split).
        # Measured: ~490ns/term sustained incl. per-MM LDWEIGHTS, fully
        # parallel with DVE; DVE reads PSUM operands at SBUF cost.
        self.use_pe = wtile is not None and os.environ.get("K_PE", "1") == "1"
        self.pspool = pspool
        self.wtile = wtile
        self.wmax = wmax
        self.wave = 0
        self.wave_coefs = [[]]    # per wave: slot i holds diag(coefs[i])
        self.wslot = {}
        self.w_d = None           # wconst dram tensor (for wave refills)
        self.ps_ring = 0
        self.ps_ntags = int(os.environ.get("K_PSTAGS", "8"))
        # projected engine busy (ns) for greedy makespan routing
        self.busy = {"dve": 0.0, "act": 0.0, "pe": 0.0}
        self.pe_cost = float(os.environ.get("K_PE_COST", "500"))
        self.dve_cost = float(os.environ.get("K_DVE_COST", "424"))
        self.act_evac = 480.0
        self.dve_evac = 258.0

    def dve_track(self, n_ops, width=1):
        self.busy["dve"] += n_ops * (width * 267.0 + 157.0)

    @property
    def whalf(self):
        return self.wmax // 2

    def _wslot_room(self, coefs):
        cur = self.wave_coefs[self.wave]
        new = {c for c in coefs if c not in self.wslot}
        return len(cur) + len(new) <= self.whalf

    def _wslot_ap(self, c):
        s = self.wslot.get(c)
        if s is None:
            cur = self.wave_coefs[self.wave]
            s = len(cur)
            cur.append(c)
            self.wslot[c] = s
        base = (self.wave % 2) * self.whalf
        return self.wtile[:, (base + s) * P:(base + s + 1) * P]

    def _wave_dma(self, wave):
        """DMA wave's coefficient table into its half of the weight tile.

        Waves ping-pong between halves and each wave's DMA is issued one
        wave EARLY (the coefficient values live in DRAM, resolved at run
        time, so emission only needs the offsets): by the time a wave's
        LDWEIGHTS run, its table landed a full wave ago."""
        dram_base = wave * self.whalf * P
        sb_base = (wave % 2) * self.whalf * P
        wq = [self.nc.sync, self.nc.gpsimd] * 3
        nchunk = len(wq)
        bounds = [round(i * self.whalf / nchunk) for i in range(nchunk + 1)]
        for ci in range(nchunk):
            a, bnd = bounds[ci] * P, bounds[ci + 1] * P
            if a < bnd:
                wq[ci].dma_start(self.wtile[:, sb_base + a:sb_base + bnd],
                                 self.w_d.ap()[:, dram_base + a:dram_base + bnd])

    def new_wave(self):
        """Advance to a fresh coefficient table half; prefetch the next."""
        if not self.use_pe or self.wave + 1 >= self.n_waves:
            return
        self.wave += 1
        self.wave_coefs.append([])
        self.wslot = {}
        if self.wave + 1 < self.n_waves:
            self._wave_dma(self.wave + 1)

    def _psum(self):
        """PSUM chain slot. Banks are the allocation granularity (8), so two
        chain slots share each bank tile (has_written bits are per-element,
        so independent accumulation groups coexist in one bank)."""
        if os.environ.get("K_PSHALF", "0") == "1":
            n = self.ps_ring
            self.ps_ring += 1
            half = n % 2
            if half == 0:
                tag = f"ps{(n // 2) % self.ps_ntags}"
                self._ps_cur = self.pspool.tile([P, 2 * F], DT, tag=tag,
                                                name=tag)
            return self._ps_cur[:, half * F:(half + 1) * F]
        tag = f"ps{self.ps_ring % self.ps_ntags}"
        self.ps_ring += 1
        t = self.pspool.tile([P, 2 * F], DT, tag=tag, name=tag)
        return t[:, 0:F]

    def _pe_accum(self, terms):
        """PSUM <- sum(c*ap) over SBUF-resident terms via diag-weight MMs."""
        ps = self._psum()
        n = len(terms)
        for i, (c, ap) in enumerate(terms):
            w = self._wslot_ap(float(c))
            self.nc.tensor.matmul(ps, w, ap, start=(i == 0), stop=(i == n - 1))
        self.n_mm += n
        self.busy["pe"] += n * self.pe_cost
        return ps

    def _pe_split(self, terms):
        """(pe_terms, dve_terms, sigma) under space/slot constraints, or None.

        sigma flips the whole accumulation's sign when -c slots are a better
        match for the live weight table (the flip is undone at evac / in the
        returned coefficient), halving slot burn for +-c coefficient pairs.
        """
        if not self.use_pe:
            return None
        pe, dve = [], []
        for c, ap in terms:
            if ap.space == bass.MemorySpace.SBUF:
                pe.append((float(c), ap))
            else:
                dve.append((c, ap))
        if len(pe) < 2:
            return None
        hits_p = sum(1 for c, _ in pe if c in self.wslot)
        hits_n = sum(1 for c, _ in pe if -c in self.wslot)
        sigma = -1.0 if hits_n > hits_p else 1.0
        pe = [(sigma * c, ap) for c, ap in pe]
        if not self._wslot_room([c for c, _ in pe]):
            return None
        return pe, dve, sigma

    def _evac(self, dst, ps, scale, bias, dve_pref=False):
        ca = self.busy["act"] + self.act_evac
        cd = self.busy["dve"] + self.dve_evac
        mode = os.environ.get("K_EVAC", "auto")
        if mode == "act":
            dve_pref = False
            cd = ca + 1.0
        elif mode == "dve":
            dve_pref = True
        if dve_pref or cd < ca:
            self.nc.vector.tensor_scalar(dst, ps, float(scale), float(bias),
                                         OP.mult, OP.add)
            self.busy["dve"] = cd
            self.n_stt += 1
        else:
            self.nc.scalar.activation(dst, ps, AF.Copy, bias=float(bias),
                                      scale=float(scale))
            self.busy["act"] = ca
            self.n_act += 1

    def _pe_chain(self, terms, dst, scale=1.0, bias=0.0, dve_seed=False,
                  seed_act=True):
        """Try PE route for sum(c*ap)*scale(+bias). Returns result ap or None.

        dst None: result stays in PSUM (requires scale==1, bias==0 handled by
        caller convention). dst given: evacuated into dst (SBUF).
        Split chains merge the DVE-side terms into dst with the PSUM partial
        as a free STT src1 operand.
        """
        limit = int(os.environ.get("K_PE_LIMIT", "100000"))
        if getattr(self, "pe_chains", 0) >= limit:
            return None
        split = self._pe_split(terms)
        if split is None:
            return None
        pe, dve, sigma = split
        n_pe, n_dve = len(pe), len(dve)
        # status quo: whole chain on DVE (+ACT seed when it would use one)
        sq_d = self.busy["dve"] + (len(terms) - 1) * self.dve_cost
        sq_a = self.busy["act"] + (self.act_evac if seed_act else 0.0)
        mk_sq = max(sq_d, sq_a, self.busy["pe"])
        ev = 0.0 if (dst is None and n_dve == 0) else \
            (0.0 if n_dve else min(self.act_evac, self.dve_evac))
        pe_d = self.busy["dve"] + n_dve * self.dve_cost
        pe_p = self.busy["pe"] + n_pe * self.pe_cost
        mk_pe = max(pe_d, self.busy["act"], pe_p) + ev
        if mk_pe >= mk_sq:
            return None
        self.pe_chains = getattr(self, "pe_chains", 0) + 1
        if dst is None and n_dve:
            # merge would need an SBUF dst anyway; use scratch
            dst_eff = self.scratch()
        else:
            dst_eff = dst
        ps = self._pe_accum(pe)  # holds sigma * sum(c_i x_i) over pe terms
        if n_dve == 0:
            if dst_eff is None:
                return ps, sigma
            self._evac(dst_eff, ps, sigma * scale, bias, dve_pref=dve_seed)
            return dst_eff, 1.0
        # DVE merge. The leftover (non-SBUF) terms are PSUM-resident, so the
        # partial must leave PSUM first (one PSUM operand per DVE op).
        self.nc.vector.tensor_scalar(dst_eff, ps, float(sigma * scale),
                                     float(bias), OP.mult, OP.add)
        self.n_stt += 1
        self.busy["dve"] += self.dve_evac
        for ck, xk in dve:
            self.nc.vector.scalar_tensor_tensor(
                dst_eff, xk, float(ck * scale), dst_eff, OP.mult, OP.add)
            self.n_stt += 1
            self.busy["dve"] += self.dve_cost
        return dst_eff, 1.0

    def pick_engine(self, n_ops=1):
        """Pick vector or gpsimd for a chain of n_ops 2-input ops."""
        self.busy["dve"] += n_ops * self.dve_cost
        if self.pool_frac <= 0.0:
            self.eng_busy[0] += n_ops * self.eng_cost[0]
            return self.nc.vector
        c0 = self.eng_busy[0] + n_ops * self.eng_cost[0]
        c1 = self.eng_busy[1] + n_ops * self.eng_cost[1]
        if c1 < c0:
            self.eng_busy[1] = c1
            return self.nc.gpsimd
        self.eng_busy[0] = c0
        return self.nc.vector

    # ---- allocation ----
    def scratch(self, dtype=None):
        tag = f"s{self.ring_idx % self.ring_size}"
        t = self.pool.tile([P, F], dtype or DT_C, tag=tag, name=tag)
        self.ring_idx += 1
        self.joint_allocs += 1
        return t[:, :]

    def persistent(self, label=""):
        if self.free_tags:
            tag = self.free_tags.pop()
        else:
            tag = f"p{self.pers_idx}"
            self.pers_idx += 1
        t = self.pool.tile([P, F], DT_C, tag=tag, name=f"{tag}_{label}")
        ap = t[:, :]
        self.pers_ids.add(id(ap))
        self.ap_tag[id(ap)] = ("p", tag)
        return ap

    def wscratch(self, slots):
        """Wide scratch tile [P, slots*F] from a per-width ring."""
        if not hasattr(self, "wring"):
            self.wring = {}
        idx = self.wring.get(slots, 0)
        self.wring[slots] = idx + 1
        mod = {2: 2, 3: 5, 4: 8}.get(slots, 10)
        tag = f"w{slots}_{idx % mod}"
        t = self.pool.tile([P, slots * F], DT_C, tag=tag, name=tag)
        return t[:, :]

    def persistent_wide(self, slots, label=""):
        if not hasattr(self, "pwide_idx"):
            self.pwide_idx = 0
        fl = self.free_wide.setdefault(slots, [])
        if fl:
            tag = fl.pop()
        else:
            tag = f"pw{slots}_{self.pwide_idx}"
            self.pwide_idx += 1
        t = self.pool.tile([P, slots * F], DT_C, tag=tag, name=f"{tag}_{label}")
        return t[:, :], tag

    def packed_pair_rot(self, Y0, Y1, t4, sign, dst0, dst1, pers,
                        A_ready=None):
        """Rotate one (y0, y1) pair by the z-angle whose 4-slot trig tile is
        t4 = [cos | sin/2 | -sin/2 | cos].

        sign=+1: z0 = c y0 + s y1, z1 = -s y0 + c y1   (Rz^T)
        sign=-1: z0 = c y0 - s y1, z1 = +s y0 + c y1   (Rz)
        dst0/dst1: [P, F] planes receiving z0/a0, z1/a1.
        A_ready: optional (A_region[P,2F], a0, a1, k0, k1) when the pair is
        already materialized adjacently (coefs a*, deferred consts k*).
        Returns (z0_expr, z1_expr) incl. rotated deferred-const terms.
        """
        if A_ready is None:
            W = self.wscratch(2)
            a0, _ = self.mat(Y0, dst=W[:, 0:F])
            a1, _ = self.mat(Y1, dst=W[:, F:2 * F])
            k0, k1 = Y0.const, Y1.const
            A = W
        else:
            A, a0, a1, k0, k1 = A_ready
        c_slot = t4[:, 0:F]
        s_slot = t4[:, F:2 * F]
        if sign > 0:
            B1 = t4[:, 0:2 * F]            # [c | s/2]
            B2 = t4[:, 2 * F:4 * F]        # [-s/2 | c]
            # P1 = [c*y0r | (s/2)*y1r]; P2 = [(-s/2)*y0r | c*y1r]
            # z0 = a0*P1s0 + 2 a1*P1s1 ; z1 = 2 a0*P2s0 + a1*P2s1
        else:
            t4r = t4.rearrange("p (a c b) -> p a c b", a=2, c=2, b=F)
            B1 = t4r[:, :, 0, :]           # [c | -s/2]
            B2 = t4r[:, :, 1, :]           # [s/2 | c]
            # P1 = [c*y0r | (-s/2)*y1r]: z0 = a0 P1s0 + 2 a1 P1s1
            # P2 = [(s/2)*y0r | c*y1r]:  z1 = 2 a0 P2s0 + a1 P2s1
        P1 = self.wscratch(2)
        P2 = self.wscratch(2)
        self.nc.vector.tensor_tensor(P1, A, B1, OP.mult)
        self.nc.vector.tensor_tensor(P2, A, B2, OP.mult)
        self.n_tt += 2
        self.nc.vector.scalar_tensor_tensor(
            dst0, P1[:, F:2 * F], 2.0 * a1 / a0, P1[:, 0:F], OP.mult, OP.add)
        self.nc.vector.scalar_tensor_tensor(
            dst1, P2[:, 0:F], 2.0 * a0 / a1, P2[:, F:2 * F], OP.mult, OP.add)
        self.n_stt += 2
        if pers:
            self.pers_ids.add(id(dst0))
            self.pers_ids.add(id(dst1))
        t0 = [(a0, dst0)]
        t1 = [(a1, dst1)]
        if k0 != 0.0:
            t0.append((k0, c_slot))
            t1.append((-sign * 2.0 * k0, s_slot))
        if k1 != 0.0:
            t0.append((sign * 2.0 * k1, s_slot))
            t1.append((k1, c_slot))
        e0 = Expr(t0)
        e1 = Expr(t1)
        if len(t0) == 1:
            e0._mat = (a0, dst0)
        if len(t1) == 1:
            e1._mat = (a1, dst1)
        return e0, e1

    def act_copy(self, dst, src, scale=1.0, bias=0.0):
        self.nc.scalar.activation(dst, src, AF.Copy, bias=float(bias),
                                  scale=float(scale))
        self.n_act += 1
        self.busy["act"] += 480.0

    def mat_exact(self, e, dst, dve_seed=False):
        """Materialize the EXACT value of e into dst (coef 1, const folded).

        Unlike mat(), the result plane holds sum(coef*plane)+const verbatim,
        so packed slot-aligned products across different slots stay
        coefficient-consistent. Costs the same n-1 STT as mat(); a leading
        ACT copy (idle engine) absorbs the lead coef and the constant when
        no unit-coefficient lead term exists.
        """
        terms = sorted(e.terms, key=lambda t: -abs(t[0]))
        if not terms:
            self.nc.vector.memset(dst, float(e.const))
            return
        const = e.const
        unit = next((i for i, (ck, _) in enumerate(terms)
                     if ck == 1.0), None)
        if len(terms) >= 2 and self._pe_chain(
                terms, dst, 1.0, const, dve_seed=dve_seed,
                seed_act=(unit is None or const != 0.0) and not dve_seed,
                ) is not None:
            return
        if len(terms) == 1:
            c0, x0 = terms[0]
            if dve_seed:
                self.nc.vector.tensor_scalar(
                    dst, x0, float(c0), const, OP.mult, OP.add)
                self.n_stt += 1
            else:
                self.act_copy(dst, x0, scale=c0, bias=const)
            return
        if unit is not None and const == 0.0:
            c0, x0 = terms.pop(unit)
            ck, xk = terms.pop(0)
            self.nc.vector.scalar_tensor_tensor(
                dst, xk, float(ck), x0, OP.mult, OP.add)
            self.n_stt += 1
        else:
            c0, x0 = terms.pop(0)
            if dve_seed:
                self.nc.vector.tensor_scalar(
                    dst, x0, float(c0), const, OP.mult, OP.add)
                self.n_stt += 1
            else:
                self.act_copy(dst, x0, scale=c0, bias=const)
        for ck, xk in terms:
            self.nc.vector.scalar_tensor_tensor(
                dst, xk, float(ck), dst, OP.mult, OP.add)
            self.n_stt += 1

    def cyc_tile(self, vec, label, dve_seed=False):
        """[P,5F] tile holding [x0|x1|x2|x0|x1] of a 3-vector of Exprs."""
        t, tag = self.persistent_wide(5, label)
        for k in range(3):
            self.mat_exact(vec[k], t[:, k * F:(k + 1) * F], dve_seed=dve_seed)
        if dve_seed:
            self.nc.vector.tensor_scalar(
                t[:, 3 * F:5 * F], t[:, 0:2 * F], 1.0, None, OP.mult)
            self.n_stt += 1
        else:
            self.act_copy(t[:, 3 * F:4 * F], t[:, 0:F])
            self.act_copy(t[:, 4 * F:5 * F], t[:, F:2 * F])
        return t, tag

    def mat_rel(self, e, dst, dve_seed=False):
        """Materialize (value incl. const)/c0 into dst; returns c0.

        Like mat() but the constant is folded into the plane (via an ACT
        seed copy on the idle scalar engine), so rotating the plane rotates
        the full affine value and downstream exprs stay pure. dve_seed=True
        seeds with a DVE tensor_scalar instead: at the backward tail the
        ACT queue latency sits on the critical path (DVE has nothing left
        to overlap), so cross-engine seeding there costs ~0.5us per chain.
        """
        assert e.terms
        terms = sorted(e.terms, key=lambda t: -abs(t[0]))
        c0, x0 = terms[0]
        if len(terms) >= 2 and self._pe_chain(
                terms, dst, 1.0 / c0, e.const / c0, dve_seed=dve_seed,
                seed_act=(e.const != 0.0) and not dve_seed) is not None:
            return c0
        if e.const != 0.0 or len(terms) == 1:
            if dve_seed:
                self.nc.vector.tensor_scalar(
                    dst, x0, 1.0, e.const / c0, OP.mult, OP.add)
                self.n_stt += 1
            else:
                self.act_copy(dst, x0, scale=1.0, bias=e.const / c0)
            rest = terms[1:]
        else:
            c1, x1 = terms[1]
            self.nc.vector.scalar_tensor_tensor(
                dst, x1, c1 / c0, x0, OP.mult, OP.add)
            self.n_stt += 1
            rest = terms[2:]
        for ck, xk in rest:
            self.nc.vector.scalar_tensor_tensor(
                dst, xk, ck / c0, dst, OP.mult, OP.add)
            self.n_stt += 1
        return c0

    def quad_rot(self, pairs, c_ap, s_ap, label):
        """Rz^T-rotate W (Y0,Y1) expr pairs sharing one joint angle.

        z0 = c Y0 + s Y1 ; z1 = -s Y0 + c Y1. s_ap holds sin/2 (the 2x is
        folded into the replicated-trig scales). Y consts fold into slot
        contents (mat_rel), so outputs are pure planes. Slot coefficients
        fold into the per-slot scales of ACT-replicated trig tiles, letting
        each product group and each combine run as ONE wide DVE op.
        Returns (z0_exprs, z1_exprs, persistent_tags): z0/z1 of pairs [0,1]
        land in persistent [P,2F] tiles (joint states w/v); pairs [2,3] in
        scratch (consumed by the same joint's dw/dv).
        """
        W = len(pairs)
        assert W == 4
        A = self.wscratch(W)
        Bt = self.wscratch(W)
        CA = self.wscratch(W)
        SB = self.wscratch(W)
        SmA = self.wscratch(W)
        CB = self.wscratch(W)
        aA, aB = [], []
        for k, (y0, y1) in enumerate(pairs):
            sl = slice(k * F, (k + 1) * F)
            aA.append(self.mat_rel(y0, A[:, sl]))
            aB.append(self.mat_rel(y1, Bt[:, sl]))
        for k in range(W):
            sl = slice(k * F, (k + 1) * F)
            self.act_copy(CA[:, sl], c_ap, scale=aA[k])
            self.act_copy(SB[:, sl], s_ap, scale=2.0 * aB[k])
            self.act_copy(SmA[:, sl], s_ap, scale=-2.0 * aA[k])
            self.act_copy(CB[:, sl], c_ap, scale=aB[k])
        v = self.nc.vector
        P1 = self.wscratch(W)
        P2 = self.wscratch(W)
        v.tensor_tensor(P1, A, CA, OP.mult)
        v.tensor_tensor(P2, Bt, SB, OP.mult)
        ZA1, t1 = self.persistent_wide(2, f"{label}a1")
        ZB1 = self.wscratch(2)
        v.tensor_tensor(ZA1, P1[:, 0:2 * F], P2[:, 0:2 * F], OP.add)
        v.tensor_tensor(ZB1, P1[:, 2 * F:4 * F], P2[:, 2 * F:4 * F], OP.add)
        v.tensor_tensor(P1, A, SmA, OP.mult)
        v.tensor_tensor(P2, Bt, CB, OP.mult)
        ZA2, t2 = self.persistent_wide(2, f"{label}a2")
        ZB2 = self.wscratch(2)
        v.tensor_tensor(ZA2, P1[:, 0:2 * F], P2[:, 0:2 * F], OP.add)
        v.tensor_tensor(ZB2, P1[:, 2 * F:4 * F], P2[:, 2 * F:4 * F], OP.add)
        self.n_tt += 8
        self.dve_track(4, 4)
        self.dve_track(4, 2)

        def _mk(region, k):
            ap = region[:, k * F:(k + 1) * F]
            self.pers_ids.add(id(ap))
            e = Expr([(1.0, ap)])
            e._mat = (1.0, ap)
            return e

        z0s = [_mk(ZA1, 0), _mk(ZA1, 1), _mk(ZB1, 0), _mk(ZB1, 1)]
        z1s = [_mk(ZA2, 0), _mk(ZA2, 1), _mk(ZB2, 0), _mk(ZB2, 1)]
        return z0s, z1s, (t1, t2)

    def pair_rot_bwd(self, lf01, af01, c_ap, s_ap, label, dve_seed=False):
        """Rz-rotate (lf0,lf1) and (af0,af1): z0 = c y0 - s y1; z1 = s y0 + c y1.

        Raw force exprs land exactly (consts folded, lead coefs in the
        ACT-replicated trig scales) in one [lf0|lf1|af0|af1] tile; the two
        rotations then cost 4 wide products + 2 wide combines on [P,2F]
        instead of 8 narrow products + 4 chain materializations.
        Returns ((RzLf0e, RzLf1e), (RzAf0e, RzAf1e)) as pure plane exprs.
        """
        T = self.wscratch(4)
        coefs = []
        for k, e in enumerate([lf01[0], lf01[1], af01[0], af01[1]]):
            coefs.append(self.mat_rel(e, T[:, k * F:(k + 1) * F],
                                      dve_seed=dve_seed))
        aL0, aL1, aA0, aA1 = coefs
        Tr = T.rearrange("p (a b f) -> p a b f", a=2, b=2, f=F)
        Av = Tr[:, :, 0, :]   # [lf0 | af0]
        Bv = Tr[:, :, 1, :]   # [lf1 | af1]
        R1 = self.wscratch(4)  # [c*aL0 | c*aA0 | -s*aL1 | -s*aA1]
        self.act_copy(R1[:, 0:F], c_ap, scale=aL0)
        self.act_copy(R1[:, F:2 * F], c_ap, scale=aA0)
        self.act_copy(R1[:, 2 * F:3 * F], s_ap, scale=-2.0 * aL1)
        self.act_copy(R1[:, 3 * F:4 * F], s_ap, scale=-2.0 * aA1)
        R2 = self.wscratch(4)  # [s*aL0 | s*aA0 | c*aL1 | c*aA1]
        self.act_copy(R2[:, 0:F], s_ap, scale=2.0 * aL0)
        self.act_copy(R2[:, F:2 * F], s_ap, scale=2.0 * aA0)
        self.act_copy(R2[:, 2 * F:3 * F], c_ap, scale=aL1)
        self.act_copy(R2[:, 3 * F:4 * F], c_ap, scale=aA1)
        v = self.nc.vector
        Pt = self.wscratch(4)
        Z = self.wscratch(4)
        v.tensor_tensor(Pt[:, 0:2 * F], Av, R1[:, 0:2 * F], OP.mult)
        v.tensor_tensor(Pt[:, 2 * F:4 * F], Bv, R1[:, 2 * F:4 * F], OP.mult)
        v.tensor_tensor(Z[:, 0:2 * F], Pt[:, 0:2 * F], Pt[:, 2 * F:4 * F],
                        OP.add)
        v.tensor_tensor(Pt[:, 0:2 * F], Av, R2[:, 0:2 * F], OP.mult)
        v.tensor_tensor(Pt[:, 2 * F:4 * F], Bv, R2[:, 2 * F:4 * F], OP.mult)
        v.tensor_tensor(Z[:, 2 * F:4 * F], Pt[:, 0:2 * F], Pt[:, 2 * F:4 * F],
                        OP.add)
        self.n_tt += 6
        self.dve_track(6, 2)

        def _e(sl):
            ap = Z[:, sl * F:(sl + 1) * F]
            self.pers_ids.add(id(ap))
            e = Expr([(1.0, ap)])
            e._mat = (1.0, ap)
            return e

        return (_e(0), _e(2)), (_e(1), _e(3))

    def cross_packed(self, cycA, cycB):
        """cross(A, B) of two cyc tiles -> [P,3F] tile [c0|c1|c2]."""
        Pa = self.wscratch(3)
        Pb = self.wscratch(3)
        self.nc.vector.tensor_tensor(
            Pa, cycA[:, F:4 * F], cycB[:, 2 * F:5 * F], OP.mult)
        self.nc.vector.tensor_tensor(
            Pb, cycA[:, 2 * F:5 * F], cycB[:, F:4 * F], OP.mult)
        self.nc.vector.tensor_tensor(Pa, Pa, Pb, OP.subtract)
        self.n_tt += 3
        self.dve_track(3, 3)
        return Pa

    def free_expr_vec(self, vec):
        for e in vec:
            for _, ap in e.terms:
                ent = self.ap_tag.pop(id(ap), None)
                if ent is None:
                    continue
                self.pers_ids.discard(id(ap))
                if ent[0] == "p":
                    self.free_tags.append(ent[1])
                else:
                    self.free_wide.setdefault(ent[1], []).append(ent[2])

    def joint_boundary(self):
        self.max_joint_allocs = max(self.max_joint_allocs, self.joint_allocs)
        self.joint_allocs = 0

    # ---- expression ops ----
    def lin(self, *pairs, const=0.0):
        acc = {}
        aps = {}
        c_acc = float(const)
        for coef, e in pairs:
            if coef == 0.0 or e is None or e is ZERO and e.const == 0.0:
                if e is not None:
                    c_acc += coef * e.const
                continue
            c_acc += coef * e.const
            for tc, ap in e.terms:
                k = id(ap)
                acc[k] = acc.get(k, 0.0) + coef * tc
                aps[k] = ap
        terms = [(c, aps[k]) for k, c in acc.items() if c != 0.0]
        return Expr(terms, c_acc)

    def mat(self, e, dst=None):
        """Materialize sum-of-terms: e == coef*ap + e.const -> (coef, ap)."""
        assert e.terms, "cannot materialize empty expr"
        if e._mat is not None and dst is None:
            return e._mat
        terms = sorted(e.terms, key=lambda t: -abs(t[0]))
        if len(terms) == 1 and dst is None:
            e._mat = (terms[0][0], terms[0][1])
            return e._mat
        c0, x0 = terms[0]
        if len(terms) == 1:
            self.nc.vector.tensor_scalar(dst, x0, 1.0, None, OP.mult)
            self.n_copy += 1
            e._mat = (c0, dst)
            return e._mat
        if not hasattr(self, "mat_hist"):
            self.mat_hist = {}
        self.mat_hist[len(terms)] = self.mat_hist.get(len(terms), 0) + 1
        r = self._pe_chain(terms, dst, 1.0, 0.0, seed_act=False)
        if r is not None:
            e._mat = (r[1], r[0])
            return e._mat
        t = dst if dst is not None else self.scratch()
        c1, x1 = terms[1]
        eng = self.pick_engine(len(terms) - 1)
        eng.scalar_tensor_tensor(t, x1, c1 / c0, x0, OP.mult, OP.add)
        self.n_stt += 1
        for ck, xk in terms[2:]:
            eng.scalar_tensor_tensor(t, xk, ck / c0, t, OP.mult, OP.add)
            self.n_stt += 1
        e._mat = (c0, t)
        return e._mat

    def mul(self, x, y):
        if not _nonzero(x) or not _nonzero(y):
            return ZERO
        if not x.terms:  # pure const
            return Expr([(x.const * c, ap) for c, ap in y.terms],
                        x.const * y.const)
        if not y.terms:
            return Expr([(y.const * c, ap) for c, ap in x.terms],
                        x.const * y.const)
        cx, ax = self.mat(x)
        cy, ay = self.mat(y)
        if ax.space == bass.MemorySpace.PSUM \
                and ay.space == bass.MemorySpace.PSUM:
            tmp = self.scratch()
            self.nc.vector.tensor_scalar(tmp, ay, 1.0, None, OP.mult)
            self.n_stt += 1
            self.busy["dve"] += self.dve_evac
            ay = tmp
        prod = self.scratch()
        self.pick_engine(1).tensor_tensor(prod, ax, ay, OP.mult)
        self.n_tt += 1
        terms = [(cx * cy, prod)]
        if y.const != 0.0:
            terms.append((cx * y.const, ax))
        if x.const != 0.0:
            terms.append((cy * x.const, ay))
        return Expr(terms, x.const * y.const)

    def snap(self, e, label="", scratch_ok=False):
        """Materialize into a stable plane; returns single-term Expr."""
        if not e.terms:
            return e
        if len(e.terms) == 1 and e._mat is None \
                and id(e.terms[0][1]) in self.pers_ids and not scratch_ok:
            out = Expr(list(e.terms), e.const)
            out._mat = e.terms[0]
            return out
        if e._mat is not None:
            c, src = e._mat
            if id(src) in self.pers_ids or scratch_ok:
                out = Expr([(c, src)], e.const)
                out._mat = (c, src)
                return out
            dst = self.persistent(label)
            self.nc.scalar.activation(dst, src, AF.Copy, bias=0.0, scale=1.0)
            self.n_copy += 1
            out = Expr([(c, dst)], e.const)
            out._mat = (c, dst)
            return out
        dst = self.scratch() if scratch_ok else self.persistent(label)
        c, ap = self.mat(e, dst=dst)
        out = Expr([(c, ap)], e.const)
        out._mat = (c, ap)
        return out

    def snap_to(self, e, dst_ap):
        """Materialize into the given plane; returns single-term Expr."""
        assert e.terms
        c, ap = self.mat(e, dst=dst_ap)
        self.pers_ids.add(id(ap))
        out = Expr([(c, ap)], e.const)
        out._mat = (c, ap)
        return out

    def snap_vec(self, vec, label="", scratch_ok=False):
        return [self.snap(e, f"{label}{i}", scratch_ok) for i, e in enumerate(vec)]

    # ---- 3-vector helpers ----
    def vadd(self, *vecs):
        return [self.lin(*[(1.0, v[i]) for v in vecs]) for i in range(3)]

    def vsub(self, a, b):
        return [self.lin((1.0, a[i]), (-1.0, b[i])) for i in range(3)]

    def cross_const(self, t, X):
        return [
            self.lin((-t[2], X[1]), (t[1], X[2])),
            self.lin((t[2], X[0]), (-t[0], X[2])),
            self.lin((-t[1], X[0]), (t[0], X[1])),
        ]

    def cross_ee(self, A, B):
        return [
            self.lin((1.0, self.mul(A[1], B[2])), (-1.0, self.mul(A[2], B[1]))),
            self.lin((1.0, self.mul(A[2], B[0])), (-1.0, self.mul(A[0], B[2]))),
            self.lin((1.0, self.mul(A[0], B[1])), (-1.0, self.mul(A[1], B[0]))),
        ]

    def matvec_const(self, M, X):
        return [
            self.lin((M[i, 0], X[0]), (M[i, 1], X[1]), (M[i, 2], X[2]))
            for i in range(3)
        ]


def build_module(params):
    trans = np.asarray(params["trans"], np.float64)
    rot_fix = np.asarray(params["rot_fix"], np.float64)
    mass = np.asarray(params["mass"], np.float64)
    com = np.asarray(params["com"], np.float64)
    inertia = np.asarray(params["inertia"], np.float64)
    damping = np.asarray(params["damping"], np.float64)

    nc = bacc.Bacc("TRN2", target_bir_lowering=False, debug=False,
                   num_devices=N_CORES)
    q_d = nc.dram_tensor("q", (BC, ND), DT, kind="ExternalInput")
    qd_d = nc.dram_tensor("qd", (BC, ND), DT, kind="ExternalInput")
    qdd_d = nc.dram_tensor("qdd_des", (BC, ND), DT, kind="ExternalInput")
    use_pe = os.environ.get("K_PE", "1") == "1"
    wmax = int(os.environ.get("K_WMAX", "46"))
    n_waves = int(os.environ.get("K_NWAVES", "9"))
    if use_pe:
        w_d = nc.dram_tensor("wconst", (P, n_waves * (wmax // 2) * P), DT,
                             kind="ExternalInput")
    tq_d = nc.dram_tensor("torque", (BC, ND), DT, kind="ExternalOutput")

    with tile_mod.TileContext(nc) as tc:
        with tc.tile_pool(name="main", bufs=1) as pool, \
             tc.tile_pool(name="io", bufs=1) as io_pool, \
             tc.psum_pool(name="psp", bufs=1) as pspool:
            wtile_ap = None
            if use_pe:
                wtile = io_pool.tile([P, wmax * P], DT, tag="wconst",
                                     name="wconst_sb")
                wtile_ap = wtile[:, :]
            b = Builder(nc, pool,
                        ring_size=int(os.environ.get("K_RING", "11")),
                        pool_frac=float(os.environ.get("K_POOL_FRAC", "0")),
                        pspool=pspool, wtile=wtile_ap, wmax=wmax)
            b.n_waves = n_waves
            if use_pe:
                b.w_d = w_d

            q_t = io_pool.tile([P, F * ND], DT, tag="q", name="q_sb")
            qd_t = io_pool.tile([P, F * ND], DT, tag="qd", name="qd_sb")
            qdd_t = io_pool.tile([P, F * ND], DT, tag="qdd", name="qdd_sb")
            # out aliases q's buffer: q is fully consumed by the trig
            # prologue long before the first backward write_out.
            out_t = io_pool.tile([P, F * ND], DT, tag="q", name="out_sb")
            dram_view = lambda t: t.ap().rearrange("(p f) d -> p (f d)", p=P)
            # q gates trig (the whole critical path): give it the sync queue
            # alone; qd/qdd ride the idle gpsimd queue in parallel.
            nc.sync.dma_start(q_t[:, :], dram_view(q_d))
            nc.gpsimd.dma_start(qd_t[:, :], dram_view(qd_d))
            nc.gpsimd.dma_start(qdd_t[:, :], dram_view(qdd_d))
            if use_pe:
                # weight slots are allocated in first-use (= emission) order,
                # so chunked DMA in slot order arrives before consumers.
                # Waves 0 and 1 load up front; later waves prefetch 1 ahead.
                b._wave_dma(0)
                if n_waves > 1:
                    b._wave_dma(1)

            q3 = q_t[:, :].rearrange("p (f d) -> p f d", d=ND)
            qd3 = qd_t[:, :].rearrange("p (f d) -> p f d", d=ND)
            qdd3 = qdd_t[:, :].rearrange("p (f d) -> p f d", d=ND)
            out3 = out_t[:, :].rearrange("p (f d) -> p f d", d=ND)

            def as_pers_expr(ap):
                b.pers_ids.add(id(ap))
                e = Expr([(1.0, ap)])
                e._mat = (1.0, ap)
                return e

            def const_col(tag, val):
                t = io_pool.tile([P, 1], DT, tag=tag, name=tag)
                nc.vector.memset(t[:, :], float(val))
                return t

            zero_t = const_col("zconst", 0.0)
            # dummy Sin on an immediately-ready tile: hoists the ~2.7us ACT
            # table load to t~0, overlapping the input DMA instead of
            # serializing after it.
            warm_t = io_pool.tile([P, 1], DT, tag="warm", name="warm")
            nc.scalar.activation(warm_t[:, :], zero_t[:, :], AF.Sin,
                                 bias=zero_t[:, :], scale=1.0)

            # ACT Sin has no range reduction (accurate only on [-pi, pi]).
            # q ~ N(0,1) so |q| <= ~5.5: one conditional wrap of 2*pi covers
            # sin(q) and cos(q) = sin((q + pi/2) wrapped).
            PI = float(np.pi)
            TWO_PI = float(2 * np.pi)
            trig = {}
            trig_raw = {}
            _kp = os.environ.get("K_PACK", "0")
            use_pack_fwd = _kp in ("1", "fwd")
            use_pack_bwd = _kp in ("1", "bwd")
            use_xpack = os.environ.get("K_XPACK", "1") == "1"
            def emit_trig(j):
                # ACT Sin is only accurate on [-pi, pi] (no range reduction)
                # and q ~ N(0,1) exceeds pi. Half-angle identities avoid any
                # wrapping: |q/2| <= ~2.9 < pi and |q/4| < pi/2 always.
                #   cos q = 1 - 2 sin^2(q/2)
                #   sin q = 2 sin(q/2) cos(q/2),  cos(q/2) = 1 - 2 sin^2(q/4)
                # Everything runs on the otherwise-idle ACT engine except one
                # DVE product for sin. Emitted lazily (joint j+1's chain goes
                # out with joint j's body) so the in-order ACT queue reaches
                # each joint's replicated-trig copies just before the DVE
                # needs them, instead of front-loading all six chains.
                if j in trig:
                    return
                qj = q3[:, :, j]
                t4w = 4 if (use_pack_fwd or use_pack_bwd) else 2
                t4, _t4tag = b.persistent_wide(t4w, f"t4_{j}")
                c_ap = t4[:, 0:F]
                s_ap = t4[:, F:2 * F]
                b.pers_ids.add(id(c_ap))
                b.pers_ids.add(id(s_ap))
                sh = b.scratch(DT)   # sin(q/2)
                nc.scalar.activation(sh, qj, AF.Sin, bias=zero_t[:, :],
                                     scale=0.5)
                sq = b.scratch(DT)   # sin(q/4)
                nc.scalar.activation(sq, qj, AF.Sin, bias=zero_t[:, :],
                                     scale=0.25)
                sq2 = b.scratch(DT)  # sin^2(q/4)
                nc.scalar.activation(sq2, sq, AF.Square)
                ch = b.scratch(DT)   # cos(q/2)
                nc.scalar.activation(ch, sq2, AF.Copy, bias=1.0, scale=-2.0)
                sh2 = b.scratch(DT)  # sin^2(q/2)
                nc.scalar.activation(sh2, sh, AF.Square)
                nc.scalar.activation(c_ap, sh2, AF.Copy, bias=1.0, scale=-2.0)
                nc.vector.tensor_tensor(s_ap, sh, ch, OP.mult)  # sin q / 2
                if use_pack_fwd or use_pack_bwd:
                    # slots 2,3: [-sin/2 | cos] for packed-rotation patterns
                    nc.scalar.activation(t4[:, 2 * F:3 * F], s_ap, AF.Copy,
                                         bias=0.0, scale=-1.0)
                    nc.scalar.activation(t4[:, 3 * F:4 * F], c_ap, AF.Copy,
                                         bias=0.0, scale=1.0)
                    b.n_act += 2
                b.n_tt += 1
                b.n_act += 6
                se = Expr([(2.0, s_ap)])
                se._mat = (2.0, s_ap)
                trig[j] = (as_pers_expr(c_ap), se)
                trig_raw[j] = t4

            emit_trig(0)
            emit_trig(1)
            b.joint_boundary()



            # qd repack on DVE, after the sin products in the in-order DVE
            # stream (qd's DMA rides the slower gpsimd queue; putting these
            # first would block the trig products behind that DMA).
            qd_pl = []
            for j in range(ND - 1):  # qd_6 is only read once (tau_6)
                d_ap = b.persistent(f"qd{j}")
                nc.vector.tensor_scalar(d_ap, qd3[:, :, j], 1.0, None, OP.mult)
                qd_pl.append(as_pers_expr(d_ap))

            def qdd_expr(j):
                # strided read (28B stride) is cheapest, measured against all
                # repack routes: ACT copy queues behind the trig prologue
                # (+5us); DVE copy costs more than the ~190ns/op penalty;
                # SBUF->SBUF de-stride DMA on the sync queue is ~19x derated
                # and its sem deps stall the forward chains (+23us).
                ap = qdd3[:, :, j]
                e = Expr([(1.0, ap)])
                e._mat = (1.0, ap)
                b.pers_ids.add(id(ap))
                return e

            def rot_inv(j, X):
                """Rz(q_j)^T @ (rot_fix_j^T @ X)"""
                if not any(_nonzero(e) for e in X):
                    return [ZERO, ZERO, ZERO]
                Fm = rot_fix[j].T
                Y = b.matvec_const(Fm, X)
                c, s = trig[j]
                z0 = b.lin((1.0, b.mul(c, Y[0])), (1.0, b.mul(s, Y[1])))
                z1 = b.lin((-1.0, b.mul(s, Y[0])), (1.0, b.mul(c, Y[1])))
                return [z0, z1, Y[2]]

            def write_out(j, e):
                dst = out3[:, :, j]
                if not e.terms:
                    b.nc.vector.memset(dst, float(e.const))
                    return
                c, ap = b.mat(e)
                nc.scalar.activation(dst, ap, AF.Copy, bias=float(e.const),
                                     scale=float(c))
                b.n_act += 1

            # ---------------- forward (joints 0..5) ----------------
            w_p = [ZERO, ZERO, ZERO]
            v_p = [ZERO, ZERO, ZERO]
            a_p = [ZERO, ZERO, ZERO]
            la_p = [ZERO, ZERO, Expr(const=GRAV)]
            states = []
            use_qpack = os.environ.get("K_QPACK", "1") == "1"
            fwd_ztags = {}
            for j in range(ND - 1):
                t_j = trans[j]
                Uv = b.vsub(v_p, b.cross_const(t_j, w_p))
                Ua = b.vsub(la_p, b.cross_const(t_j, a_p))
                Uv = [b.snap(e, f"Uv{j}", scratch_ok=True)
                      if len(e.terms) > 1 else e for e in Uv]
                Ua = [b.snap(e, f"Ua{j}", scratch_ok=True)
                      if len(e.terms) > 1 else e for e in Ua]
                packed_done = False
                if use_qpack and j > 0:
                    Fm = rot_fix[j].T
                    Yw = b.matvec_const(Fm, w_p)
                    Yv = b.matvec_const(Fm, Uv)
                    Ya = b.matvec_const(Fm, a_p)
                    Yu = b.matvec_const(Fm, Ua)
                    pairs = [(Yw[0], Yw[1]), (Yv[0], Yv[1]),
                             (Ya[0], Ya[1]), (Yu[0], Yu[1])]
                    if all(e.terms for pr in pairs for e in pr):
                        t4 = trig_raw[j]
                        z0s, z1s, ztags = b.quad_rot(
                            pairs, t4[:, 0:F], t4[:, F:2 * F], f"q{j}")
                        Rw = [z0s[0], z1s[0], Yw[2]]
                        Rv = [z0s[1], z1s[1], Yv[2]]
                        Ra = [z0s[2], z1s[2], Ya[2]]
                        Rla = [z0s[3], z1s[3], Yu[2]]
                        fwd_ztags[j] = ztags
                        packed_done = True
                _kpj = int(os.environ.get("K_PACK_J", "-1"))
                if not packed_done and use_pack_fwd and j > 0 \
                        and (_kpj < 0 or j == _kpj):
                    Fm = rot_fix[j].T

                    def _rows(X):
                        return [b.lin((Fm[i, 0], X[0]), (Fm[i, 1], X[1]),
                                      (Fm[i, 2], X[2])) for i in range(3)]

                    Yw, Yv, Ya, Yu = _rows(w_p), _rows(Uv), _rows(a_p), _rows(Ua)
                    heads = [Yw[0], Yw[1], Yv[0], Yv[1],
                             Ya[0], Ya[1], Yu[0], Yu[1]]
                    if all(e.terms for e in heads):
                        t4 = trig_raw[j]
                        zw = b.packed_pair_rot(Yw[0], Yw[1], t4, +1,
                                               b.persistent(f"w{j}0"),
                                               b.persistent(f"w{j}1"), True)
                        zv = b.packed_pair_rot(Yv[0], Yv[1], t4, +1,
                                               b.persistent(f"v{j}0"),
                                               b.persistent(f"v{j}1"), True)
                        za = b.packed_pair_rot(Ya[0], Ya[1], t4, +1,
                                               b.scratch(), b.scratch(), False)
                        zu = b.packed_pair_rot(Yu[0], Yu[1], t4, +1,
                                               b.scratch(), b.scratch(), False)
                        Rw = [zw[0], zw[1], Yw[2]]
                        Rv = [zv[0], zv[1], Yv[2]]
                        Ra = [za[0], za[1], Ya[2]]
                        Rla = [zu[0], zu[1], Yu[2]]
                        packed_done = True
                if not packed_done:
                    Rw = rot_inv(j, w_p)
                    Rv = rot_inv(j, Uv)
                    Ra = rot_inv(j, a_p)
                    Rla = rot_inv(j, Ua)
                qdj = qd_pl[j]
                qddj = qdd_expr(j)
                w = [Rw[0], Rw[1], b.lin((1.0, Rw[2]), (1.0, qdj))]
                w = b.snap_vec(w, f"w{j}_")
                v = b.snap_vec(Rv, f"v{j}_")
                dw = [
                    b.lin((1.0, Ra[0]), (1.0, b.mul(w[1], qdj))),
                    b.lin((1.0, Ra[1]), (-1.0, b.mul(w[0], qdj))),
                    b.lin((1.0, Ra[2]), (1.0, qddj)),
                ]
                dv = [
                    b.lin((1.0, Rla[0]), (1.0, b.mul(v[1], qdj))),
                    b.lin((1.0, Rla[1]), (-1.0, b.mul(v[0], qdj))),
                    Rla[2],
                ]
                dw = b.snap_vec(dw, f"dw{j}_")
                dv = b.snap_vec(dv, f"dv{j}_")
                states.append((w, v, dw, dv))
                w_p, v_p, a_p, la_p = w, v, dw, dv
                if j + 2 < ND - 1:
                    emit_trig(j + 2)
                if j in (1, 3):
                    b.new_wave()
                b.joint_boundary()

            # ---------------- backward (j = 5..0) ----------------
            # tau_6 = damping_6 * qd_6 is qd_6's only read: use the strided
            # column directly instead of a dense repack copy.
            qd6 = Expr([(1.0, qd3[:, :, ND - 1])])
            qd6._mat = (1.0, qd3[:, :, ND - 1])
            b.pers_ids.add(id(qd6.terms[0][1]))
            write_out(ND - 1, b.lin((damping[ND - 1], qd6)))

            lin_f = [ZERO, ZERO, ZERO]
            ang_f = [ZERO, ZERO, ZERO]
            bw_pack = None
            use_brot = os.environ.get("K_BROT", "1") == "1"
            use_dveseed = os.environ.get("K_DVESEED", "0") == "1"
            for j in range(ND - 2, -1, -1):
                if j in (5, 4, 3, 2, 1):
                    b.new_wave()
                have_child = any(_nonzero(e) for e in lin_f + ang_f)
                if have_child and use_pack_bwd and bw_pack is not None:
                    bw_tile_p, lfc, afc, prev_tag = bw_pack
                    t4 = trig_raw[j + 1]
                    Rf = rot_fix[j + 1]
                    zlf = b.packed_pair_rot(
                        None, None, t4, -1, b.scratch(), b.scratch(), False,
                        A_ready=(bw_tile_p[:, 0:2 * F],) + lfc)
                    zaf = b.packed_pair_rot(
                        None, None, t4, -1, b.scratch(), b.scratch(), False,
                        A_ready=(bw_tile_p[:, 2 * F:4 * F],) + afc)
                    b.free_wide.setdefault(4, []).append(prev_tag)
                    RzLf = [zlf[0], zlf[1], lin_f[2]]
                    RzAf = [zaf[0], zaf[1], ang_f[2]]
                    Rc_lf = b.matvec_const(Rf, RzLf)
                    Rc_lf = [b.snap(e, f"rclf{j}", scratch_ok=True)
                             if len(e.terms) > 2 and (j > 0 or i < 2) else e
                             for i, e in enumerate(Rc_lf)]
                    Rc_af = b.matvec_const(Rf, RzAf)
                    child_ang = b.vadd(b.cross_const(trans[j + 1], Rc_lf), Rc_af)
                    child_lin = Rc_lf
                elif have_child and use_brot:
                    Rf = rot_fix[j + 1]
                    t4c = trig_raw[j + 1]
                    lfp, afp = b.pair_rot_bwd(
                        (lin_f[0], lin_f[1]), (ang_f[0], ang_f[1]),
                        t4c[:, 0:F], t4c[:, F:2 * F], f"br{j}",
                        dve_seed=(j <= 1 and use_dveseed))
                    RzLf = [lfp[0], lfp[1], lin_f[2]]
                    RzAf = [afp[0], afp[1], ang_f[2]]
                    Rc_lf = b.matvec_const(Rf, RzLf)
                    Rc_lf = [b.snap(e, f"rclf{j}", scratch_ok=True)
                             if len(e.terms) > 2 and (j > 0 or i < 2) else e
                             for i, e in enumerate(Rc_lf)]
                    Rc_af = b.matvec_const(Rf, RzAf)
                    child_ang = b.vadd(b.cross_const(trans[j + 1], Rc_lf), Rc_af)
                    child_lin = Rc_lf
                elif have_child:
                    cs, ss = trig[j + 1]
                    Rf = rot_fix[j + 1]
                    # xy-rotation outputs fan out 3x through the rot_fix
                    # matvec: snapping them saves (t-1)(f-1) chain slots.
                    RzLf = [
                        b.snap(b.lin((1.0, b.mul(cs, lin_f[0])),
                                     (-1.0, b.mul(ss, lin_f[1]))),
                               f"rzlf{j}0", scratch_ok=True),
                        b.snap(b.lin((1.0, b.mul(ss, lin_f[0])),
                                     (1.0, b.mul(cs, lin_f[1]))),
                               f"rzlf{j}1", scratch_ok=True),
                        lin_f[2],
                    ]
                    Rc_lf = b.matvec_const(Rf, RzLf)
                    Rc_lf = [b.snap(e, f"rclf{j}", scratch_ok=True)
                             if len(e.terms) > 2 and (j > 0 or i < 2) else e
                             for i, e in enumerate(Rc_lf)]
                    RzAf = [
                        b.snap(b.lin((1.0, b.mul(cs, ang_f[0])),
                                     (-1.0, b.mul(ss, ang_f[1]))),
                               f"rzaf{j}0", scratch_ok=True),
                        b.snap(b.lin((1.0, b.mul(ss, ang_f[0])),
                                     (1.0, b.mul(cs, ang_f[1]))),
                               f"rzaf{j}1", scratch_ok=True),
                        ang_f[2],
                    ]
                    Rc_af = b.matvec_const(Rf, RzAf)
                    child_ang = b.vadd(b.cross_const(trans[j + 1], Rc_lf), Rc_af)
                    child_lin = Rc_lf
                else:
                    child_ang = [ZERO, ZERO, ZERO]
                    child_lin = [ZERO, ZERO, ZERO]

                # free previous joint's planes only after the child rotation
                # has consumed the raw lf/af term planes
                if j != ND - 2:
                    for vec in states[j + 1]:
                        b.free_expr_vec(vec)
                    if j + 1 in fwd_ztags:
                        for tag in fwd_ztags.pop(j + 1):
                            b.free_wide.setdefault(2, []).append(tag)

                w, v, dw, dv = states[j]
                m = float(mass[j])
                mc = m * com[j]
                cxm = np.array([
                    [0.0, -com[j][2], com[j][1]],
                    [com[j][2], 0.0, -com[j][0]],
                    [-com[j][1], com[j][0], 0.0],
                ])
                Isp = inertia[j] + m * (cxm @ cxm.T)

                if j == 0 and not _nonzero(w[0]) and not _nonzero(w[1]) \
                        and not any(_nonzero(e) for e in v):
                    # Base joint: only ang_f.z feeds torque_0 and nothing
                    # consumes lin_f_0/ang_f_0 further down. With w=(0,0,qd0)
                    # and v=0, tmp_a.z = (w x IcV_a).z + (v x IcV_l).z = 0,
                    # so af_z = (Isp dw + mc x dv).z + child_ang.z only.
                    af2 = b.lin(
                        (Isp[2, 0], dw[0]), (Isp[2, 1], dw[1]),
                        (Isp[2, 2], dw[2]),
                        (mc[0], dv[1]), (-mc[1], dv[0]),
                        (1.0, child_ang[2]))
                    write_out(0, b.lin((1.0, af2), (damping[0], qd_pl[0])))
                    b.joint_boundary()
                    continue

                IcA_l = b.vsub([b.lin((m, dv[i])) for i in range(3)],
                               b.cross_const(mc, dw))
                IcA_a = b.vadd(b.matvec_const(Isp, dw), b.cross_const(mc, dv))
                IcV_l = b.vsub([b.lin((m, v[i])) for i in range(3)],
                               b.cross_const(mc, w))
                IcV_a = b.vadd(b.matvec_const(Isp, w), b.cross_const(mc, v))
                if use_xpack:
                    # Packed crosses: operands land EXACTLY (coef 1, const
                    # folded) in cyclically-duplicated [x0|x1|x2|x0|x1]
                    # tiles; each cross is then 2 wide products + 1 wide
                    # subtract instead of 6 narrow products + per-component
                    # chain slots.
                    tail = j <= 1 and use_dveseed
                    cw, cw_tag = b.cyc_tile(w, f"cw{j}", dve_seed=tail)
                    cv, cv_tag = b.cyc_tile(v, f"cv{j}", dve_seed=tail)
                    cva, cva_tag = b.cyc_tile(IcV_a, f"cva{j}", dve_seed=tail)
                    cvl, cvl_tag = b.cyc_tile(IcV_l, f"cvl{j}", dve_seed=tail)
                    C1 = b.cross_packed(cw, cva)
                    C2 = b.cross_packed(cv, cvl)
                    C3 = b.cross_packed(cw, cvl)
                    nc.vector.tensor_tensor(C1, C1, C2, OP.add)
                    b.n_tt += 1

                    def _slices(t):
                        out = []
                        for k in range(3):
                            ap = t[:, k * F:(k + 1) * F]
                            b.pers_ids.add(id(ap))
                            e = Expr([(1.0, ap)])
                            e._mat = (1.0, ap)
                            out.append(e)
                        return out

                    tmp_a = _slices(C1)
                    tmp_l = _slices(C3)
                    for tag in (cw_tag, cv_tag, cva_tag, cvl_tag):
                        b.free_wide.setdefault(5, []).append(tag)
                else:
                    tmp_a = b.vadd(b.cross_ee(w, IcV_a), b.cross_ee(v, IcV_l))
                    tmp_l = b.cross_ee(w, IcV_l)
                lf_new = b.vadd(IcA_l, tmp_l, child_lin)
                af_new = b.vadd(IcA_a, tmp_a, child_ang)
                if use_pack_bwd and j > 0:
                    bw_tile, bw_tag = b.persistent_wide(4, f"bw{j}")
                    lf0 = b.snap_to(lf_new[0], bw_tile[:, 0:F])
                    lf1 = b.snap_to(lf_new[1], bw_tile[:, F:2 * F])
                    af0 = b.snap_to(af_new[0], bw_tile[:, 2 * F:3 * F])
                    af1 = b.snap_to(af_new[1], bw_tile[:, 3 * F:4 * F])
                    lf2 = b.snap(lf_new[2], f"lf{j}2")
                    af2 = b.snap(af_new[2], f"af{j}2")
                    lin_f = [lf0, lf1, lf2]
                    ang_f = [af0, af1, af2]
                    bw_pack = (bw_tile,
                               (lf0._mat[0], lf1._mat[0],
                                lf0.const, lf1.const),
                               (af0._mat[0], af1._mat[0],
                                af0.const, af1.const), bw_tag)
                elif use_brot:
                    # carry lf/af xy raw: the next joint's packed rotation
                    # materializes them directly into its operand tile
                    lin_f = [lf_new[0], lf_new[1],
                             b.snap(lf_new[2], f"lf{j}2")]
                    ang_f = [af_new[0], af_new[1],
                             b.snap(af_new[2], f"af{j}2")]
                    bw_pack = None
                else:
                    lin_f = b.snap_vec(lf_new, f"lf{j}_")
                    ang_f = b.snap_vec(af_new, f"af{j}_")
                    bw_pack = None
                write_out(j, b.lin((1.0, ang_f[2]), (damping[j], qd_pl[j])))
                b.joint_boundary()

            nc.sync.dma_start(dram_view(tq_d), out_t[:, :])

            stats = dict(stt=b.n_stt, tt=b.n_tt, act=b.n_act, copy=b.n_copy,
                         mm=b.n_mm, nw=[len(w) for w in b.wave_coefs],
                         busy={k: round(v / 1000.0, 1)
                               for k, v in b.busy.items()},
                         pers=b.pers_idx, max_joint_allocs=b.max_joint_allocs,
                         ring=b.ring_size,
                         mat_hist=dict(sorted(getattr(b, "mat_hist", {}).items())))
            stats["wcoefs"] = [list(w) for w in b.wave_coefs]
            stats["wmax"] = wmax if use_pe else 0
            stats["n_waves"] = n_waves

    nc.compile()
    return nc, stats


_CACHE = {}


def _get_module(params):
    import hashlib
    key = b"".join(np.ascontiguousarray(np.asarray(params[k], np.float32)).tobytes()
                   for k in ("trans", "rot_fix", "mass", "com", "inertia",
                             "damping"))
    h = hashlib.sha1(key).hexdigest()
    if h not in _CACHE:
        _CACHE[h] = build_module(params)
    return _CACHE[h]


def run(q, qd, qdd_des, trans, rot_fix, mass, com, inertia, damping,
        trace=False):
    q = np.asarray(q)
    qd = np.asarray(qd)
    qdd_des = np.asarray(qdd_des)
    assert q.shape == (B_TOTAL, ND), f"unexpected q shape {q.shape}"
    assert qd.shape == (B_TOTAL, ND) and qdd_des.shape == (B_TOTAL, ND)
    params = dict(trans=trans, rot_fix=rot_fix, mass=mass, com=com,
                  inertia=inertia, damping=damping)
    nc, stats = _get_module(params)
    wconst = None
    if stats.get("wmax"):
        whalf = stats["wmax"] // 2
        nw = stats["n_waves"]
        wconst = np.zeros((P, nw * whalf * P), np.float32)
        eye = np.eye(P, dtype=np.float32)
        for wv, coefs in enumerate(stats["wcoefs"]):
            for s, cval in enumerate(coefs):
                a = (wv * whalf + s) * P
                wconst[:, a:a + P] = np.float32(cval) * eye
    in_maps = []
    for c in range(N_CORES):
        sl = slice(c * BC, (c + 1) * BC)
        m = {
            "q": np.ascontiguousarray(q[sl], np.float32),
            "qd": np.ascontiguousarray(qd[sl], np.float32),
            "qdd_des": np.ascontiguousarray(qdd_des[sl], np.float32),
        }
        if wconst is not None:
            m["wconst"] = wconst
        in_maps.append(m)
    res = bass_utils.run_bass_kernel_spmd(
        nc, in_maps, core_ids=list(range(N_CORES)), trace=trace)
    out = np.concatenate([res.results[c]["torque"] for c in range(N_CORES)],
                         axis=0)
    return out.astype(np.float32), res, stats


def kernel(q, qd, qdd_des, trans, rot_fix, mass, com, inertia, damping):
    out, _, _ = run(q, qd, qdd_des, trans, rot_fix, mass, com, inertia,
                    damping, trace=False)
    return out



# revision 53
# speedup vs baseline: 1.0228x; 1.0195x over previous
"""Trainium2 Bass kernel for batched 7-DOF RNEA inverse dynamics.

Layout: pure data-parallel over 8 NeuronCores (32768 batch elements each).
Per core, every per-element scalar lives as an SBUF "plane" [128, 256] f32
(batch element e = partition*256 + free). All per-link parameters (trans,
rot_fix, mass, com, inertia, damping) are baked into the instruction stream
as immediates at build time.

The math is emitted through a small expression compiler:
  Expr = sum(coef * plane) + const
Linear combinations are free (term concatenation); they materialize as
chains of fused scalar_tensor_tensor ops ((in0*imm) + in1, in-place
accumulation) only when a product or an output needs a raw plane. Products
are DVE tensor_tensor ops. sin/cos and the final affine outputs go to the
scalar engine (ACT).

Algebraic structure used:
  - Rinv @ x with Rinv = Rz(q)^T @ rot_fix^T: constant matvec folded into
    stt-chain immediates, then one complex rotation (4 products).
  - cross(t_inv, Rinv x) = -Rinv (t x x): constant cross matrices fold into
    linear immediates, killing 12 products per forward joint.
  - forward state for joint 6 is never computed (the reference's backward
    recursion never reads it).
  - ACT Sin has no range reduction (accurate only on [-pi, pi]) and
    q ~ N(0,1) exceeds pi; half-angle identities avoid wrapping:
    cos q = 1-2sin^2(q/2), sin q = 2 sin(q/2)(1-2sin^2(q/4)).

Measured (8 cores, trn2): HW exec ~309 us, max abs err ~8e-5 on torque
absmax ~139 (fp32). Vector engine is the bottleneck (~650 DVE ops after
packing, from ~950); measured DVE op costs: 359 ns/[128,256] STT or TT
(267 ns data + ~92 ns SBUF-access/issue overhead), tensor_scalar 225 ns
(2x_2p), ACT 420 ns and fully parallel with DVE. Wide-op packing exploits
the per-op overhead plus term-count reduction:
  - forward quad-rot (K_QPACK): the 4 per-joint Rz^T rotations run as 4
    wide [P,4F] products against ACT-replicated trig tiles (slot scales
    absorb the mats' lead coefs; consts folded into slot contents via
    ACT-seeded exact chains) + 4 wide combines; outputs are pure planes.
  - backward child rotation (K_BROT): lf/af xy carried as RAW exprs,
    materialized into one [lf0|lf1|af0|af1] tile at the consuming joint,
    rotated with 4+2 wide [P,2F] ops.
  - backward crosses (K_XPACK): w, v, IcV_a, IcV_l land exactly in
    cyclically-duplicated [x0|x1|x2|x0|x1] tiles; each cross is 2 wide
    [P,3F] products + 1 wide subtract.
  - base joint computes only ang_f.z (everything else is dead there).
  - trig chains emitted lazily (joint j+2 with joint j's body) so the
    in-order ACT queue reaches each joint's replicated-trig copies just
    before the DVE needs them; front-loading all six chains costs ~5 us.
Rejected with on-HW measurements: TensorE accumulate (475 ns/[128,256]
matmul term, only ~50% overlap with DVE traffic), GPSIMD (SBUF ports
physically shared with DVE: 2-input ops serialize ~100%), bf16/fp16
(abs-err budget ~3e-3 at near-zero outputs vs ~1e-2+ rounding of O(30)
intermediates), and state-major layouts for PE matvecs (DVE free-dim
cycle cost explodes at <128 active partitions).
"""

import os
import sys

for _p in ("/opt/trn_rl_repo",):
    if os.path.isdir(_p) and _p not in sys.path:
        sys.path.append(_p)

import numpy as np

import concourse.bass as bass
import concourse.bacc as bacc
import concourse.mybir as mybir
from concourse import tile as tile_mod
from concourse import bass_utils

N_CORES = 8
ND = 7
B_TOTAL = 262144
BC = B_TOTAL // N_CORES  # 32768 per core
P = 128
F = BC // P  # 256
GRAV = 9.81
DT = mybir.dt.float32
DT_C = mybir.dt.bfloat16 if os.environ.get("K_BF16", "0") == "1" else mybir.dt.float32
OP = mybir.AluOpType
AF = mybir.ActivationFunctionType


class Expr:
    """value = sum(coef * plane_ap) + const"""

    __slots__ = ("terms", "const", "_mat")

    def __init__(self, terms=(), const=0.0):
        self.terms = list(terms)
        self.const = float(const)
        self._mat = None  # cached (coef, ap) of materialized sum-of-terms


ZERO = Expr()


def _nonzero(e):
    return bool(e.terms) or e.const != 0.0


class Builder:
    def __init__(self, nc, pool, ring_size=64, pool_frac=0.0,
                 pspool=None, wtile=None, wmax=0):
        self.nc = nc
        self.pool = pool
        self.n_stt = 0
        self.n_tt = 0
        self.n_act = 0
        self.n_copy = 0
        self.n_mm = 0
        # engine load balancing between DVE and GPSIMD (2-input ops)
        self.eng_busy = [0.0, 0.0]  # ns accumulated: [vector, gpsimd]
        self.eng_cost = [445.0, 980.0]
        self.pool_frac = pool_frac
        self.ring_size = ring_size
        self.ring_idx = 0
        self.joint_allocs = 0
        self.max_joint_allocs = 0
        self.pers_idx = 0
        self.free_tags = []       # recycled persistent tags
        self.free_wide = {}       # recycled wide tags by slot count
        self.pers_ids = set()     # id(ap) of planes safe to reference long-term
        self.ap_tag = {}          # id(ap) -> tag (for freeing)
        # ---- PE (TensorEngine) chain offload ----
        # Constant-coefficient linear combinations accumulate in PSUM via
        # diag-weight fp32 matmuls (exact: HW does the 4-pass H/L split).
        # Measured: ~490ns/term sustained incl. per-MM LDWEIGHTS, fully
        # parallel with DVE; DVE reads PSUM operands at SBUF cost.
        self.use_pe = wtile is not None and os.environ.get("K_PE", "1") == "1"
        self.pspool = pspool
        self.wtile = wtile
        self.wmax = wmax
        self.wave = 0
        self.wave_coefs = [[]]    # per wave: slot i holds diag(coefs[i])
        self.wslot = {}
        self.w_d = None           # wconst dram tensor (for wave refills)
        self.ps_ring = 0
        self.ps_ntags = int(os.environ.get("K_PSTAGS", "8"))
        # projected engine busy (ns) for greedy makespan routing
        self.busy = {"dve": 0.0, "act": 0.0, "pe": 0.0}
        self.pe_cost = float(os.environ.get("K_PE_COST", "500"))
        self.dve_cost = float(os.environ.get("K_DVE_COST", "424"))
        self.act_evac = 480.0
        self.dve_evac = 258.0

    def dve_track(self, n_ops, width=1):
        self.busy["dve"] += n_ops * (width * 267.0 + 157.0)

    @property
    def whalf(self):
        return self.wmax // 2

    def _wslot_room(self, coefs):
        cur = self.wave_coefs[self.wave]
        new = {c for c in coefs if c not in self.wslot}
        return len(cur) + len(new) <= self.whalf

    def _wslot_ap(self, c):
        s = self.wslot.get(c)
        if s is None:
            cur = self.wave_coefs[self.wave]
            s = len(cur)
            cur.append(c)
            self.wslot[c] = s
        base = (self.wave % 2) * self.whalf
        return self.wtile[:, (base + s) * P:(base + s + 1) * P]

    def _wave_dma(self, wave):
        """DMA wave's coefficient table into its half of the weight tile.

        Waves ping-pong between halves and each wave's DMA is issued one
        wave EARLY (the coefficient values live in DRAM, resolved at run
        time, so emission only needs the offsets): by the time a wave's
        LDWEIGHTS run, its table landed a full wave ago."""
        dram_base = wave * self.whalf * P
        sb_base = (wave % 2) * self.whalf * P
        wq = [self.nc.sync, self.nc.gpsimd] * 3
        nchunk = len(wq)
        bounds = [round(i * self.whalf / nchunk) for i in range(nchunk + 1)]
        for ci in range(nchunk):
            a, bnd = bounds[ci] * P, bounds[ci + 1] * P
            if a < bnd:
                wq[ci].dma_start(self.wtile[:, sb_base + a:sb_base + bnd],
                                 self.w_d.ap()[:, dram_base + a:dram_base + bnd])

    def new_wave(self):
        """Advance to a fresh coefficient table half; prefetch the next."""
        if not self.use_pe or self.wave + 1 >= self.n_waves:
            return
        self.wave += 1
        self.wave_coefs.append([])
        self.wslot = {}
        if self.wave + 1 < self.n_waves:
            self._wave_dma(self.wave + 1)

    def _psum(self):
        """PSUM chain slot. Banks are the allocation granularity (8), so two
        chain slots share each bank tile (has_written bits are per-element,
        so independent accumulation groups coexist in one bank)."""
        if os.environ.get("K_PSHALF", "0") == "1":
            n = self.ps_ring
            self.ps_ring += 1
            half = n % 2
            if half == 0:
                tag = f"ps{(n // 2) % self.ps_ntags}"
                self._ps_cur = self.pspool.tile([P, 2 * F], DT, tag=tag,
                                                name=tag)
            return self._ps_cur[:, half * F:(half + 1) * F]
        tag = f"ps{self.ps_ring % self.ps_ntags}"
        self.ps_ring += 1
        t = self.pspool.tile([P, 2 * F], DT, tag=tag, name=tag)
        return t[:, 0:F]

    def _pe_accum(self, terms):
        """PSUM <- sum(c*ap) over SBUF-resident terms via diag-weight MMs."""
        ps = self._psum()
        n = len(terms)
        for i, (c, ap) in enumerate(terms):
            w = self._wslot_ap(float(c))
            self.nc.tensor.matmul(ps, w, ap, start=(i == 0), stop=(i == n - 1))
        self.n_mm += n
        self.busy["pe"] += n * self.pe_cost
        return ps

    def _pe_split(self, terms):
        """(pe_terms, dve_terms, sigma) under space/slot constraints, or None.

        sigma flips the whole accumulation's sign when -c slots are a better
        match for the live weight table (the flip is undone at evac / in the
        returned coefficient), halving slot burn for +-c coefficient pairs.
        """
        if not self.use_pe:
            return None
        pe, dve = [], []
        for c, ap in terms:
            if ap.space == bass.MemorySpace.SBUF:
                pe.append((float(c), ap))
            else:
                dve.append((c, ap))
        if len(pe) < 2:
            return None
        hits_p = sum(1 for c, _ in pe if c in self.wslot)
        hits_n = sum(1 for c, _ in pe if -c in self.wslot)
        sigma = -1.0 if hits_n > hits_p else 1.0
        pe = [(sigma * c, ap) for c, ap in pe]
        if not self._wslot_room([c for c, _ in pe]):
            return None
        return pe, dve, sigma

    def _evac(self, dst, ps, scale, bias, dve_pref=False):
        ca = self.busy["act"] + self.act_evac
        cd = self.busy["dve"] + self.dve_evac
        mode = os.environ.get("K_EVAC", "auto")
        if mode == "act":
            dve_pref = False
            cd = ca + 1.0
        elif mode == "dve":
            dve_pref = True
        if dve_pref or cd < ca:
            self.nc.vector.tensor_scalar(dst, ps, float(scale), float(bias),
                                         OP.mult, OP.add)
            self.busy["dve"] = cd
            self.n_stt += 1
        else:
            self.nc.scalar.activation(dst, ps, AF.Copy, bias=float(bias),
                                      scale=float(scale))
            self.busy["act"] = ca
            self.n_act += 1

    def _pe_chain(self, terms, dst, scale=1.0, bias=0.0, dve_seed=False,
                  seed_act=True):
        """Try PE route for sum(c*ap)*scale(+bias). Returns result ap or None.

        dst None: result stays in PSUM (requires scale==1, bias==0 handled by
        caller convention). dst given: evacuated into dst (SBUF).
        Split chains merge the DVE-side terms into dst with the PSUM partial
        as a free STT src1 operand.
        """
        limit = int(os.environ.get("K_PE_LIMIT", "100000"))
        if getattr(self, "pe_chains", 0) >= limit:
            return None
        split = self._pe_split(terms)
        if split is None:
            return None
        pe, dve, sigma = split
        n_pe, n_dve = len(pe), len(dve)
        # status quo: whole chain on DVE (+ACT seed when it would use one)
        sq_d = self.busy["dve"] + (len(terms) - 1) * self.dve_cost
        sq_a = self.busy["act"] + (self.act_evac if seed_act else 0.0)
        mk_sq = max(sq_d, sq_a, self.busy["pe"])
        ev = 0.0 if (dst is None and n_dve == 0) else \
            (0.0 if n_dve else min(self.act_evac, self.dve_evac))
        pe_d = self.busy["dve"] + n_dve * self.dve_cost
        pe_p = self.busy["pe"] + n_pe * self.pe_cost
        mk_pe = max(pe_d, self.busy["act"], pe_p) + ev
        if mk_pe >= mk_sq:
            return None
        self.pe_chains = getattr(self, "pe_chains", 0) + 1
        if dst is None and n_dve:
            # merge would need an SBUF dst anyway; use scratch
            dst_eff = self.scratch()
        else:
            dst_eff = dst
        ps = self._pe_accum(pe)  # holds sigma * sum(c_i x_i) over pe terms
        if n_dve == 0:
            if dst_eff is None:
                return ps, sigma
            self._evac(dst_eff, ps, sigma * scale, bias, dve_pref=dve_seed)
            return dst_eff, 1.0
        # DVE merge. The leftover (non-SBUF) terms are PSUM-resident, so the
        # partial must leave PSUM first (one PSUM operand per DVE op).
        self.nc.vector.tensor_scalar(dst_eff, ps, float(sigma * scale),
                                     float(bias), OP.mult, OP.add)
        self.n_stt += 1
        self.busy["dve"] += self.dve_evac
        for ck, xk in dve:
            self.nc.vector.scalar_tensor_tensor(
                dst_eff, xk, float(ck * scale), dst_eff, OP.mult, OP.add)
            self.n_stt += 1
            self.busy["dve"] += self.dve_cost
        return dst_eff, 1.0

    def pick_engine(self, n_ops=1):
        """Pick vector or gpsimd for a chain of n_ops 2-input ops."""
        self.busy["dve"] += n_ops * self.dve_cost
        if self.pool_frac <= 0.0:
            self.eng_busy[0] += n_ops * self.eng_cost[0]
            return self.nc.vector
        c0 = self.eng_busy[0] + n_ops * self.eng_cost[0]
        c1 = self.eng_busy[1] + n_ops * self.eng_cost[1]
        if c1 < c0:
            self.eng_busy[1] = c1
            return self.nc.gpsimd
        self.eng_busy[0] = c0
        return self.nc.vector

    # ---- allocation ----
    def scratch(self, dtype=None):
        tag = f"s{self.ring_idx % self.ring_size}"
        t = self.pool.tile([P, F], dtype or DT_C, tag=tag, name=tag)
        self.ring_idx += 1
        self.joint_allocs += 1
        return t[:, :]

    def persistent(self, label=""):
        if self.free_tags:
            tag = self.free_tags.pop()
        else:
            tag = f"p{self.pers_idx}"
            self.pers_idx += 1
        t = self.pool.tile([P, F], DT_C, tag=tag, name=f"{tag}_{label}")
        ap = t[:, :]
        self.pers_ids.add(id(ap))
        self.ap_tag[id(ap)] = ("p", tag)
        return ap

    def wscratch(self, slots):
        """Wide scratch tile [P, slots*F] from a per-width ring."""
        if not hasattr(self, "wring"):
            self.wring = {}
        idx = self.wring.get(slots, 0)
        self.wring[slots] = idx + 1
        mod = {2: 2, 3: 5, 4: 8}.get(slots, 10)
        tag = f"w{slots}_{idx % mod}"
        t = self.pool.tile([P, slots * F], DT_C, tag=tag, name=tag)
        return t[:, :]

    def persistent_wide(self, slots, label=""):
        if not hasattr(self, "pwide_idx"):
            self.pwide_idx = 0
        fl = self.free_wide.setdefault(slots, [])
        if fl:
            tag = fl.pop()
        else:
            tag = f"pw{slots}_{self.pwide_idx}"
            self.pwide_idx += 1
        t = self.pool.tile([P, slots * F], DT_C, tag=tag, name=f"{tag}_{label}")
        return t[:, :], tag

    def packed_pair_rot(self, Y0, Y1, t4, sign, dst0, dst1, pers,
                        A_ready=None):
        """Rotate one (y0, y1) pair by the z-angle whose 4-slot trig tile is
        t4 = [cos | sin/2 | -sin/2 | cos].

        sign=+1: z0 = c y0 + s y1, z1 = -s y0 + c y1   (Rz^T)
        sign=-1: z0 = c y0 - s y1, z1 = +s y0 + c y1   (Rz)
        dst0/dst1: [P, F] planes receiving z0/a0, z1/a1.
        A_ready: optional (A_region[P,2F], a0, a1, k0, k1) when the pair is
        already materialized adjacently (coefs a*, deferred consts k*).
        Returns (z0_expr, z1_expr) incl. rotated deferred-const terms.
        """
        if A_ready is None:
            W = self.wscratch(2)
            a0, _ = self.mat(Y0, dst=W[:, 0:F])
            a1, _ = self.mat(Y1, dst=W[:, F:2 * F])
            k0, k1 = Y0.const, Y1.const
            A = W
        else:
            A, a0, a1, k0, k1 = A_ready
        c_slot = t4[:, 0:F]
        s_slot = t4[:, F:2 * F]
        if sign > 0:
            B1 = t4[:, 0:2 * F]            # [c | s/2]
            B2 = t4[:, 2 * F:4 * F]        # [-s/2 | c]
            # P1 = [c*y0r | (s/2)*y1r]; P2 = [(-s/2)*y0r | c*y1r]
            # z0 = a0*P1s0 + 2 a1*P1s1 ; z1 = 2 a0*P2s0 + a1*P2s1
        else:
            t4r = t4.rearrange("p (a c b) -> p a c b", a=2, c=2, b=F)
            B1 = t4r[:, :, 0, :]           # [c | -s/2]
            B2 = t4r[:, :, 1, :]           # [s/2 | c]
            # P1 = [c*y0r | (-s/2)*y1r]: z0 = a0 P1s0 + 2 a1 P1s1
            # P2 = [(s/2)*y0r | c*y1r]:  z1 = 2 a0 P2s0 + a1 P2s1
        P1 = self.wscratch(2)
        P2 = self.wscratch(2)
        self.nc.vector.tensor_tensor(P1, A, B1, OP.mult)
        self.nc.vector.tensor_tensor(P2, A, B2, OP.mult)
        self.n_tt += 2
        self.nc.vector.scalar_tensor_tensor(
            dst0, P1[:, F:2 * F], 2.0 * a1 / a0, P1[:, 0:F], OP.mult, OP.add)
        self.nc.vector.scalar_tensor_tensor(
            dst1, P2[:, 0:F], 2.0 * a0 / a1, P2[:, F:2 * F], OP.mult, OP.add)
        self.n_stt += 2
        if pers:
            self.pers_ids.add(id(dst0))
            self.pers_ids.add(id(dst1))
        t0 = [(a0, dst0)]
        t1 = [(a1, dst1)]
        if k0 != 0.0:
            t0.append((k0, c_slot))
            t1.append((-sign * 2.0 * k0, s_slot))
        if k1 != 0.0:
            t0.append((sign * 2.0 * k1, s_slot))
            t1.append((k1, c_slot))
        e0 = Expr(t0)
        e1 = Expr(t1)
        if len(t0) == 1:
            e0._mat = (a0, dst0)
        if len(t1) == 1:
            e1._mat = (a1, dst1)
        return e0, e1

    def quad_rot2(self, pairs, c_ap, s_ap, label):
        """quad_rot with broadcast trig reads: no replicated-scaled trig
        copies (ACT -16/joint). Operands materialize EXACTLY (A=y0, B=2*y1);
        the sin/2 scale and z1's overall 1/2 ride the broadcast pattern and
        the returned Expr coefficients.
          z0 = c y0 + s y1 = (A*c + B*(s/2))        -> coef 1
          z1 = -s y0 + c y1 = 0.5*(B*c - 4*A*(s/2)) -> coef 0.5
        """
        W = len(pairs)
        assert W == 4
        A = self.wscratch(W)
        Bt = self.wscratch(W)
        for k, (y0, y1) in enumerate(pairs):
            sl = slice(k * F, (k + 1) * F)
            self.mat_exact(y0, A[:, sl])
            self.mat_exact(y1, Bt[:, sl], scale=2.0)
        cb = c_ap[:, None, :].broadcast_to([P, W, F])
        sb = s_ap[:, None, :].broadcast_to([P, W, F])
        v = self.nc.vector
        P1 = self.wscratch(W)
        P2 = self.wscratch(W)
        v.tensor_tensor(P1, A, cb, OP.mult)
        v.tensor_tensor(P2, Bt, sb, OP.mult)
        ZA1, t1 = self.persistent_wide(2, f"{label}a1")
        ZB1 = self.wscratch(2)
        v.tensor_tensor(ZA1, P1[:, 0:2 * F], P2[:, 0:2 * F], OP.add)
        v.tensor_tensor(ZB1, P1[:, 2 * F:4 * F], P2[:, 2 * F:4 * F], OP.add)
        v.tensor_tensor(P1, A, sb, OP.mult)
        v.tensor_tensor(P2, Bt, cb, OP.mult)
        ZA2, t2 = self.persistent_wide(2, f"{label}a2")
        ZB2 = self.wscratch(2)
        v.scalar_tensor_tensor(ZA2, P1[:, 0:2 * F], -4.0, P2[:, 0:2 * F],
                               OP.mult, OP.add)
        v.scalar_tensor_tensor(ZB2, P1[:, 2 * F:4 * F], -4.0,
                               P2[:, 2 * F:4 * F], OP.mult, OP.add)
        self.n_tt += 6
        self.n_stt += 2
        self.dve_track(4, 4)
        self.dve_track(4, 2)

        def _mk(region, k, coef):
            ap = region[:, k * F:(k + 1) * F]
            self.pers_ids.add(id(ap))
            e = Expr([(coef, ap)])
            e._mat = (coef, ap)
            return e

        z0s = [_mk(ZA1, 0, 1.0), _mk(ZA1, 1, 1.0),
               _mk(ZB1, 0, 1.0), _mk(ZB1, 1, 1.0)]
        z1s = [_mk(ZA2, 0, 0.5), _mk(ZA2, 1, 0.5),
               _mk(ZB2, 0, 0.5), _mk(ZB2, 1, 0.5)]
        return z0s, z1s, (t1, t2)

    def pair_rot_bwd2(self, lf01, af01, c_ap, s_ap, label, dve_seed=False):
        """pair_rot_bwd with broadcast trig reads (ACT -8/joint).
          z0 = c y0 - s y1 -> coef 1 ; z1 = s y0 + c y1 -> coef 0.5
        """
        T = self.wscratch(4)
        self.mat_exact(lf01[0], T[:, 0:F], dve_seed=dve_seed)
        self.mat_exact(af01[0], T[:, F:2 * F], dve_seed=dve_seed)
        self.mat_exact(lf01[1], T[:, 2 * F:3 * F], dve_seed=dve_seed,
                       scale=2.0)
        self.mat_exact(af01[1], T[:, 3 * F:4 * F], dve_seed=dve_seed,
                       scale=2.0)
        Av = T[:, 0:2 * F]
        Bv = T[:, 2 * F:4 * F]
        cb = c_ap[:, None, :].broadcast_to([P, 2, F])
        sb = s_ap[:, None, :].broadcast_to([P, 2, F])
        v = self.nc.vector
        Pt = self.wscratch(4)
        Z = self.wscratch(4)
        v.tensor_tensor(Pt[:, 0:2 * F], Av, cb, OP.mult)
        v.tensor_tensor(Pt[:, 2 * F:4 * F], Bv, sb, OP.mult)
        v.tensor_tensor(Z[:, 0:2 * F], Pt[:, 0:2 * F], Pt[:, 2 * F:4 * F],
                        OP.subtract)
        v.tensor_tensor(Pt[:, 0:2 * F], Av, sb, OP.mult)
        v.tensor_tensor(Pt[:, 2 * F:4 * F], Bv, cb, OP.mult)
        v.scalar_tensor_tensor(Z[:, 2 * F:4 * F], Pt[:, 0:2 * F], 4.0,
                               Pt[:, 2 * F:4 * F], OP.mult, OP.add)
        self.n_tt += 5
        self.n_stt += 1
        self.dve_track(6, 2)

        def _e(sl, coef):
            ap = Z[:, sl * F:(sl + 1) * F]
            self.pers_ids.add(id(ap))
            e = Expr([(coef, ap)])
            e._mat = (coef, ap)
            return e

        return (_e(0, 1.0), _e(2, 0.5)), (_e(1, 1.0), _e(3, 0.5))

    def act_copy(self, dst, src, scale=1.0, bias=0.0):
        self.nc.scalar.activation(dst, src, AF.Copy, bias=float(bias),
                                  scale=float(scale))
        self.n_act += 1
        self.busy["act"] += 480.0

    def mat_exact(self, e, dst, dve_seed=False, scale=1.0):
        """Materialize the EXACT value of e into dst (coef 1, const folded).

        Unlike mat(), the result plane holds sum(coef*plane)+const verbatim,
        so packed slot-aligned products across different slots stay
        coefficient-consistent. Costs the same n-1 STT as mat(); a leading
        ACT copy (idle engine) absorbs the lead coef and the constant when
        no unit-coefficient lead term exists.
        """
        terms = sorted(e.terms, key=lambda t: -abs(t[0]))
        if not terms:
            self.nc.vector.memset(dst, float(e.const) * scale)
            return
        const = e.const * scale
        if len(terms) >= 2 and self._pe_chain(
                terms, dst, scale, const, dve_seed=dve_seed,
                seed_act=not dve_seed) is not None:
            return
        if scale != 1.0:
            terms = [(c * scale, ap) for c, ap in terms]
        unit = next((i for i, (ck, _) in enumerate(terms)
                     if ck == 1.0), None)
        if len(terms) == 1:
            c0, x0 = terms[0]
            if dve_seed:
                self.nc.vector.tensor_scalar(
                    dst, x0, float(c0), const, OP.mult, OP.add)
                self.n_stt += 1
            else:
                self.act_copy(dst, x0, scale=c0, bias=const)
            return
        if unit is not None and const == 0.0:
            c0, x0 = terms.pop(unit)
            ck, xk = terms.pop(0)
            self.nc.vector.scalar_tensor_tensor(
                dst, xk, float(ck), x0, OP.mult, OP.add)
            self.n_stt += 1
        else:
            c0, x0 = terms.pop(0)
            if dve_seed:
                self.nc.vector.tensor_scalar(
                    dst, x0, float(c0), const, OP.mult, OP.add)
                self.n_stt += 1
            else:
                self.act_copy(dst, x0, scale=c0, bias=const)
        for ck, xk in terms:
            self.nc.vector.scalar_tensor_tensor(
                dst, xk, float(ck), dst, OP.mult, OP.add)
            self.n_stt += 1

    def cyc_tile(self, vec, label, dve_seed=False):
        """[P,5F] tile holding [x0|x1|x2|x0|x1] of a 3-vector of Exprs."""
        t, tag = self.persistent_wide(5, label)
        for k in range(3):
            self.mat_exact(vec[k], t[:, k * F:(k + 1) * F], dve_seed=dve_seed)
        if dve_seed:
            self.nc.vector.tensor_scalar(
                t[:, 3 * F:5 * F], t[:, 0:2 * F], 1.0, None, OP.mult)
            self.n_stt += 1
        else:
            self.act_copy(t[:, 3 * F:4 * F], t[:, 0:F])
            self.act_copy(t[:, 4 * F:5 * F], t[:, F:2 * F])
        return t, tag

    def mat_rel(self, e, dst, dve_seed=False):
        """Materialize (value incl. const)/c0 into dst; returns c0.

        Like mat() but the constant is folded into the plane (via an ACT
        seed copy on the idle scalar engine), so rotating the plane rotates
        the full affine value and downstream exprs stay pure. dve_seed=True
        seeds with a DVE tensor_scalar instead: at the backward tail the
        ACT queue latency sits on the critical path (DVE has nothing left
        to overlap), so cross-engine seeding there costs ~0.5us per chain.
        """
        assert e.terms
        terms = sorted(e.terms, key=lambda t: -abs(t[0]))
        c0, x0 = terms[0]
        if len(terms) >= 2 and self._pe_chain(
                terms, dst, 1.0 / c0, e.const / c0, dve_seed=dve_seed,
                seed_act=(e.const != 0.0) and not dve_seed) is not None:
            return c0
        if e.const != 0.0 or len(terms) == 1:
            if dve_seed:
                self.nc.vector.tensor_scalar(
                    dst, x0, 1.0, e.const / c0, OP.mult, OP.add)
                self.n_stt += 1
            else:
                self.act_copy(dst, x0, scale=1.0, bias=e.const / c0)
            rest = terms[1:]
        else:
            c1, x1 = terms[1]
            self.nc.vector.scalar_tensor_tensor(
                dst, x1, c1 / c0, x0, OP.mult, OP.add)
            self.n_stt += 1
            rest = terms[2:]
        for ck, xk in rest:
            self.nc.vector.scalar_tensor_tensor(
                dst, xk, ck / c0, dst, OP.mult, OP.add)
            self.n_stt += 1
        return c0

    def quad_rot(self, pairs, c_ap, s_ap, label):
        """Rz^T-rotate W (Y0,Y1) expr pairs sharing one joint angle.

        z0 = c Y0 + s Y1 ; z1 = -s Y0 + c Y1. s_ap holds sin/2 (the 2x is
        folded into the replicated-trig scales). Y consts fold into slot
        contents (mat_rel), so outputs are pure planes. Slot coefficients
        fold into the per-slot scales of ACT-replicated trig tiles, letting
        each product group and each combine run as ONE wide DVE op.
        Returns (z0_exprs, z1_exprs, persistent_tags): z0/z1 of pairs [0,1]
        land in persistent [P,2F] tiles (joint states w/v); pairs [2,3] in
        scratch (consumed by the same joint's dw/dv).
        """
        W = len(pairs)
        assert W == 4
        A = self.wscratch(W)
        Bt = self.wscratch(W)
        CA = self.wscratch(W)
        SB = self.wscratch(W)
        SmA = self.wscratch(W)
        CB = self.wscratch(W)
        aA, aB = [], []
        for k, (y0, y1) in enumerate(pairs):
            sl = slice(k * F, (k + 1) * F)
            aA.append(self.mat_rel(y0, A[:, sl]))
            aB.append(self.mat_rel(y1, Bt[:, sl]))
        for k in range(W):
            sl = slice(k * F, (k + 1) * F)
            self.act_copy(CA[:, sl], c_ap, scale=aA[k])
            self.act_copy(SB[:, sl], s_ap, scale=2.0 * aB[k])
            self.act_copy(SmA[:, sl], s_ap, scale=-2.0 * aA[k])
            self.act_copy(CB[:, sl], c_ap, scale=aB[k])
        v = self.nc.vector
        P1 = self.wscratch(W)
        P2 = self.wscratch(W)
        v.tensor_tensor(P1, A, CA, OP.mult)
        v.tensor_tensor(P2, Bt, SB, OP.mult)
        ZA1, t1 = self.persistent_wide(2, f"{label}a1")
        ZB1 = self.wscratch(2)
        v.tensor_tensor(ZA1, P1[:, 0:2 * F], P2[:, 0:2 * F], OP.add)
        v.tensor_tensor(ZB1, P1[:, 2 * F:4 * F], P2[:, 2 * F:4 * F], OP.add)
        v.tensor_tensor(P1, A, SmA, OP.mult)
        v.tensor_tensor(P2, Bt, CB, OP.mult)
        ZA2, t2 = self.persistent_wide(2, f"{label}a2")
        ZB2 = self.wscratch(2)
        v.tensor_tensor(ZA2, P1[:, 0:2 * F], P2[:, 0:2 * F], OP.add)
        v.tensor_tensor(ZB2, P1[:, 2 * F:4 * F], P2[:, 2 * F:4 * F], OP.add)
        self.n_tt += 8
        self.dve_track(4, 4)
        self.dve_track(4, 2)

        def _mk(region, k):
            ap = region[:, k * F:(k + 1) * F]
            self.pers_ids.add(id(ap))
            e = Expr([(1.0, ap)])
            e._mat = (1.0, ap)
            return e

        z0s = [_mk(ZA1, 0), _mk(ZA1, 1), _mk(ZB1, 0), _mk(ZB1, 1)]
        z1s = [_mk(ZA2, 0), _mk(ZA2, 1), _mk(ZB2, 0), _mk(ZB2, 1)]
        return z0s, z1s, (t1, t2)

    def pair_rot_bwd(self, lf01, af01, c_ap, s_ap, label, dve_seed=False):
        """Rz-rotate (lf0,lf1) and (af0,af1): z0 = c y0 - s y1; z1 = s y0 + c y1.

        Raw force exprs land exactly (consts folded, lead coefs in the
        ACT-replicated trig scales) in one [lf0|lf1|af0|af1] tile; the two
        rotations then cost 4 wide products + 2 wide combines on [P,2F]
        instead of 8 narrow products + 4 chain materializations.
        Returns ((RzLf0e, RzLf1e), (RzAf0e, RzAf1e)) as pure plane exprs.
        """
        T = self.wscratch(4)
        coefs = []
        for k, e in enumerate([lf01[0], lf01[1], af01[0], af01[1]]):
            coefs.append(self.mat_rel(e, T[:, k * F:(k + 1) * F],
                                      dve_seed=dve_seed))
        aL0, aL1, aA0, aA1 = coefs
        Tr = T.rearrange("p (a b f) -> p a b f", a=2, b=2, f=F)
        Av = Tr[:, :, 0, :]   # [lf0 | af0]
        Bv = Tr[:, :, 1, :]   # [lf1 | af1]
        R1 = self.wscratch(4)  # [c*aL0 | c*aA0 | -s*aL1 | -s*aA1]
        self.act_copy(R1[:, 0:F], c_ap, scale=aL0)
        self.act_copy(R1[:, F:2 * F], c_ap, scale=aA0)
        self.act_copy(R1[:, 2 * F:3 * F], s_ap, scale=-2.0 * aL1)
        self.act_copy(R1[:, 3 * F:4 * F], s_ap, scale=-2.0 * aA1)
        R2 = self.wscratch(4)  # [s*aL0 | s*aA0 | c*aL1 | c*aA1]
        self.act_copy(R2[:, 0:F], s_ap, scale=2.0 * aL0)
        self.act_copy(R2[:, F:2 * F], s_ap, scale=2.0 * aA0)
        self.act_copy(R2[:, 2 * F:3 * F], c_ap, scale=aL1)
        self.act_copy(R2[:, 3 * F:4 * F], c_ap, scale=aA1)
        v = self.nc.vector
        Pt = self.wscratch(4)
        Z = self.wscratch(4)
        v.tensor_tensor(Pt[:, 0:2 * F], Av, R1[:, 0:2 * F], OP.mult)
        v.tensor_tensor(Pt[:, 2 * F:4 * F], Bv, R1[:, 2 * F:4 * F], OP.mult)
        v.tensor_tensor(Z[:, 0:2 * F], Pt[:, 0:2 * F], Pt[:, 2 * F:4 * F],
                        OP.add)
        v.tensor_tensor(Pt[:, 0:2 * F], Av, R2[:, 0:2 * F], OP.mult)
        v.tensor_tensor(Pt[:, 2 * F:4 * F], Bv, R2[:, 2 * F:4 * F], OP.mult)
        v.tensor_tensor(Z[:, 2 * F:4 * F], Pt[:, 0:2 * F], Pt[:, 2 * F:4 * F],
                        OP.add)
        self.n_tt += 6
        self.dve_track(6, 2)

        def _e(sl):
            ap = Z[:, sl * F:(sl + 1) * F]
            self.pers_ids.add(id(ap))
            e = Expr([(1.0, ap)])
            e._mat = (1.0, ap)
            return e

        return (_e(0), _e(2)), (_e(1), _e(3))

    def cross_packed(self, cycA, cycB):
        """cross(A, B) of two cyc tiles -> [P,3F] tile [c0|c1|c2]."""
        Pa = self.wscratch(3)
        Pb = self.wscratch(3)
        self.nc.vector.tensor_tensor(
            Pa, cycA[:, F:4 * F], cycB[:, 2 * F:5 * F], OP.mult)
        self.nc.vector.tensor_tensor(
            Pb, cycA[:, 2 * F:5 * F], cycB[:, F:4 * F], OP.mult)
        self.nc.vector.tensor_tensor(Pa, Pa, Pb, OP.subtract)
        self.n_tt += 3
        self.dve_track(3, 3)
        return Pa

    def free_expr_vec(self, vec):
        for e in vec:
            for _, ap in e.terms:
                ent = self.ap_tag.pop(id(ap), None)
                if ent is None:
                    continue
                self.pers_ids.discard(id(ap))
                if ent[0] == "p":
                    self.free_tags.append(ent[1])
                else:
                    self.free_wide.setdefault(ent[1], []).append(ent[2])

    def joint_boundary(self):
        self.max_joint_allocs = max(self.max_joint_allocs, self.joint_allocs)
        self.joint_allocs = 0

    # ---- expression ops ----
    def lin(self, *pairs, const=0.0):
        acc = {}
        aps = {}
        c_acc = float(const)
        for coef, e in pairs:
            if coef == 0.0 or e is None or e is ZERO and e.const == 0.0:
                if e is not None:
                    c_acc += coef * e.const
                continue
            c_acc += coef * e.const
            for tc, ap in e.terms:
                k = id(ap)
                acc[k] = acc.get(k, 0.0) + coef * tc
                aps[k] = ap
        terms = [(c, aps[k]) for k, c in acc.items() if c != 0.0]
        return Expr(terms, c_acc)

    def mat(self, e, dst=None):
        """Materialize sum-of-terms: e == coef*ap + e.const -> (coef, ap)."""
        assert e.terms, "cannot materialize empty expr"
        if e._mat is not None and dst is None:
            return e._mat
        terms = sorted(e.terms, key=lambda t: -abs(t[0]))
        if len(terms) == 1 and dst is None:
            e._mat = (terms[0][0], terms[0][1])
            return e._mat
        c0, x0 = terms[0]
        if len(terms) == 1:
            self.nc.vector.tensor_scalar(dst, x0, 1.0, None, OP.mult)
            self.n_copy += 1
            e._mat = (c0, dst)
            return e._mat
        if not hasattr(self, "mat_hist"):
            self.mat_hist = {}
        self.mat_hist[len(terms)] = self.mat_hist.get(len(terms), 0) + 1
        r = self._pe_chain(terms, dst, 1.0, 0.0, seed_act=False)
        if r is not None:
            e._mat = (r[1], r[0])
            return e._mat
        t = dst if dst is not None else self.scratch()
        c1, x1 = terms[1]
        eng = self.pick_engine(len(terms) - 1)
        eng.scalar_tensor_tensor(t, x1, c1 / c0, x0, OP.mult, OP.add)
        self.n_stt += 1
        for ck, xk in terms[2:]:
            eng.scalar_tensor_tensor(t, xk, ck / c0, t, OP.mult, OP.add)
            self.n_stt += 1
        e._mat = (c0, t)
        return e._mat

    def mul(self, x, y):
        if not _nonzero(x) or not _nonzero(y):
            return ZERO
        if not x.terms:  # pure const
            return Expr([(x.const * c, ap) for c, ap in y.terms],
                        x.const * y.const)
        if not y.terms:
            return Expr([(y.const * c, ap) for c, ap in x.terms],
                        x.const * y.const)
        cx, ax = self.mat(x)
        cy, ay = self.mat(y)
        if ax.space == bass.MemorySpace.PSUM \
                and ay.space == bass.MemorySpace.PSUM:
            tmp = self.scratch()
            self.nc.vector.tensor_scalar(tmp, ay, 1.0, None, OP.mult)
            self.n_stt += 1
            self.busy["dve"] += self.dve_evac
            ay = tmp
        prod = self.scratch()
        self.pick_engine(1).tensor_tensor(prod, ax, ay, OP.mult)
        self.n_tt += 1
        terms = [(cx * cy, prod)]
        if y.const != 0.0:
            terms.append((cx * y.const, ax))
        if x.const != 0.0:
            terms.append((cy * x.const, ay))
        return Expr(terms, x.const * y.const)

    def snap(self, e, label="", scratch_ok=False):
        """Materialize into a stable plane; returns single-term Expr."""
        if not e.terms:
            return e
        if len(e.terms) == 1 and e._mat is None \
                and id(e.terms[0][1]) in self.pers_ids and not scratch_ok:
            out = Expr(list(e.terms), e.const)
            out._mat = e.terms[0]
            return out
        if e._mat is not None:
            c, src = e._mat
            if id(src) in self.pers_ids or scratch_ok:
                out = Expr([(c, src)], e.const)
                out._mat = (c, src)
                return out
            dst = self.persistent(label)
            self.nc.scalar.activation(dst, src, AF.Copy, bias=0.0, scale=1.0)
            self.n_copy += 1
            out = Expr([(c, dst)], e.const)
            out._mat = (c, dst)
            return out
        dst = self.scratch() if scratch_ok else self.persistent(label)
        c, ap = self.mat(e, dst=dst)
        out = Expr([(c, ap)], e.const)
        out._mat = (c, ap)
        return out

    def snap_to(self, e, dst_ap):
        """Materialize into the given plane; returns single-term Expr."""
        assert e.terms
        c, ap = self.mat(e, dst=dst_ap)
        self.pers_ids.add(id(ap))
        out = Expr([(c, ap)], e.const)
        out._mat = (c, ap)
        return out

    def snap_vec(self, vec, label="", scratch_ok=False):
        return [self.snap(e, f"{label}{i}", scratch_ok) for i, e in enumerate(vec)]

    # ---- 3-vector helpers ----
    def vadd(self, *vecs):
        return [self.lin(*[(1.0, v[i]) for v in vecs]) for i in range(3)]

    def vsub(self, a, b):
        return [self.lin((1.0, a[i]), (-1.0, b[i])) for i in range(3)]

    def cross_const(self, t, X):
        return [
            self.lin((-t[2], X[1]), (t[1], X[2])),
            self.lin((t[2], X[0]), (-t[0], X[2])),
            self.lin((-t[1], X[0]), (t[0], X[1])),
        ]

    def cross_ee(self, A, B):
        return [
            self.lin((1.0, self.mul(A[1], B[2])), (-1.0, self.mul(A[2], B[1]))),
            self.lin((1.0, self.mul(A[2], B[0])), (-1.0, self.mul(A[0], B[2]))),
            self.lin((1.0, self.mul(A[0], B[1])), (-1.0, self.mul(A[1], B[0]))),
        ]

    def matvec_const(self, M, X):
        return [
            self.lin((M[i, 0], X[0]), (M[i, 1], X[1]), (M[i, 2], X[2]))
            for i in range(3)
        ]


def build_module(params):
    trans = np.asarray(params["trans"], np.float64)
    rot_fix = np.asarray(params["rot_fix"], np.float64)
    mass = np.asarray(params["mass"], np.float64)
    com = np.asarray(params["com"], np.float64)
    inertia = np.asarray(params["inertia"], np.float64)
    damping = np.asarray(params["damping"], np.float64)

    nc = bacc.Bacc("TRN2", target_bir_lowering=False, debug=False,
                   num_devices=N_CORES)
    q_d = nc.dram_tensor("q", (BC, ND), DT, kind="ExternalInput")
    qd_d = nc.dram_tensor("qd", (BC, ND), DT, kind="ExternalInput")
    qdd_d = nc.dram_tensor("qdd_des", (BC, ND), DT, kind="ExternalInput")
    use_pe = os.environ.get("K_PE", "1") == "1"
    wmax = int(os.environ.get("K_WMAX", "46"))
    n_waves = int(os.environ.get("K_NWAVES", "9"))
    if use_pe:
        w_d = nc.dram_tensor("wconst", (P, n_waves * (wmax // 2) * P), DT,
                             kind="ExternalInput")
    tq_d = nc.dram_tensor("torque", (BC, ND), DT, kind="ExternalOutput")

    with tile_mod.TileContext(nc) as tc:
        with tc.tile_pool(name="main", bufs=1) as pool, \
             tc.tile_pool(name="io", bufs=1) as io_pool, \
             tc.psum_pool(name="psp", bufs=1) as pspool:
            wtile_ap = None
            if use_pe:
                wtile = io_pool.tile([P, wmax * P], DT, tag="wconst",
                                     name="wconst_sb")
                wtile_ap = wtile[:, :]
            b = Builder(nc, pool,
                        ring_size=int(os.environ.get("K_RING", "11")),
                        pool_frac=float(os.environ.get("K_POOL_FRAC", "0")),
                        pspool=pspool, wtile=wtile_ap, wmax=wmax)
            b.n_waves = n_waves
            if use_pe:
                b.w_d = w_d

            q_t = io_pool.tile([P, F * ND], DT, tag="q", name="q_sb")
            qd_t = io_pool.tile([P, F * ND], DT, tag="qd", name="qd_sb")
            qdd_t = io_pool.tile([P, F * ND], DT, tag="qdd", name="qdd_sb")
            # out aliases q's buffer: q is fully consumed by the trig
            # prologue long before the first backward write_out.
            out_t = io_pool.tile([P, F * ND], DT, tag="q", name="out_sb")
            dram_view = lambda t: t.ap().rearrange("(p f) d -> p (f d)", p=P)
            # q gates trig (the whole critical path): give it the sync queue
            # alone; qd/qdd ride the idle gpsimd queue in parallel.
            nc.sync.dma_start(q_t[:, :], dram_view(q_d))
            nc.gpsimd.dma_start(qd_t[:, :], dram_view(qd_d))
            nc.gpsimd.dma_start(qdd_t[:, :], dram_view(qdd_d))
            if use_pe:
                # weight slots are allocated in first-use (= emission) order,
                # so chunked DMA in slot order arrives before consumers.
                # Waves 0 and 1 load up front; later waves prefetch 1 ahead.
                b._wave_dma(0)
                if n_waves > 1:
                    b._wave_dma(1)

            q3 = q_t[:, :].rearrange("p (f d) -> p f d", d=ND)
            qd3 = qd_t[:, :].rearrange("p (f d) -> p f d", d=ND)
            qdd3 = qdd_t[:, :].rearrange("p (f d) -> p f d", d=ND)
            out3 = out_t[:, :].rearrange("p (f d) -> p f d", d=ND)

            def as_pers_expr(ap):
                b.pers_ids.add(id(ap))
                e = Expr([(1.0, ap)])
                e._mat = (1.0, ap)
                return e

            def const_col(tag, val):
                t = io_pool.tile([P, 1], DT, tag=tag, name=tag)
                nc.vector.memset(t[:, :], float(val))
                return t

            zero_t = const_col("zconst", 0.0)
            # dummy Sin on an immediately-ready tile: hoists the ~2.7us ACT
            # table load to t~0, overlapping the input DMA instead of
            # serializing after it.
            warm_t = io_pool.tile([P, 1], DT, tag="warm", name="warm")
            nc.scalar.activation(warm_t[:, :], zero_t[:, :], AF.Sin,
                                 bias=zero_t[:, :], scale=1.0)

            # ACT Sin has no range reduction (accurate only on [-pi, pi]).
            # q ~ N(0,1) so |q| <= ~5.5: one conditional wrap of 2*pi covers
            # sin(q) and cos(q) = sin((q + pi/2) wrapped).
            PI = float(np.pi)
            TWO_PI = float(2 * np.pi)
            trig = {}
            trig_raw = {}
            _kp = os.environ.get("K_PACK", "0")
            use_pack_fwd = _kp in ("1", "fwd")
            use_pack_bwd = _kp in ("1", "bwd")
            use_xpack = os.environ.get("K_XPACK", "1") == "1"
            def emit_trig(j):
                # ACT Sin is only accurate on [-pi, pi] (no range reduction)
                # and q ~ N(0,1) exceeds pi. Half-angle identities avoid any
                # wrapping: |q/2| <= ~2.9 < pi and |q/4| < pi/2 always.
                #   cos q = 1 - 2 sin^2(q/2)
                #   sin q = 2 sin(q/2) cos(q/2),  cos(q/2) = 1 - 2 sin^2(q/4)
                # Everything runs on the otherwise-idle ACT engine except one
                # DVE product for sin. Emitted lazily (joint j+1's chain goes
                # out with joint j's body) so the in-order ACT queue reaches
                # each joint's replicated-trig copies just before the DVE
                # needs them, instead of front-loading all six chains.
                if j in trig:
                    return
                qj = q3[:, :, j]
                t4w = 4 if (use_pack_fwd or use_pack_bwd) else 2
                t4, _t4tag = b.persistent_wide(t4w, f"t4_{j}")
                c_ap = t4[:, 0:F]
                s_ap = t4[:, F:2 * F]
                b.pers_ids.add(id(c_ap))
                b.pers_ids.add(id(s_ap))
                sh = b.scratch(DT)   # sin(q/2)
                nc.scalar.activation(sh, qj, AF.Sin, bias=zero_t[:, :],
                                     scale=0.5)
                sq = b.scratch(DT)   # sin(q/4)
                nc.scalar.activation(sq, qj, AF.Sin, bias=zero_t[:, :],
                                     scale=0.25)
                sq2 = b.scratch(DT)  # sin^2(q/4)
                nc.scalar.activation(sq2, sq, AF.Square)
                ch = b.scratch(DT)   # cos(q/2)
                nc.scalar.activation(ch, sq2, AF.Copy, bias=1.0, scale=-2.0)
                sh2 = b.scratch(DT)  # sin^2(q/2)
                nc.scalar.activation(sh2, sh, AF.Square)
                nc.scalar.activation(c_ap, sh2, AF.Copy, bias=1.0, scale=-2.0)
                nc.vector.tensor_tensor(s_ap, sh, ch, OP.mult)  # sin q / 2
                if use_pack_fwd or use_pack_bwd:
                    # slots 2,3: [-sin/2 | cos] for packed-rotation patterns
                    nc.scalar.activation(t4[:, 2 * F:3 * F], s_ap, AF.Copy,
                                         bias=0.0, scale=-1.0)
                    nc.scalar.activation(t4[:, 3 * F:4 * F], c_ap, AF.Copy,
                                         bias=0.0, scale=1.0)
                    b.n_act += 2
                b.n_tt += 1
                b.n_act += 6
                se = Expr([(2.0, s_ap)])
                se._mat = (2.0, s_ap)
                trig[j] = (as_pers_expr(c_ap), se)
                trig_raw[j] = t4

            emit_trig(0)
            emit_trig(1)
            b.joint_boundary()



            # qd repack on DVE, after the sin products in the in-order DVE
            # stream (qd's DMA rides the slower gpsimd queue; putting these
            # first would block the trig products behind that DMA).
            qd_pl = []
            for j in range(ND - 1):  # qd_6 is only read once (tau_6)
                d_ap = b.persistent(f"qd{j}")
                nc.vector.tensor_scalar(d_ap, qd3[:, :, j], 1.0, None, OP.mult)
                qd_pl.append(as_pers_expr(d_ap))

            def qdd_expr(j):
                # strided read (28B stride) is cheapest, measured against all
                # repack routes: ACT copy queues behind the trig prologue
                # (+5us); DVE copy costs more than the ~190ns/op penalty;
                # SBUF->SBUF de-stride DMA on the sync queue is ~19x derated
                # and its sem deps stall the forward chains (+23us).
                ap = qdd3[:, :, j]
                e = Expr([(1.0, ap)])
                e._mat = (1.0, ap)
                b.pers_ids.add(id(ap))
                return e

            def rot_inv(j, X):
                """Rz(q_j)^T @ (rot_fix_j^T @ X)"""
                if not any(_nonzero(e) for e in X):
                    return [ZERO, ZERO, ZERO]
                Fm = rot_fix[j].T
                Y = b.matvec_const(Fm, X)
                c, s = trig[j]
                z0 = b.lin((1.0, b.mul(c, Y[0])), (1.0, b.mul(s, Y[1])))
                z1 = b.lin((-1.0, b.mul(s, Y[0])), (1.0, b.mul(c, Y[1])))
                return [z0, z1, Y[2]]

            def write_out(j, e):
                dst = out3[:, :, j]
                if not e.terms:
                    b.nc.vector.memset(dst, float(e.const))
                    return
                c, ap = b.mat(e)
                nc.scalar.activation(dst, ap, AF.Copy, bias=float(e.const),
                                     scale=float(c))
                b.n_act += 1

            # ---------------- forward (joints 0..5) ----------------
            w_p = [ZERO, ZERO, ZERO]
            v_p = [ZERO, ZERO, ZERO]
            a_p = [ZERO, ZERO, ZERO]
            la_p = [ZERO, ZERO, Expr(const=GRAV)]
            states = []
            use_qpack = os.environ.get("K_QPACK", "1") == "1"
            fwd_ztags = {}
            for j in range(ND - 1):
                t_j = trans[j]
                Uv = b.vsub(v_p, b.cross_const(t_j, w_p))
                Ua = b.vsub(la_p, b.cross_const(t_j, a_p))
                Uv = [b.snap(e, f"Uv{j}", scratch_ok=True)
                      if len(e.terms) > 1 else e for e in Uv]
                Ua = [b.snap(e, f"Ua{j}", scratch_ok=True)
                      if len(e.terms) > 1 else e for e in Ua]
                packed_done = False
                if use_qpack and j > 0:
                    Fm = rot_fix[j].T
                    Yw = b.matvec_const(Fm, w_p)
                    Yv = b.matvec_const(Fm, Uv)
                    Ya = b.matvec_const(Fm, a_p)
                    Yu = b.matvec_const(Fm, Ua)
                    pairs = [(Yw[0], Yw[1]), (Yv[0], Yv[1]),
                             (Ya[0], Ya[1]), (Yu[0], Yu[1])]
                    if all(e.terms for pr in pairs for e in pr):
                        t4 = trig_raw[j]
                        if os.environ.get("K_QR2", "1") == "1":
                            z0s, z1s, ztags = b.quad_rot2(
                                pairs, t4[:, 0:F], t4[:, F:2 * F], f"q{j}")
                        else:
                            z0s, z1s, ztags = b.quad_rot(
                                pairs, t4[:, 0:F], t4[:, F:2 * F], f"q{j}")
                        Rw = [z0s[0], z1s[0], Yw[2]]
                        Rv = [z0s[1], z1s[1], Yv[2]]
                        Ra = [z0s[2], z1s[2], Ya[2]]
                        Rla = [z0s[3], z1s[3], Yu[2]]
                        fwd_ztags[j] = ztags
                        packed_done = True
                _kpj = int(os.environ.get("K_PACK_J", "-1"))
                if not packed_done and use_pack_fwd and j > 0 \
                        and (_kpj < 0 or j == _kpj):
                    Fm = rot_fix[j].T

                    def _rows(X):
                        return [b.lin((Fm[i, 0], X[0]), (Fm[i, 1], X[1]),
                                      (Fm[i, 2], X[2])) for i in range(3)]

                    Yw, Yv, Ya, Yu = _rows(w_p), _rows(Uv), _rows(a_p), _rows(Ua)
                    heads = [Yw[0], Yw[1], Yv[0], Yv[1],
                             Ya[0], Ya[1], Yu[0], Yu[1]]
                    if all(e.terms for e in heads):
                        t4 = trig_raw[j]
                        zw = b.packed_pair_rot(Yw[0], Yw[1], t4, +1,
                                               b.persistent(f"w{j}0"),
                                               b.persistent(f"w{j}1"), True)
                        zv = b.packed_pair_rot(Yv[0], Yv[1], t4, +1,
                                               b.persistent(f"v{j}0"),
                                               b.persistent(f"v{j}1"), True)
                        za = b.packed_pair_rot(Ya[0], Ya[1], t4, +1,
                                               b.scratch(), b.scratch(), False)
                        zu = b.packed_pair_rot(Yu[0], Yu[1], t4, +1,
                                               b.scratch(), b.scratch(), False)
                        Rw = [zw[0], zw[1], Yw[2]]
                        Rv = [zv[0], zv[1], Yv[2]]
                        Ra = [za[0], za[1], Ya[2]]
                        Rla = [zu[0], zu[1], Yu[2]]
                        packed_done = True
                if not packed_done:
                    Rw = rot_inv(j, w_p)
                    Rv = rot_inv(j, Uv)
                    Ra = rot_inv(j, a_p)
                    Rla = rot_inv(j, Ua)
                qdj = qd_pl[j]
                qddj = qdd_expr(j)
                w = [Rw[0], Rw[1], b.lin((1.0, Rw[2]), (1.0, qdj))]
                w = b.snap_vec(w, f"w{j}_")
                v = b.snap_vec(Rv, f"v{j}_")
                dw = [
                    b.lin((1.0, Ra[0]), (1.0, b.mul(w[1], qdj))),
                    b.lin((1.0, Ra[1]), (-1.0, b.mul(w[0], qdj))),
                    b.lin((1.0, Ra[2]), (1.0, qddj)),
                ]
                dv = [
                    b.lin((1.0, Rla[0]), (1.0, b.mul(v[1], qdj))),
                    b.lin((1.0, Rla[1]), (-1.0, b.mul(v[0], qdj))),
                    Rla[2],
                ]
                dw = b.snap_vec(dw, f"dw{j}_")
                dv = b.snap_vec(dv, f"dv{j}_")
                states.append((w, v, dw, dv))
                w_p, v_p, a_p, la_p = w, v, dw, dv
                if j + 2 < ND - 1:
                    emit_trig(j + 2)
                if j in (1, 3):
                    b.new_wave()
                b.joint_boundary()

            # ---------------- backward (j = 5..0) ----------------
            # tau_6 = damping_6 * qd_6 is qd_6's only read: use the strided
            # column directly instead of a dense repack copy.
            qd6 = Expr([(1.0, qd3[:, :, ND - 1])])
            qd6._mat = (1.0, qd3[:, :, ND - 1])
            b.pers_ids.add(id(qd6.terms[0][1]))
            write_out(ND - 1, b.lin((damping[ND - 1], qd6)))

            lin_f = [ZERO, ZERO, ZERO]
            ang_f = [ZERO, ZERO, ZERO]
            bw_pack = None
            use_brot = os.environ.get("K_BROT", "1") == "1"
            use_dveseed = os.environ.get("K_DVESEED", "0") == "1"
            for j in range(ND - 2, -1, -1):
                if j in (5, 4, 3, 2, 1):
                    b.new_wave()
                have_child = any(_nonzero(e) for e in lin_f + ang_f)
                if have_child and use_pack_bwd and bw_pack is not None:
                    bw_tile_p, lfc, afc, prev_tag = bw_pack
                    t4 = trig_raw[j + 1]
                    Rf = rot_fix[j + 1]
                    zlf = b.packed_pair_rot(
                        None, None, t4, -1, b.scratch(), b.scratch(), False,
                        A_ready=(bw_tile_p[:, 0:2 * F],) + lfc)
                    zaf = b.packed_pair_rot(
                        None, None, t4, -1, b.scratch(), b.scratch(), False,
                        A_ready=(bw_tile_p[:, 2 * F:4 * F],) + afc)
                    b.free_wide.setdefault(4, []).append(prev_tag)
                    RzLf = [zlf[0], zlf[1], lin_f[2]]
                    RzAf = [zaf[0], zaf[1], ang_f[2]]
                    Rc_lf = b.matvec_const(Rf, RzLf)
                    Rc_lf = [b.snap(e, f"rclf{j}", scratch_ok=True)
                             if len(e.terms) > 2 and (j > 0 or i < 2) else e
                             for i, e in enumerate(Rc_lf)]
                    Rc_af = b.matvec_const(Rf, RzAf)
                    child_ang = b.vadd(b.cross_const(trans[j + 1], Rc_lf), Rc_af)
                    child_lin = Rc_lf
                elif have_child and use_brot:
                    Rf = rot_fix[j + 1]
                    t4c = trig_raw[j + 1]
                    _brot2 = os.environ.get("K_QR2", "1") == "1"
                    _brfn = b.pair_rot_bwd2 if _brot2 else b.pair_rot_bwd
                    lfp, afp = _brfn(
                        (lin_f[0], lin_f[1]), (ang_f[0], ang_f[1]),
                        t4c[:, 0:F], t4c[:, F:2 * F], f"br{j}",
                        dve_seed=(j <= 1 and use_dveseed))
                    RzLf = [lfp[0], lfp[1], lin_f[2]]
                    RzAf = [afp[0], afp[1], ang_f[2]]
                    Rc_lf = b.matvec_const(Rf, RzLf)
                    Rc_lf = [b.snap(e, f"rclf{j}", scratch_ok=True)
                             if len(e.terms) > 2 and (j > 0 or i < 2) else e
                             for i, e in enumerate(Rc_lf)]
                    Rc_af = b.matvec_const(Rf, RzAf)
                    child_ang = b.vadd(b.cross_const(trans[j + 1], Rc_lf), Rc_af)
                    child_lin = Rc_lf
                elif have_child:
                    cs, ss = trig[j + 1]
                    Rf = rot_fix[j + 1]
                    # xy-rotation outputs fan out 3x through the rot_fix
                    # matvec: snapping them saves (t-1)(f-1) chain slots.
                    RzLf = [
                        b.snap(b.lin((1.0, b.mul(cs, lin_f[0])),
                                     (-1.0, b.mul(ss, lin_f[1]))),
                               f"rzlf{j}0", scratch_ok=True),
                        b.snap(b.lin((1.0, b.mul(ss, lin_f[0])),
                                     (1.0, b.mul(cs, lin_f[1]))),
                               f"rzlf{j}1", scratch_ok=True),
                        lin_f[2],
                    ]
                    Rc_lf = b.matvec_const(Rf, RzLf)
                    Rc_lf = [b.snap(e, f"rclf{j}", scratch_ok=True)
                             if len(e.terms) > 2 and (j > 0 or i < 2) else e
                             for i, e in enumerate(Rc_lf)]
                    RzAf = [
                        b.snap(b.lin((1.0, b.mul(cs, ang_f[0])),
                                     (-1.0, b.mul(ss, ang_f[1]))),
                               f"rzaf{j}0", scratch_ok=True),
                        b.snap(b.lin((1.0, b.mul(ss, ang_f[0])),
                                     (1.0, b.mul(cs, ang_f[1]))),
                               f"rzaf{j}1", scratch_ok=True),
                        ang_f[2],
                    ]
                    Rc_af = b.matvec_const(Rf, RzAf)
                    child_ang = b.vadd(b.cross_const(trans[j + 1], Rc_lf), Rc_af)
                    child_lin = Rc_lf
                else:
                    child_ang = [ZERO, ZERO, ZERO]
                    child_lin = [ZERO, ZERO, ZERO]

                # free previous joint's planes only after the child rotation
                # has consumed the raw lf/af term planes
                if j != ND - 2:
                    for vec in states[j + 1]:
                        b.free_expr_vec(vec)
                    if j + 1 in fwd_ztags:
                        for tag in fwd_ztags.pop(j + 1):
                            b.free_wide.setdefault(2, []).append(tag)

                w, v, dw, dv = states[j]
                m = float(mass[j])
                mc = m * com[j]
                cxm = np.array([
                    [0.0, -com[j][2], com[j][1]],
                    [com[j][2], 0.0, -com[j][0]],
                    [-com[j][1], com[j][0], 0.0],
                ])
                Isp = inertia[j] + m * (cxm @ cxm.T)

                if j == 0 and not _nonzero(w[0]) and not _nonzero(w[1]) \
                        and not any(_nonzero(e) for e in v):
                    # Base joint: only ang_f.z feeds torque_0 and nothing
                    # consumes lin_f_0/ang_f_0 further down. With w=(0,0,qd0)
                    # and v=0, tmp_a.z = (w x IcV_a).z + (v x IcV_l).z = 0,
                    # so af_z = (Isp dw + mc x dv).z + child_ang.z only.
                    af2 = b.lin(
                        (Isp[2, 0], dw[0]), (Isp[2, 1], dw[1]),
                        (Isp[2, 2], dw[2]),
                        (mc[0], dv[1]), (-mc[1], dv[0]),
                        (1.0, child_ang[2]))
                    write_out(0, b.lin((1.0, af2), (damping[0], qd_pl[0])))
                    b.joint_boundary()
                    continue

                IcA_l = b.vsub([b.lin((m, dv[i])) for i in range(3)],
                               b.cross_const(mc, dw))
                IcA_a = b.vadd(b.matvec_const(Isp, dw), b.cross_const(mc, dv))
                IcV_l = b.vsub([b.lin((m, v[i])) for i in range(3)],
                               b.cross_const(mc, w))
                IcV_a = b.vadd(b.matvec_const(Isp, w), b.cross_const(mc, v))
                if use_xpack:
                    # Packed crosses: operands land EXACTLY (coef 1, const
                    # folded) in cyclically-duplicated [x0|x1|x2|x0|x1]
                    # tiles; each cross is then 2 wide products + 1 wide
                    # subtract instead of 6 narrow products + per-component
                    # chain slots.
                    tail = j <= 1 and use_dveseed
                    cw, cw_tag = b.cyc_tile(w, f"cw{j}", dve_seed=tail)
                    cv, cv_tag = b.cyc_tile(v, f"cv{j}", dve_seed=tail)
                    cva, cva_tag = b.cyc_tile(IcV_a, f"cva{j}", dve_seed=tail)
                    cvl, cvl_tag = b.cyc_tile(IcV_l, f"cvl{j}", dve_seed=tail)
                    C1 = b.cross_packed(cw, cva)
                    C2 = b.cross_packed(cv, cvl)
                    C3 = b.cross_packed(cw, cvl)
                    nc.vector.tensor_tensor(C1, C1, C2, OP.add)
                    b.n_tt += 1

                    def _slices(t):
                        out = []
                        for k in range(3):
                            ap = t[:, k * F:(k + 1) * F]
                            b.pers_ids.add(id(ap))
                            e = Expr([(1.0, ap)])
                            e._mat = (1.0, ap)
                            out.append(e)
                        return out

                    tmp_a = _slices(C1)
                    tmp_l = _slices(C3)
                    for tag in (cw_tag, cv_tag, cva_tag, cvl_tag):
                        b.free_wide.setdefault(5, []).append(tag)
                else:
                    tmp_a = b.vadd(b.cross_ee(w, IcV_a), b.cross_ee(v, IcV_l))
                    tmp_l = b.cross_ee(w, IcV_l)
                lf_new = b.vadd(IcA_l, tmp_l, child_lin)
                af_new = b.vadd(IcA_a, tmp_a, child_ang)
                if use_pack_bwd and j > 0:
                    bw_tile, bw_tag = b.persistent_wide(4, f"bw{j}")
                    lf0 = b.snap_to(lf_new[0], bw_tile[:, 0:F])
                    lf1 = b.snap_to(lf_new[1], bw_tile[:, F:2 * F])
                    af0 = b.snap_to(af_new[0], bw_tile[:, 2 * F:3 * F])
                    af1 = b.snap_to(af_new[1], bw_tile[:, 3 * F:4 * F])
                    lf2 = b.snap(lf_new[2], f"lf{j}2")
                    af2 = b.snap(af_new[2], f"af{j}2")
                    lin_f = [lf0, lf1, lf2]
                    ang_f = [af0, af1, af2]
                    bw_pack = (bw_tile,
                               (lf0._mat[0], lf1._mat[0],
                                lf0.const, lf1.const),
                               (af0._mat[0], af1._mat[0],
                                af0.const, af1.const), bw_tag)
                elif use_brot:
                    # carry lf/af xy raw: the next joint's packed rotation
                    # materializes them directly into its operand tile
                    lin_f = [lf_new[0], lf_new[1],
                             b.snap(lf_new[2], f"lf{j}2")]
                    ang_f = [af_new[0], af_new[1],
                             b.snap(af_new[2], f"af{j}2")]
                    bw_pack = None
                else:
                    lin_f = b.snap_vec(lf_new, f"lf{j}_")
                    ang_f = b.snap_vec(af_new, f"af{j}_")
                    bw_pack = None
                write_out(j, b.lin((1.0, ang_f[2]), (damping[j], qd_pl[j])))
                b.joint_boundary()

            nc.sync.dma_start(dram_view(tq_d), out_t[:, :])

            stats = dict(stt=b.n_stt, tt=b.n_tt, act=b.n_act, copy=b.n_copy,
                         mm=b.n_mm, nw=[len(w) for w in b.wave_coefs],
                         busy={k: round(v / 1000.0, 1)
                               for k, v in b.busy.items()},
                         pers=b.pers_idx, max_joint_allocs=b.max_joint_allocs,
                         ring=b.ring_size,
                         mat_hist=dict(sorted(getattr(b, "mat_hist", {}).items())))
            stats["wcoefs"] = [list(w) for w in b.wave_coefs]
            stats["wmax"] = wmax if use_pe else 0
            stats["n_waves"] = n_waves

    nc.compile()
    return nc, stats


_CACHE = {}


def _get_module(params):
    import hashlib
    key = b"".join(np.ascontiguousarray(np.asarray(params[k], np.float32)).tobytes()
                   for k in ("trans", "rot_fix", "mass", "com", "inertia",
                             "damping"))
    h = hashlib.sha1(key).hexdigest()
    if h not in _CACHE:
        _CACHE[h] = build_module(params)
    return _CACHE[h]


def run(q, qd, qdd_des, trans, rot_fix, mass, com, inertia, damping,
        trace=False):
    q = np.asarray(q)
    qd = np.asarray(qd)
    qdd_des = np.asarray(qdd_des)
    assert q.shape == (B_TOTAL, ND), f"unexpected q shape {q.shape}"
    assert qd.shape == (B_TOTAL, ND) and qdd_des.shape == (B_TOTAL, ND)
    params = dict(trans=trans, rot_fix=rot_fix, mass=mass, com=com,
                  inertia=inertia, damping=damping)
    nc, stats = _get_module(params)
    wconst = None
    if stats.get("wmax"):
        whalf = stats["wmax"] // 2
        nw = stats["n_waves"]
        wconst = np.zeros((P, nw * whalf * P), np.float32)
        eye = np.eye(P, dtype=np.float32)
        for wv, coefs in enumerate(stats["wcoefs"]):
            for s, cval in enumerate(coefs):
                a = (wv * whalf + s) * P
                wconst[:, a:a + P] = np.float32(cval) * eye
    in_maps = []
    for c in range(N_CORES):
        sl = slice(c * BC, (c + 1) * BC)
        m = {
            "q": np.ascontiguousarray(q[sl], np.float32),
            "qd": np.ascontiguousarray(qd[sl], np.float32),
            "qdd_des": np.ascontiguousarray(qdd_des[sl], np.float32),
        }
        if wconst is not None:
            m["wconst"] = wconst
        in_maps.append(m)
    res = bass_utils.run_bass_kernel_spmd(
        nc, in_maps, core_ids=list(range(N_CORES)), trace=trace)
    out = np.concatenate([res.results[c]["torque"] for c in range(N_CORES)],
                         axis=0)
    return out.astype(np.float32), res, stats


def kernel(q, qd, qdd_des, trans, rot_fix, mass, com, inertia, damping):
    out, _, _ = run(q, qd, qdd_des, trans, rot_fix, mass, com, inertia,
                    damping, trace=False)
    return out



# revision 54
# speedup vs baseline: 1.0342x; 1.0111x over previous
"""Trainium2 Bass kernel for batched 7-DOF RNEA inverse dynamics.

Layout: pure data-parallel over 8 NeuronCores (32768 batch elements each).
Per core, every per-element scalar lives as an SBUF "plane" [128, 256] f32
(batch element e = partition*256 + free). All per-link parameters (trans,
rot_fix, mass, com, inertia, damping) are baked into the instruction stream
as immediates at build time.

The math is emitted through a small expression compiler:
  Expr = sum(coef * plane) + const
Linear combinations are free (term concatenation); they materialize as
chains of fused scalar_tensor_tensor ops ((in0*imm) + in1, in-place
accumulation) only when a product or an output needs a raw plane. Products
are DVE tensor_tensor ops. sin/cos and the final affine outputs go to the
scalar engine (ACT).

Algebraic structure used:
  - Rinv @ x with Rinv = Rz(q)^T @ rot_fix^T: constant matvec folded into
    stt-chain immediates, then one complex rotation (4 products).
  - cross(t_inv, Rinv x) = -Rinv (t x x): constant cross matrices fold into
    linear immediates, killing 12 products per forward joint.
  - forward state for joint 6 is never computed (the reference's backward
    recursion never reads it).
  - ACT Sin has no range reduction (accurate only on [-pi, pi]) and
    q ~ N(0,1) exceeds pi; half-angle identities avoid wrapping:
    cos q = 1-2sin^2(q/2), sin q = 2 sin(q/2)(1-2sin^2(q/4)).

Measured (8 cores, trn2): HW exec ~309 us, max abs err ~8e-5 on torque
absmax ~139 (fp32). Vector engine is the bottleneck (~650 DVE ops after
packing, from ~950); measured DVE op costs: 359 ns/[128,256] STT or TT
(267 ns data + ~92 ns SBUF-access/issue overhead), tensor_scalar 225 ns
(2x_2p), ACT 420 ns and fully parallel with DVE. Wide-op packing exploits
the per-op overhead plus term-count reduction:
  - forward quad-rot (K_QPACK): the 4 per-joint Rz^T rotations run as 4
    wide [P,4F] products against ACT-replicated trig tiles (slot scales
    absorb the mats' lead coefs; consts folded into slot contents via
    ACT-seeded exact chains) + 4 wide combines; outputs are pure planes.
  - backward child rotation (K_BROT): lf/af xy carried as RAW exprs,
    materialized into one [lf0|lf1|af0|af1] tile at the consuming joint,
    rotated with 4+2 wide [P,2F] ops.
  - backward crosses (K_XPACK): w, v, IcV_a, IcV_l land exactly in
    cyclically-duplicated [x0|x1|x2|x0|x1] tiles; each cross is 2 wide
    [P,3F] products + 1 wide subtract.
  - base joint computes only ang_f.z (everything else is dead there).
  - trig chains emitted lazily (joint j+2 with joint j's body) so the
    in-order ACT queue reaches each joint's replicated-trig copies just
    before the DVE needs them; front-loading all six chains costs ~5 us.
Rejected with on-HW measurements: TensorE accumulate (475 ns/[128,256]
matmul term, only ~50% overlap with DVE traffic), GPSIMD (SBUF ports
physically shared with DVE: 2-input ops serialize ~100%), bf16/fp16
(abs-err budget ~3e-3 at near-zero outputs vs ~1e-2+ rounding of O(30)
intermediates), and state-major layouts for PE matvecs (DVE free-dim
cycle cost explodes at <128 active partitions).
"""

import os
import sys

for _p in ("/opt/trn_rl_repo",):
    if os.path.isdir(_p) and _p not in sys.path:
        sys.path.append(_p)

import numpy as np

import concourse.bass as bass
import concourse.bacc as bacc
import concourse.mybir as mybir
from concourse import tile as tile_mod
from concourse import bass_utils

N_CORES = 8
ND = 7
B_TOTAL = 262144
BC = B_TOTAL // N_CORES  # 32768 per core
P = 128
F = BC // P  # 256
GRAV = 9.81
DT = mybir.dt.float32
DT_C = mybir.dt.bfloat16 if os.environ.get("K_BF16", "0") == "1" else mybir.dt.float32
OP = mybir.AluOpType
AF = mybir.ActivationFunctionType


class Expr:
    """value = sum(coef * plane_ap) + const"""

    __slots__ = ("terms", "const", "_mat")

    def __init__(self, terms=(), const=0.0):
        self.terms = list(terms)
        self.const = float(const)
        self._mat = None  # cached (coef, ap) of materialized sum-of-terms


ZERO = Expr()


def _nonzero(e):
    return bool(e.terms) or e.const != 0.0


class Builder:
    def __init__(self, nc, pool, ring_size=64, pool_frac=0.0,
                 pspool=None, wtile=None, wmax=0):
        self.nc = nc
        self.pool = pool
        self.n_stt = 0
        self.n_tt = 0
        self.n_act = 0
        self.n_copy = 0
        self.n_mm = 0
        # engine load balancing between DVE and GPSIMD (2-input ops)
        self.eng_busy = [0.0, 0.0]  # ns accumulated: [vector, gpsimd]
        self.eng_cost = [445.0, 980.0]
        self.pool_frac = pool_frac
        self.ring_size = ring_size
        self.ring_idx = 0
        self.joint_allocs = 0
        self.max_joint_allocs = 0
        self.pers_idx = 0
        self.free_tags = []       # recycled persistent tags
        self.free_wide = {}       # recycled wide tags by slot count
        self.pers_ids = set()     # id(ap) of planes safe to reference long-term
        self.ap_tag = {}          # id(ap) -> tag (for freeing)
        # ---- PE (TensorEngine) chain offload ----
        # Constant-coefficient linear combinations accumulate in PSUM via
        # diag-weight fp32 matmuls (exact: HW does the 4-pass H/L split).
        # Measured: ~490ns/term sustained incl. per-MM LDWEIGHTS, fully
        # parallel with DVE; DVE reads PSUM operands at SBUF cost.
        self.use_pe = wtile is not None and os.environ.get("K_PE", "1") == "1"
        self.pspool = pspool
        self.wtile = wtile
        self.wmax = wmax
        self.wave = 0
        self.wave_coefs = [[]]    # per wave: slot i holds diag(coefs[i])
        self.wslot = {}
        self.w_d = None           # wconst dram tensor (for wave refills)
        self.ps_ring = 0
        self.ps_ntags = int(os.environ.get("K_PSTAGS", "8"))
        # projected engine busy (ns) for greedy makespan routing
        self.busy = {"dve": 0.0, "act": 0.0, "pe": 0.0}
        self.pe_cost = float(os.environ.get("K_PE_COST", "500"))
        self.dve_cost = float(os.environ.get("K_DVE_COST", "424"))
        self.act_evac = 480.0
        self.dve_evac = 258.0

    def dve_track(self, n_ops, width=1):
        self.busy["dve"] += n_ops * (width * 267.0 + 157.0)

    @property
    def whalf(self):
        return self.wmax // 2

    def _wslot_room(self, coefs):
        cur = self.wave_coefs[self.wave]
        new = {c for c in coefs if c not in self.wslot}
        return len(cur) + len(new) <= self.whalf

    def _wslot_ap(self, c):
        s = self.wslot.get(c)
        if s is None:
            cur = self.wave_coefs[self.wave]
            s = len(cur)
            cur.append(c)
            self.wslot[c] = s
        base = (self.wave % 2) * self.whalf
        return self.wtile[:, (base + s) * P:(base + s + 1) * P]

    def _wave_dma(self, wave):
        """DMA wave's coefficient table into its half of the weight tile.

        Waves ping-pong between halves and each wave's DMA is issued one
        wave EARLY (the coefficient values live in DRAM, resolved at run
        time, so emission only needs the offsets): by the time a wave's
        LDWEIGHTS run, its table landed a full wave ago."""
        dram_base = wave * self.whalf * P
        sb_base = (wave % 2) * self.whalf * P
        wq = [self.nc.sync, self.nc.gpsimd] * 3
        nchunk = len(wq)
        bounds = [round(i * self.whalf / nchunk) for i in range(nchunk + 1)]
        for ci in range(nchunk):
            a, bnd = bounds[ci] * P, bounds[ci + 1] * P
            if a < bnd:
                wq[ci].dma_start(self.wtile[:, sb_base + a:sb_base + bnd],
                                 self.w_d.ap()[:, dram_base + a:dram_base + bnd])

    def new_wave(self):
        """Advance to a fresh coefficient table half; prefetch the next."""
        if not self.use_pe or self.wave + 1 >= self.n_waves:
            return
        self.wave += 1
        self.wave_coefs.append([])
        self.wslot = {}
        if self.wave + 1 < self.n_waves:
            self._wave_dma(self.wave + 1)

    def _psum(self):
        """PSUM chain slot. Banks are the allocation granularity (8), so two
        chain slots share each bank tile (has_written bits are per-element,
        so independent accumulation groups coexist in one bank)."""
        if os.environ.get("K_PSHALF", "0") == "1":
            n = self.ps_ring
            self.ps_ring += 1
            half = n % 2
            if half == 0:
                tag = f"ps{(n // 2) % self.ps_ntags}"
                self._ps_cur = self.pspool.tile([P, 2 * F], DT, tag=tag,
                                                name=tag)
            return self._ps_cur[:, half * F:(half + 1) * F]
        tag = f"ps{self.ps_ring % self.ps_ntags}"
        self.ps_ring += 1
        t = self.pspool.tile([P, 2 * F], DT, tag=tag, name=tag)
        return t[:, 0:F]

    def _pe_accum(self, terms):
        """PSUM <- sum(c*ap) over SBUF-resident terms via diag-weight MMs."""
        ps = self._psum()
        n = len(terms)
        for i, (c, ap) in enumerate(terms):
            w = self._wslot_ap(float(c))
            self.nc.tensor.matmul(ps, w, ap, start=(i == 0), stop=(i == n - 1))
        self.n_mm += n
        self.busy["pe"] += n * self.pe_cost
        return ps

    def _pe_split(self, terms):
        """(pe_terms, dve_terms, sigma) under space/slot constraints, or None.

        sigma flips the whole accumulation's sign when -c slots are a better
        match for the live weight table (the flip is undone at evac / in the
        returned coefficient), halving slot burn for +-c coefficient pairs.
        """
        if not self.use_pe:
            return None
        pe, dve = [], []
        for c, ap in terms:
            if ap.space == bass.MemorySpace.SBUF:
                pe.append((float(c), ap))
            else:
                dve.append((c, ap))
        if len(pe) < 2:
            return None
        hits_p = sum(1 for c, _ in pe if c in self.wslot)
        hits_n = sum(1 for c, _ in pe if -c in self.wslot)
        sigma = -1.0 if hits_n > hits_p else 1.0
        pe = [(sigma * c, ap) for c, ap in pe]
        if not self._wslot_room([c for c, _ in pe]):
            return None
        return pe, dve, sigma

    def _evac(self, dst, ps, scale, bias, dve_pref=False):
        ca = self.busy["act"] + self.act_evac
        cd = self.busy["dve"] + self.dve_evac
        mode = os.environ.get("K_EVAC", "auto")
        if mode == "act":
            dve_pref = False
            cd = ca + 1.0
        elif mode == "dve":
            dve_pref = True
        if dve_pref or cd < ca:
            self.nc.vector.tensor_scalar(dst, ps, float(scale), float(bias),
                                         OP.mult, OP.add)
            self.busy["dve"] = cd
            self.n_stt += 1
        else:
            self.nc.scalar.activation(dst, ps, AF.Copy, bias=float(bias),
                                      scale=float(scale))
            self.busy["act"] = ca
            self.n_act += 1

    def _pe_chain(self, terms, dst, scale=1.0, bias=0.0, dve_seed=False,
                  seed_act=True):
        """Try PE route for sum(c*ap)*scale(+bias). Returns result ap or None.

        dst None: result stays in PSUM (requires scale==1, bias==0 handled by
        caller convention). dst given: evacuated into dst (SBUF).
        Split chains merge the DVE-side terms into dst with the PSUM partial
        as a free STT src1 operand.
        """
        limit = int(os.environ.get("K_PE_LIMIT", "100000"))
        if getattr(self, "pe_chains", 0) >= limit:
            return None
        split = self._pe_split(terms)
        if split is None:
            return None
        pe, dve, sigma = split
        n_pe, n_dve = len(pe), len(dve)
        # status quo: whole chain on DVE (+ACT seed when it would use one)
        sq_d = self.busy["dve"] + (len(terms) - 1) * self.dve_cost
        sq_a = self.busy["act"] + (self.act_evac if seed_act else 0.0)
        mk_sq = max(sq_d, sq_a, self.busy["pe"])
        ev = 0.0 if (dst is None and n_dve == 0) else \
            (0.0 if n_dve else min(self.act_evac, self.dve_evac))
        pe_d = self.busy["dve"] + n_dve * self.dve_cost
        pe_p = self.busy["pe"] + n_pe * self.pe_cost
        mk_pe = max(pe_d, self.busy["act"], pe_p) + ev
        if mk_pe >= mk_sq:
            return None
        self.pe_chains = getattr(self, "pe_chains", 0) + 1
        if dst is None and n_dve:
            # merge would need an SBUF dst anyway; use scratch
            dst_eff = self.scratch()
        else:
            dst_eff = dst
        ps = self._pe_accum(pe)  # holds sigma * sum(c_i x_i) over pe terms
        if n_dve == 0:
            if dst_eff is None:
                return ps, sigma
            self._evac(dst_eff, ps, sigma * scale, bias, dve_pref=dve_seed)
            return dst_eff, 1.0
        # DVE merge. The leftover (non-SBUF) terms are PSUM-resident, so the
        # partial must leave PSUM first (one PSUM operand per DVE op).
        self.nc.vector.tensor_scalar(dst_eff, ps, float(sigma * scale),
                                     float(bias), OP.mult, OP.add)
        self.n_stt += 1
        self.busy["dve"] += self.dve_evac
        for ck, xk in dve:
            self.nc.vector.scalar_tensor_tensor(
                dst_eff, xk, float(ck * scale), dst_eff, OP.mult, OP.add)
            self.n_stt += 1
            self.busy["dve"] += self.dve_cost
        return dst_eff, 1.0

    def pick_engine(self, n_ops=1):
        """Pick vector or gpsimd for a chain of n_ops 2-input ops."""
        self.busy["dve"] += n_ops * self.dve_cost
        if self.pool_frac <= 0.0:
            self.eng_busy[0] += n_ops * self.eng_cost[0]
            return self.nc.vector
        c0 = self.eng_busy[0] + n_ops * self.eng_cost[0]
        c1 = self.eng_busy[1] + n_ops * self.eng_cost[1]
        if c1 < c0:
            self.eng_busy[1] = c1
            return self.nc.gpsimd
        self.eng_busy[0] = c0
        return self.nc.vector

    # ---- allocation ----
    def scratch(self, dtype=None):
        tag = f"s{self.ring_idx % self.ring_size}"
        t = self.pool.tile([P, F], dtype or DT_C, tag=tag, name=tag)
        self.ring_idx += 1
        self.joint_allocs += 1
        return t[:, :]

    def persistent(self, label=""):
        if self.free_tags:
            tag = self.free_tags.pop()
        else:
            tag = f"p{self.pers_idx}"
            self.pers_idx += 1
        t = self.pool.tile([P, F], DT_C, tag=tag, name=f"{tag}_{label}")
        ap = t[:, :]
        self.pers_ids.add(id(ap))
        self.ap_tag[id(ap)] = ("p", tag)
        return ap

    def wscratch(self, slots):
        """Wide scratch tile [P, slots*F] from a per-width ring."""
        if not hasattr(self, "wring"):
            self.wring = {}
        idx = self.wring.get(slots, 0)
        self.wring[slots] = idx + 1
        mod = {2: 2, 3: 5, 4: 8}.get(slots, 10)
        tag = f"w{slots}_{idx % mod}"
        t = self.pool.tile([P, slots * F], DT_C, tag=tag, name=tag)
        return t[:, :]

    def persistent_wide(self, slots, label=""):
        if not hasattr(self, "pwide_idx"):
            self.pwide_idx = 0
        fl = self.free_wide.setdefault(slots, [])
        if fl:
            tag = fl.pop()
        else:
            tag = f"pw{slots}_{self.pwide_idx}"
            self.pwide_idx += 1
        t = self.pool.tile([P, slots * F], DT_C, tag=tag, name=f"{tag}_{label}")
        return t[:, :], tag

    def packed_pair_rot(self, Y0, Y1, t4, sign, dst0, dst1, pers,
                        A_ready=None):
        """Rotate one (y0, y1) pair by the z-angle whose 4-slot trig tile is
        t4 = [cos | sin/2 | -sin/2 | cos].

        sign=+1: z0 = c y0 + s y1, z1 = -s y0 + c y1   (Rz^T)
        sign=-1: z0 = c y0 - s y1, z1 = +s y0 + c y1   (Rz)
        dst0/dst1: [P, F] planes receiving z0/a0, z1/a1.
        A_ready: optional (A_region[P,2F], a0, a1, k0, k1) when the pair is
        already materialized adjacently (coefs a*, deferred consts k*).
        Returns (z0_expr, z1_expr) incl. rotated deferred-const terms.
        """
        if A_ready is None:
            W = self.wscratch(2)
            a0, _ = self.mat(Y0, dst=W[:, 0:F])
            a1, _ = self.mat(Y1, dst=W[:, F:2 * F])
            k0, k1 = Y0.const, Y1.const
            A = W
        else:
            A, a0, a1, k0, k1 = A_ready
        c_slot = t4[:, 0:F]
        s_slot = t4[:, F:2 * F]
        if sign > 0:
            B1 = t4[:, 0:2 * F]            # [c | s/2]
            B2 = t4[:, 2 * F:4 * F]        # [-s/2 | c]
            # P1 = [c*y0r | (s/2)*y1r]; P2 = [(-s/2)*y0r | c*y1r]
            # z0 = a0*P1s0 + 2 a1*P1s1 ; z1 = 2 a0*P2s0 + a1*P2s1
        else:
            t4r = t4.rearrange("p (a c b) -> p a c b", a=2, c=2, b=F)
            B1 = t4r[:, :, 0, :]           # [c | -s/2]
            B2 = t4r[:, :, 1, :]           # [s/2 | c]
            # P1 = [c*y0r | (-s/2)*y1r]: z0 = a0 P1s0 + 2 a1 P1s1
            # P2 = [(s/2)*y0r | c*y1r]:  z1 = 2 a0 P2s0 + a1 P2s1
        P1 = self.wscratch(2)
        P2 = self.wscratch(2)
        self.nc.vector.tensor_tensor(P1, A, B1, OP.mult)
        self.nc.vector.tensor_tensor(P2, A, B2, OP.mult)
        self.n_tt += 2
        self.nc.vector.scalar_tensor_tensor(
            dst0, P1[:, F:2 * F], 2.0 * a1 / a0, P1[:, 0:F], OP.mult, OP.add)
        self.nc.vector.scalar_tensor_tensor(
            dst1, P2[:, 0:F], 2.0 * a0 / a1, P2[:, F:2 * F], OP.mult, OP.add)
        self.n_stt += 2
        if pers:
            self.pers_ids.add(id(dst0))
            self.pers_ids.add(id(dst1))
        t0 = [(a0, dst0)]
        t1 = [(a1, dst1)]
        if k0 != 0.0:
            t0.append((k0, c_slot))
            t1.append((-sign * 2.0 * k0, s_slot))
        if k1 != 0.0:
            t0.append((sign * 2.0 * k1, s_slot))
            t1.append((k1, c_slot))
        e0 = Expr(t0)
        e1 = Expr(t1)
        if len(t0) == 1:
            e0._mat = (a0, dst0)
        if len(t1) == 1:
            e1._mat = (a1, dst1)
        return e0, e1

    def quad_rot2(self, pairs, c_ap, s_ap, label):
        """quad_rot with broadcast trig reads: no replicated-scaled trig
        copies (ACT -16/joint). Operands materialize EXACTLY (A=y0, B=2*y1);
        the sin/2 scale and z1's overall 1/2 ride the broadcast pattern and
        the returned Expr coefficients.
          z0 = c y0 + s y1 = (A*c + B*(s/2))        -> coef 1
          z1 = -s y0 + c y1 = 0.5*(B*c - 4*A*(s/2)) -> coef 0.5
        """
        W = len(pairs)
        assert W == 4
        A = self.wscratch(W)
        Bt = self.wscratch(W)
        for k, (y0, y1) in enumerate(pairs):
            sl = slice(k * F, (k + 1) * F)
            self.mat_exact(y0, A[:, sl])
            self.mat_exact(y1, Bt[:, sl], scale=2.0)
        cb = c_ap[:, None, :].broadcast_to([P, W, F])
        sb = s_ap[:, None, :].broadcast_to([P, W, F])
        v = self.nc.vector
        P1 = self.wscratch(W)
        P2 = self.wscratch(W)
        v.tensor_tensor(P1, A, cb, OP.mult)
        v.tensor_tensor(P2, Bt, sb, OP.mult)
        ZA1, t1 = self.persistent_wide(2, f"{label}a1")
        ZB1 = self.wscratch(2)
        v.tensor_tensor(ZA1, P1[:, 0:2 * F], P2[:, 0:2 * F], OP.add)
        v.tensor_tensor(ZB1, P1[:, 2 * F:4 * F], P2[:, 2 * F:4 * F], OP.add)
        v.tensor_tensor(P1, A, sb, OP.mult)
        v.tensor_tensor(P2, Bt, cb, OP.mult)
        ZA2, t2 = self.persistent_wide(2, f"{label}a2")
        ZB2 = self.wscratch(2)
        v.scalar_tensor_tensor(ZA2, P1[:, 0:2 * F], -4.0, P2[:, 0:2 * F],
                               OP.mult, OP.add)
        v.scalar_tensor_tensor(ZB2, P1[:, 2 * F:4 * F], -4.0,
                               P2[:, 2 * F:4 * F], OP.mult, OP.add)
        self.n_tt += 6
        self.n_stt += 2
        self.dve_track(4, 4)
        self.dve_track(4, 2)

        def _mk(region, k, coef):
            ap = region[:, k * F:(k + 1) * F]
            self.pers_ids.add(id(ap))
            e = Expr([(coef, ap)])
            e._mat = (coef, ap)
            return e

        z0s = [_mk(ZA1, 0, 1.0), _mk(ZA1, 1, 1.0),
               _mk(ZB1, 0, 1.0), _mk(ZB1, 1, 1.0)]
        z1s = [_mk(ZA2, 0, 0.5), _mk(ZA2, 1, 0.5),
               _mk(ZB2, 0, 0.5), _mk(ZB2, 1, 0.5)]
        return z0s, z1s, (t1, t2)

    def pair_rot_bwd2(self, lf01, af01, c_ap, s_ap, label, dve_seed=False):
        """pair_rot_bwd with broadcast trig reads (ACT -8/joint).
          z0 = c y0 - s y1 -> coef 1 ; z1 = s y0 + c y1 -> coef 0.5
        """
        T = self.wscratch(4)
        self.mat_exact(lf01[0], T[:, 0:F], dve_seed=dve_seed)
        self.mat_exact(af01[0], T[:, F:2 * F], dve_seed=dve_seed)
        self.mat_exact(lf01[1], T[:, 2 * F:3 * F], dve_seed=dve_seed,
                       scale=2.0)
        self.mat_exact(af01[1], T[:, 3 * F:4 * F], dve_seed=dve_seed,
                       scale=2.0)
        Av = T[:, 0:2 * F]
        Bv = T[:, 2 * F:4 * F]
        cb = c_ap[:, None, :].broadcast_to([P, 2, F])
        sb = s_ap[:, None, :].broadcast_to([P, 2, F])
        v = self.nc.vector
        Pt = self.wscratch(4)
        Z = self.wscratch(4)
        v.tensor_tensor(Pt[:, 0:2 * F], Av, cb, OP.mult)
        v.tensor_tensor(Pt[:, 2 * F:4 * F], Bv, sb, OP.mult)
        v.tensor_tensor(Z[:, 0:2 * F], Pt[:, 0:2 * F], Pt[:, 2 * F:4 * F],
                        OP.subtract)
        v.tensor_tensor(Pt[:, 0:2 * F], Av, sb, OP.mult)
        v.tensor_tensor(Pt[:, 2 * F:4 * F], Bv, cb, OP.mult)
        v.scalar_tensor_tensor(Z[:, 2 * F:4 * F], Pt[:, 0:2 * F], 4.0,
                               Pt[:, 2 * F:4 * F], OP.mult, OP.add)
        self.n_tt += 5
        self.n_stt += 1
        self.dve_track(6, 2)

        def _e(sl, coef):
            ap = Z[:, sl * F:(sl + 1) * F]
            self.pers_ids.add(id(ap))
            e = Expr([(coef, ap)])
            e._mat = (coef, ap)
            return e

        return (_e(0, 1.0), _e(2, 0.5)), (_e(1, 1.0), _e(3, 0.5))

    def act_copy(self, dst, src, scale=1.0, bias=0.0):
        self.nc.scalar.activation(dst, src, AF.Copy, bias=float(bias),
                                  scale=float(scale))
        self.n_act += 1
        self.busy["act"] += 480.0

    def mat_exact(self, e, dst, dve_seed=False, scale=1.0):
        """Materialize the EXACT value of e into dst (coef 1, const folded).

        Unlike mat(), the result plane holds sum(coef*plane)+const verbatim,
        so packed slot-aligned products across different slots stay
        coefficient-consistent. Costs the same n-1 STT as mat(); a leading
        ACT copy (idle engine) absorbs the lead coef and the constant when
        no unit-coefficient lead term exists.
        """
        terms = sorted(e.terms, key=lambda t: -abs(t[0]))
        if not terms:
            self.nc.vector.memset(dst, float(e.const) * scale)
            return
        const = e.const * scale
        if len(terms) >= 2 and self._pe_chain(
                terms, dst, scale, const, dve_seed=dve_seed,
                seed_act=not dve_seed) is not None:
            return
        if scale != 1.0:
            terms = [(c * scale, ap) for c, ap in terms]
        unit = next((i for i, (ck, _) in enumerate(terms)
                     if ck == 1.0), None)
        if len(terms) == 1:
            c0, x0 = terms[0]
            if dve_seed:
                self.nc.vector.tensor_scalar(
                    dst, x0, float(c0), const, OP.mult, OP.add)
                self.n_stt += 1
            else:
                self.act_copy(dst, x0, scale=c0, bias=const)
            return
        if unit is not None and const == 0.0:
            c0, x0 = terms.pop(unit)
            ck, xk = terms.pop(0)
            self.nc.vector.scalar_tensor_tensor(
                dst, xk, float(ck), x0, OP.mult, OP.add)
            self.n_stt += 1
        else:
            c0, x0 = terms.pop(0)
            if dve_seed:
                self.nc.vector.tensor_scalar(
                    dst, x0, float(c0), const, OP.mult, OP.add)
                self.n_stt += 1
            else:
                self.act_copy(dst, x0, scale=c0, bias=const)
        for ck, xk in terms:
            self.nc.vector.scalar_tensor_tensor(
                dst, xk, float(ck), dst, OP.mult, OP.add)
            self.n_stt += 1

    def cyc_tile(self, vec, label, dve_seed=False):
        """[P,5F] tile holding [x0|x1|x2|x0|x1] of a 3-vector of Exprs."""
        t, tag = self.persistent_wide(5, label)
        for k in range(3):
            self.mat_exact(vec[k], t[:, k * F:(k + 1) * F], dve_seed=dve_seed)
        if dve_seed:
            self.nc.vector.tensor_scalar(
                t[:, 3 * F:5 * F], t[:, 0:2 * F], 1.0, None, OP.mult)
            self.n_stt += 1
        else:
            self.act_copy(t[:, 3 * F:4 * F], t[:, 0:F])
            self.act_copy(t[:, 4 * F:5 * F], t[:, F:2 * F])
        return t, tag

    def mat_rel(self, e, dst, dve_seed=False):
        """Materialize (value incl. const)/c0 into dst; returns c0.

        Like mat() but the constant is folded into the plane (via an ACT
        seed copy on the idle scalar engine), so rotating the plane rotates
        the full affine value and downstream exprs stay pure. dve_seed=True
        seeds with a DVE tensor_scalar instead: at the backward tail the
        ACT queue latency sits on the critical path (DVE has nothing left
        to overlap), so cross-engine seeding there costs ~0.5us per chain.
        """
        assert e.terms
        terms = sorted(e.terms, key=lambda t: -abs(t[0]))
        c0, x0 = terms[0]
        if len(terms) >= 2 and self._pe_chain(
                terms, dst, 1.0 / c0, e.const / c0, dve_seed=dve_seed,
                seed_act=(e.const != 0.0) and not dve_seed) is not None:
            return c0
        if e.const != 0.0 or len(terms) == 1:
            if dve_seed:
                self.nc.vector.tensor_scalar(
                    dst, x0, 1.0, e.const / c0, OP.mult, OP.add)
                self.n_stt += 1
            else:
                self.act_copy(dst, x0, scale=1.0, bias=e.const / c0)
            rest = terms[1:]
        else:
            c1, x1 = terms[1]
            self.nc.vector.scalar_tensor_tensor(
                dst, x1, c1 / c0, x0, OP.mult, OP.add)
            self.n_stt += 1
            rest = terms[2:]
        for ck, xk in rest:
            self.nc.vector.scalar_tensor_tensor(
                dst, xk, ck / c0, dst, OP.mult, OP.add)
            self.n_stt += 1
        return c0

    def quad_rot(self, pairs, c_ap, s_ap, label):
        """Rz^T-rotate W (Y0,Y1) expr pairs sharing one joint angle.

        z0 = c Y0 + s Y1 ; z1 = -s Y0 + c Y1. s_ap holds sin/2 (the 2x is
        folded into the replicated-trig scales). Y consts fold into slot
        contents (mat_rel), so outputs are pure planes. Slot coefficients
        fold into the per-slot scales of ACT-replicated trig tiles, letting
        each product group and each combine run as ONE wide DVE op.
        Returns (z0_exprs, z1_exprs, persistent_tags): z0/z1 of pairs [0,1]
        land in persistent [P,2F] tiles (joint states w/v); pairs [2,3] in
        scratch (consumed by the same joint's dw/dv).
        """
        W = len(pairs)
        assert W == 4
        A = self.wscratch(W)
        Bt = self.wscratch(W)
        CA = self.wscratch(W)
        SB = self.wscratch(W)
        SmA = self.wscratch(W)
        CB = self.wscratch(W)
        aA, aB = [], []
        for k, (y0, y1) in enumerate(pairs):
            sl = slice(k * F, (k + 1) * F)
            aA.append(self.mat_rel(y0, A[:, sl]))
            aB.append(self.mat_rel(y1, Bt[:, sl]))
        for k in range(W):
            sl = slice(k * F, (k + 1) * F)
            self.act_copy(CA[:, sl], c_ap, scale=aA[k])
            self.act_copy(SB[:, sl], s_ap, scale=2.0 * aB[k])
            self.act_copy(SmA[:, sl], s_ap, scale=-2.0 * aA[k])
            self.act_copy(CB[:, sl], c_ap, scale=aB[k])
        v = self.nc.vector
        P1 = self.wscratch(W)
        P2 = self.wscratch(W)
        v.tensor_tensor(P1, A, CA, OP.mult)
        v.tensor_tensor(P2, Bt, SB, OP.mult)
        ZA1, t1 = self.persistent_wide(2, f"{label}a1")
        ZB1 = self.wscratch(2)
        v.tensor_tensor(ZA1, P1[:, 0:2 * F], P2[:, 0:2 * F], OP.add)
        v.tensor_tensor(ZB1, P1[:, 2 * F:4 * F], P2[:, 2 * F:4 * F], OP.add)
        v.tensor_tensor(P1, A, SmA, OP.mult)
        v.tensor_tensor(P2, Bt, CB, OP.mult)
        ZA2, t2 = self.persistent_wide(2, f"{label}a2")
        ZB2 = self.wscratch(2)
        v.tensor_tensor(ZA2, P1[:, 0:2 * F], P2[:, 0:2 * F], OP.add)
        v.tensor_tensor(ZB2, P1[:, 2 * F:4 * F], P2[:, 2 * F:4 * F], OP.add)
        self.n_tt += 8
        self.dve_track(4, 4)
        self.dve_track(4, 2)

        def _mk(region, k):
            ap = region[:, k * F:(k + 1) * F]
            self.pers_ids.add(id(ap))
            e = Expr([(1.0, ap)])
            e._mat = (1.0, ap)
            return e

        z0s = [_mk(ZA1, 0), _mk(ZA1, 1), _mk(ZB1, 0), _mk(ZB1, 1)]
        z1s = [_mk(ZA2, 0), _mk(ZA2, 1), _mk(ZB2, 0), _mk(ZB2, 1)]
        return z0s, z1s, (t1, t2)

    def pair_rot_bwd(self, lf01, af01, c_ap, s_ap, label, dve_seed=False):
        """Rz-rotate (lf0,lf1) and (af0,af1): z0 = c y0 - s y1; z1 = s y0 + c y1.

        Raw force exprs land exactly (consts folded, lead coefs in the
        ACT-replicated trig scales) in one [lf0|lf1|af0|af1] tile; the two
        rotations then cost 4 wide products + 2 wide combines on [P,2F]
        instead of 8 narrow products + 4 chain materializations.
        Returns ((RzLf0e, RzLf1e), (RzAf0e, RzAf1e)) as pure plane exprs.
        """
        T = self.wscratch(4)
        coefs = []
        for k, e in enumerate([lf01[0], lf01[1], af01[0], af01[1]]):
            coefs.append(self.mat_rel(e, T[:, k * F:(k + 1) * F],
                                      dve_seed=dve_seed))
        aL0, aL1, aA0, aA1 = coefs
        Tr = T.rearrange("p (a b f) -> p a b f", a=2, b=2, f=F)
        Av = Tr[:, :, 0, :]   # [lf0 | af0]
        Bv = Tr[:, :, 1, :]   # [lf1 | af1]
        R1 = self.wscratch(4)  # [c*aL0 | c*aA0 | -s*aL1 | -s*aA1]
        self.act_copy(R1[:, 0:F], c_ap, scale=aL0)
        self.act_copy(R1[:, F:2 * F], c_ap, scale=aA0)
        self.act_copy(R1[:, 2 * F:3 * F], s_ap, scale=-2.0 * aL1)
        self.act_copy(R1[:, 3 * F:4 * F], s_ap, scale=-2.0 * aA1)
        R2 = self.wscratch(4)  # [s*aL0 | s*aA0 | c*aL1 | c*aA1]
        self.act_copy(R2[:, 0:F], s_ap, scale=2.0 * aL0)
        self.act_copy(R2[:, F:2 * F], s_ap, scale=2.0 * aA0)
        self.act_copy(R2[:, 2 * F:3 * F], c_ap, scale=aL1)
        self.act_copy(R2[:, 3 * F:4 * F], c_ap, scale=aA1)
        v = self.nc.vector
        Pt = self.wscratch(4)
        Z = self.wscratch(4)
        v.tensor_tensor(Pt[:, 0:2 * F], Av, R1[:, 0:2 * F], OP.mult)
        v.tensor_tensor(Pt[:, 2 * F:4 * F], Bv, R1[:, 2 * F:4 * F], OP.mult)
        v.tensor_tensor(Z[:, 0:2 * F], Pt[:, 0:2 * F], Pt[:, 2 * F:4 * F],
                        OP.add)
        v.tensor_tensor(Pt[:, 0:2 * F], Av, R2[:, 0:2 * F], OP.mult)
        v.tensor_tensor(Pt[:, 2 * F:4 * F], Bv, R2[:, 2 * F:4 * F], OP.mult)
        v.tensor_tensor(Z[:, 2 * F:4 * F], Pt[:, 0:2 * F], Pt[:, 2 * F:4 * F],
                        OP.add)
        self.n_tt += 6
        self.dve_track(6, 2)

        def _e(sl):
            ap = Z[:, sl * F:(sl + 1) * F]
            self.pers_ids.add(id(ap))
            e = Expr([(1.0, ap)])
            e._mat = (1.0, ap)
            return e

        return (_e(0), _e(2)), (_e(1), _e(3))

    def cross_packed(self, cycA, cycB):
        """cross(A, B) of two cyc tiles -> [P,3F] tile [c0|c1|c2]."""
        Pa = self.wscratch(3)
        Pb = self.wscratch(3)
        self.nc.vector.tensor_tensor(
            Pa, cycA[:, F:4 * F], cycB[:, 2 * F:5 * F], OP.mult)
        self.nc.vector.tensor_tensor(
            Pb, cycA[:, 2 * F:5 * F], cycB[:, F:4 * F], OP.mult)
        self.nc.vector.tensor_tensor(Pa, Pa, Pb, OP.subtract)
        self.n_tt += 3
        self.dve_track(3, 3)
        return Pa

    def free_expr_vec(self, vec):
        for e in vec:
            for _, ap in e.terms:
                ent = self.ap_tag.pop(id(ap), None)
                if ent is None:
                    continue
                self.pers_ids.discard(id(ap))
                if ent[0] == "p":
                    self.free_tags.append(ent[1])
                else:
                    self.free_wide.setdefault(ent[1], []).append(ent[2])

    def joint_boundary(self):
        self.max_joint_allocs = max(self.max_joint_allocs, self.joint_allocs)
        self.joint_allocs = 0

    # ---- expression ops ----
    def lin(self, *pairs, const=0.0):
        acc = {}
        aps = {}
        c_acc = float(const)
        for coef, e in pairs:
            if coef == 0.0 or e is None or e is ZERO and e.const == 0.0:
                if e is not None:
                    c_acc += coef * e.const
                continue
            c_acc += coef * e.const
            for tc, ap in e.terms:
                k = id(ap)
                acc[k] = acc.get(k, 0.0) + coef * tc
                aps[k] = ap
        terms = [(c, aps[k]) for k, c in acc.items() if c != 0.0]
        return Expr(terms, c_acc)

    def mat(self, e, dst=None):
        """Materialize sum-of-terms: e == coef*ap + e.const -> (coef, ap)."""
        assert e.terms, "cannot materialize empty expr"
        if e._mat is not None and dst is None:
            return e._mat
        terms = sorted(e.terms, key=lambda t: -abs(t[0]))
        if len(terms) == 1 and dst is None:
            e._mat = (terms[0][0], terms[0][1])
            return e._mat
        c0, x0 = terms[0]
        if len(terms) == 1:
            self.nc.vector.tensor_scalar(dst, x0, 1.0, None, OP.mult)
            self.n_copy += 1
            e._mat = (c0, dst)
            return e._mat
        if not hasattr(self, "mat_hist"):
            self.mat_hist = {}
        self.mat_hist[len(terms)] = self.mat_hist.get(len(terms), 0) + 1
        r = self._pe_chain(terms, dst, 1.0, 0.0, seed_act=False)
        if r is not None:
            e._mat = (r[1], r[0])
            return e._mat
        t = dst if dst is not None else self.scratch()
        c1, x1 = terms[1]
        eng = self.pick_engine(len(terms) - 1)
        eng.scalar_tensor_tensor(t, x1, c1 / c0, x0, OP.mult, OP.add)
        self.n_stt += 1
        for ck, xk in terms[2:]:
            eng.scalar_tensor_tensor(t, xk, ck / c0, t, OP.mult, OP.add)
            self.n_stt += 1
        e._mat = (c0, t)
        return e._mat

    def mul(self, x, y):
        if not _nonzero(x) or not _nonzero(y):
            return ZERO
        if not x.terms:  # pure const
            return Expr([(x.const * c, ap) for c, ap in y.terms],
                        x.const * y.const)
        if not y.terms:
            return Expr([(y.const * c, ap) for c, ap in x.terms],
                        x.const * y.const)
        cx, ax = self.mat(x)
        cy, ay = self.mat(y)
        if ax.space == bass.MemorySpace.PSUM \
                and ay.space == bass.MemorySpace.PSUM:
            tmp = self.scratch()
            self.nc.vector.tensor_scalar(tmp, ay, 1.0, None, OP.mult)
            self.n_stt += 1
            self.busy["dve"] += self.dve_evac
            ay = tmp
        prod = self.scratch()
        self.pick_engine(1).tensor_tensor(prod, ax, ay, OP.mult)
        self.n_tt += 1
        terms = [(cx * cy, prod)]
        if y.const != 0.0:
            terms.append((cx * y.const, ax))
        if x.const != 0.0:
            terms.append((cy * x.const, ay))
        return Expr(terms, x.const * y.const)

    def snap(self, e, label="", scratch_ok=False):
        """Materialize into a stable plane; returns single-term Expr."""
        if not e.terms:
            return e
        if len(e.terms) == 1 and e._mat is None \
                and id(e.terms[0][1]) in self.pers_ids and not scratch_ok:
            out = Expr(list(e.terms), e.const)
            out._mat = e.terms[0]
            return out
        if e._mat is not None:
            c, src = e._mat
            if id(src) in self.pers_ids or scratch_ok:
                out = Expr([(c, src)], e.const)
                out._mat = (c, src)
                return out
            dst = self.persistent(label)
            self.nc.scalar.activation(dst, src, AF.Copy, bias=0.0, scale=1.0)
            self.n_copy += 1
            out = Expr([(c, dst)], e.const)
            out._mat = (c, dst)
            return out
        dst = self.scratch() if scratch_ok else self.persistent(label)
        c, ap = self.mat(e, dst=dst)
        out = Expr([(c, ap)], e.const)
        out._mat = (c, ap)
        return out

    def snap_to(self, e, dst_ap):
        """Materialize into the given plane; returns single-term Expr."""
        assert e.terms
        c, ap = self.mat(e, dst=dst_ap)
        self.pers_ids.add(id(ap))
        out = Expr([(c, ap)], e.const)
        out._mat = (c, ap)
        return out

    def snap_vec(self, vec, label="", scratch_ok=False):
        return [self.snap(e, f"{label}{i}", scratch_ok) for i, e in enumerate(vec)]

    # ---- 3-vector helpers ----
    def vadd(self, *vecs):
        return [self.lin(*[(1.0, v[i]) for v in vecs]) for i in range(3)]

    def vsub(self, a, b):
        return [self.lin((1.0, a[i]), (-1.0, b[i])) for i in range(3)]

    def cross_const(self, t, X):
        return [
            self.lin((-t[2], X[1]), (t[1], X[2])),
            self.lin((t[2], X[0]), (-t[0], X[2])),
            self.lin((-t[1], X[0]), (t[0], X[1])),
        ]

    def cross_ee(self, A, B):
        return [
            self.lin((1.0, self.mul(A[1], B[2])), (-1.0, self.mul(A[2], B[1]))),
            self.lin((1.0, self.mul(A[2], B[0])), (-1.0, self.mul(A[0], B[2]))),
            self.lin((1.0, self.mul(A[0], B[1])), (-1.0, self.mul(A[1], B[0]))),
        ]

    def matvec_const(self, M, X):
        return [
            self.lin((M[i, 0], X[0]), (M[i, 1], X[1]), (M[i, 2], X[2]))
            for i in range(3)
        ]


def build_module(params):
    trans = np.asarray(params["trans"], np.float64)
    rot_fix = np.asarray(params["rot_fix"], np.float64)
    mass = np.asarray(params["mass"], np.float64)
    com = np.asarray(params["com"], np.float64)
    inertia = np.asarray(params["inertia"], np.float64)
    damping = np.asarray(params["damping"], np.float64)

    nc = bacc.Bacc("TRN2", target_bir_lowering=False, debug=False,
                   num_devices=N_CORES)
    q_d = nc.dram_tensor("q", (BC, ND), DT, kind="ExternalInput")
    qd_d = nc.dram_tensor("qd", (BC, ND), DT, kind="ExternalInput")
    qdd_d = nc.dram_tensor("qdd_des", (BC, ND), DT, kind="ExternalInput")
    use_pe = os.environ.get("K_PE", "1") == "1"
    wmax = int(os.environ.get("K_WMAX", "46"))
    n_waves = int(os.environ.get("K_NWAVES", "10"))
    if use_pe:
        w_d = nc.dram_tensor("wconst", (P, n_waves * (wmax // 2) * P), DT,
                             kind="ExternalInput")
    tq_d = nc.dram_tensor("torque", (BC, ND), DT, kind="ExternalOutput")

    with tile_mod.TileContext(nc) as tc:
        with tc.tile_pool(name="main", bufs=1) as pool, \
             tc.tile_pool(name="io", bufs=1) as io_pool, \
             tc.psum_pool(name="psp", bufs=1) as pspool:
            wtile_ap = None
            if use_pe:
                wtile = io_pool.tile([P, wmax * P], DT, tag="wconst",
                                     name="wconst_sb")
                wtile_ap = wtile[:, :]
            b = Builder(nc, pool,
                        ring_size=int(os.environ.get("K_RING", "11")),
                        pool_frac=float(os.environ.get("K_POOL_FRAC", "0")),
                        pspool=pspool, wtile=wtile_ap, wmax=wmax)
            b.n_waves = n_waves
            if use_pe:
                b.w_d = w_d

            q_t = io_pool.tile([P, F * ND], DT, tag="q", name="q_sb")
            qd_t = io_pool.tile([P, F * ND], DT, tag="qd", name="qd_sb")
            qdd_t = io_pool.tile([P, F * ND], DT, tag="qdd", name="qdd_sb")
            # out aliases q's buffer: q is fully consumed by the trig
            # prologue long before the first backward write_out.
            out_t = io_pool.tile([P, F * ND], DT, tag="q", name="out_sb")
            dram_view = lambda t: t.ap().rearrange("(p f) d -> p (f d)", p=P)
            # q gates trig (the whole critical path): give it the sync queue
            # alone; qd/qdd ride the idle gpsimd queue in parallel.
            nc.sync.dma_start(q_t[:, :], dram_view(q_d))
            nc.gpsimd.dma_start(qd_t[:, :], dram_view(qd_d))
            nc.gpsimd.dma_start(qdd_t[:, :], dram_view(qdd_d))
            if use_pe:
                # weight slots are allocated in first-use (= emission) order,
                # so chunked DMA in slot order arrives before consumers.
                # Waves 0 and 1 load up front; later waves prefetch 1 ahead.
                b._wave_dma(0)
                if n_waves > 1:
                    b._wave_dma(1)

            q3 = q_t[:, :].rearrange("p (f d) -> p f d", d=ND)
            qd3 = qd_t[:, :].rearrange("p (f d) -> p f d", d=ND)
            qdd3 = qdd_t[:, :].rearrange("p (f d) -> p f d", d=ND)
            out3 = out_t[:, :].rearrange("p (f d) -> p f d", d=ND)

            def as_pers_expr(ap):
                b.pers_ids.add(id(ap))
                e = Expr([(1.0, ap)])
                e._mat = (1.0, ap)
                return e

            def const_col(tag, val):
                t = io_pool.tile([P, 1], DT, tag=tag, name=tag)
                nc.vector.memset(t[:, :], float(val))
                return t

            zero_t = const_col("zconst", 0.0)
            # dummy Sin on an immediately-ready tile: hoists the ~2.7us ACT
            # table load to t~0, overlapping the input DMA instead of
            # serializing after it.
            warm_t = io_pool.tile([P, 1], DT, tag="warm", name="warm")
            nc.scalar.activation(warm_t[:, :], zero_t[:, :], AF.Sin,
                                 bias=zero_t[:, :], scale=1.0)

            # ACT Sin has no range reduction (accurate only on [-pi, pi]).
            # q ~ N(0,1) so |q| <= ~5.5: one conditional wrap of 2*pi covers
            # sin(q) and cos(q) = sin((q + pi/2) wrapped).
            PI = float(np.pi)
            TWO_PI = float(2 * np.pi)
            trig = {}
            trig_raw = {}
            _kp = os.environ.get("K_PACK", "0")
            use_pack_fwd = _kp in ("1", "fwd")
            use_pack_bwd = _kp in ("1", "bwd")
            use_xpack = os.environ.get("K_XPACK", "1") == "1"
            def emit_trig(j):
                # ACT Sin is only accurate on [-pi, pi] (no range reduction)
                # and q ~ N(0,1) exceeds pi. Half-angle identities avoid any
                # wrapping: |q/2| <= ~2.9 < pi and |q/4| < pi/2 always.
                #   cos q = 1 - 2 sin^2(q/2)
                #   sin q = 2 sin(q/2) cos(q/2),  cos(q/2) = 1 - 2 sin^2(q/4)
                # Everything runs on the otherwise-idle ACT engine except one
                # DVE product for sin. Emitted lazily (joint j+1's chain goes
                # out with joint j's body) so the in-order ACT queue reaches
                # each joint's replicated-trig copies just before the DVE
                # needs them, instead of front-loading all six chains.
                if j in trig:
                    return
                qj = q3[:, :, j]
                t4w = 4 if (use_pack_fwd or use_pack_bwd) else 2
                t4, _t4tag = b.persistent_wide(t4w, f"t4_{j}")
                c_ap = t4[:, 0:F]
                s_ap = t4[:, F:2 * F]
                b.pers_ids.add(id(c_ap))
                b.pers_ids.add(id(s_ap))
                sh = b.scratch(DT)   # sin(q/2)
                nc.scalar.activation(sh, qj, AF.Sin, bias=zero_t[:, :],
                                     scale=0.5)
                sq = b.scratch(DT)   # sin(q/4)
                nc.scalar.activation(sq, qj, AF.Sin, bias=zero_t[:, :],
                                     scale=0.25)
                sq2 = b.scratch(DT)  # sin^2(q/4)
                nc.scalar.activation(sq2, sq, AF.Square)
                ch = b.scratch(DT)   # cos(q/2)
                nc.scalar.activation(ch, sq2, AF.Copy, bias=1.0, scale=-2.0)
                sh2 = b.scratch(DT)  # sin^2(q/2)
                nc.scalar.activation(sh2, sh, AF.Square)
                nc.scalar.activation(c_ap, sh2, AF.Copy, bias=1.0, scale=-2.0)
                nc.vector.tensor_tensor(s_ap, sh, ch, OP.mult)  # sin q / 2
                if use_pack_fwd or use_pack_bwd:
                    # slots 2,3: [-sin/2 | cos] for packed-rotation patterns
                    nc.scalar.activation(t4[:, 2 * F:3 * F], s_ap, AF.Copy,
                                         bias=0.0, scale=-1.0)
                    nc.scalar.activation(t4[:, 3 * F:4 * F], c_ap, AF.Copy,
                                         bias=0.0, scale=1.0)
                    b.n_act += 2
                b.n_tt += 1
                b.n_act += 6
                se = Expr([(2.0, s_ap)])
                se._mat = (2.0, s_ap)
                trig[j] = (as_pers_expr(c_ap), se)
                trig_raw[j] = t4

            emit_trig(0)
            emit_trig(1)
            b.joint_boundary()



            # qd repack on DVE, after the sin products in the in-order DVE
            # stream (qd's DMA rides the slower gpsimd queue; putting these
            # first would block the trig products behind that DMA).
            qd_pl = []
            for j in range(ND - 1):  # qd_6 is only read once (tau_6)
                d_ap = b.persistent(f"qd{j}")
                nc.vector.tensor_scalar(d_ap, qd3[:, :, j], 1.0, None, OP.mult)
                qd_pl.append(as_pers_expr(d_ap))

            def qdd_expr(j):
                # strided read (28B stride) is cheapest, measured against all
                # repack routes: ACT copy queues behind the trig prologue
                # (+5us); DVE copy costs more than the ~190ns/op penalty;
                # SBUF->SBUF de-stride DMA on the sync queue is ~19x derated
                # and its sem deps stall the forward chains (+23us).
                ap = qdd3[:, :, j]
                e = Expr([(1.0, ap)])
                e._mat = (1.0, ap)
                b.pers_ids.add(id(ap))
                return e

            def rot_inv(j, X):
                """Rz(q_j)^T @ (rot_fix_j^T @ X)"""
                if not any(_nonzero(e) for e in X):
                    return [ZERO, ZERO, ZERO]
                Fm = rot_fix[j].T
                Y = b.matvec_const(Fm, X)
                c, s = trig[j]
                z0 = b.lin((1.0, b.mul(c, Y[0])), (1.0, b.mul(s, Y[1])))
                z1 = b.lin((-1.0, b.mul(s, Y[0])), (1.0, b.mul(c, Y[1])))
                return [z0, z1, Y[2]]

            def write_out(j, e):
                dst = out3[:, :, j]
                if not e.terms:
                    b.nc.vector.memset(dst, float(e.const))
                    return
                c, ap = b.mat(e)
                nc.scalar.activation(dst, ap, AF.Copy, bias=float(e.const),
                                     scale=float(c))
                b.n_act += 1

            # ---------------- forward (joints 0..5) ----------------
            w_p = [ZERO, ZERO, ZERO]
            v_p = [ZERO, ZERO, ZERO]
            a_p = [ZERO, ZERO, ZERO]
            la_p = [ZERO, ZERO, Expr(const=GRAV)]
            states = []
            use_qpack = os.environ.get("K_QPACK", "1") == "1"
            fwd_ztags = {}
            for j in range(ND - 1):
                t_j = trans[j]
                Uv = b.vsub(v_p, b.cross_const(t_j, w_p))
                Ua = b.vsub(la_p, b.cross_const(t_j, a_p))
                Uv = [b.snap(e, f"Uv{j}", scratch_ok=True)
                      if len(e.terms) > 1 else e for e in Uv]
                Ua = [b.snap(e, f"Ua{j}", scratch_ok=True)
                      if len(e.terms) > 1 else e for e in Ua]
                packed_done = False
                if use_qpack and j > 0:
                    Fm = rot_fix[j].T
                    Yw = b.matvec_const(Fm, w_p)
                    Yv = b.matvec_const(Fm, Uv)
                    Ya = b.matvec_const(Fm, a_p)
                    Yu = b.matvec_const(Fm, Ua)
                    pairs = [(Yw[0], Yw[1]), (Yv[0], Yv[1]),
                             (Ya[0], Ya[1]), (Yu[0], Yu[1])]
                    if all(e.terms for pr in pairs for e in pr):
                        t4 = trig_raw[j]
                        if os.environ.get("K_QR2", "1") == "1":
                            z0s, z1s, ztags = b.quad_rot2(
                                pairs, t4[:, 0:F], t4[:, F:2 * F], f"q{j}")
                        else:
                            z0s, z1s, ztags = b.quad_rot(
                                pairs, t4[:, 0:F], t4[:, F:2 * F], f"q{j}")
                        Rw = [z0s[0], z1s[0], Yw[2]]
                        Rv = [z0s[1], z1s[1], Yv[2]]
                        Ra = [z0s[2], z1s[2], Ya[2]]
                        Rla = [z0s[3], z1s[3], Yu[2]]
                        fwd_ztags[j] = ztags
                        packed_done = True
                _kpj = int(os.environ.get("K_PACK_J", "-1"))
                if not packed_done and use_pack_fwd and j > 0 \
                        and (_kpj < 0 or j == _kpj):
                    Fm = rot_fix[j].T

                    def _rows(X):
                        return [b.lin((Fm[i, 0], X[0]), (Fm[i, 1], X[1]),
                                      (Fm[i, 2], X[2])) for i in range(3)]

                    Yw, Yv, Ya, Yu = _rows(w_p), _rows(Uv), _rows(a_p), _rows(Ua)
                    heads = [Yw[0], Yw[1], Yv[0], Yv[1],
                             Ya[0], Ya[1], Yu[0], Yu[1]]
                    if all(e.terms for e in heads):
                        t4 = trig_raw[j]
                        zw = b.packed_pair_rot(Yw[0], Yw[1], t4, +1,
                                               b.persistent(f"w{j}0"),
                                               b.persistent(f"w{j}1"), True)
                        zv = b.packed_pair_rot(Yv[0], Yv[1], t4, +1,
                                               b.persistent(f"v{j}0"),
                                               b.persistent(f"v{j}1"), True)
                        za = b.packed_pair_rot(Ya[0], Ya[1], t4, +1,
                                               b.scratch(), b.scratch(), False)
                        zu = b.packed_pair_rot(Yu[0], Yu[1], t4, +1,
                                               b.scratch(), b.scratch(), False)
                        Rw = [zw[0], zw[1], Yw[2]]
                        Rv = [zv[0], zv[1], Yv[2]]
                        Ra = [za[0], za[1], Ya[2]]
                        Rla = [zu[0], zu[1], Yu[2]]
                        packed_done = True
                if not packed_done:
                    Rw = rot_inv(j, w_p)
                    Rv = rot_inv(j, Uv)
                    Ra = rot_inv(j, a_p)
                    Rla = rot_inv(j, Ua)
                qdj = qd_pl[j]
                qddj = qdd_expr(j)
                w = [Rw[0], Rw[1], b.lin((1.0, Rw[2]), (1.0, qdj))]
                w = b.snap_vec(w, f"w{j}_")
                v = b.snap_vec(Rv, f"v{j}_")
                dw = [
                    b.lin((1.0, Ra[0]), (1.0, b.mul(w[1], qdj))),
                    b.lin((1.0, Ra[1]), (-1.0, b.mul(w[0], qdj))),
                    b.lin((1.0, Ra[2]), (1.0, qddj)),
                ]
                dv = [
                    b.lin((1.0, Rla[0]), (1.0, b.mul(v[1], qdj))),
                    b.lin((1.0, Rla[1]), (-1.0, b.mul(v[0], qdj))),
                    Rla[2],
                ]
                dw = b.snap_vec(dw, f"dw{j}_")
                dv = b.snap_vec(dv, f"dv{j}_")
                states.append((w, v, dw, dv))
                w_p, v_p, a_p, la_p = w, v, dw, dv
                if j + 2 < ND - 1:
                    emit_trig(j + 2)
                if j in (1, 3):
                    b.new_wave()
                b.joint_boundary()

            # ---------------- backward (j = 5..0) ----------------
            # tau_6 = damping_6 * qd_6 is qd_6's only read: use the strided
            # column directly instead of a dense repack copy.
            qd6 = Expr([(1.0, qd3[:, :, ND - 1])])
            qd6._mat = (1.0, qd3[:, :, ND - 1])
            b.pers_ids.add(id(qd6.terms[0][1]))
            write_out(ND - 1, b.lin((damping[ND - 1], qd6)))

            lin_f = [ZERO, ZERO, ZERO]
            ang_f = [ZERO, ZERO, ZERO]
            bw_pack = None
            use_brot = os.environ.get("K_BROT", "1") == "1"
            use_dveseed = os.environ.get("K_DVESEED", "0") == "1"
            for j in range(ND - 2, -1, -1):
                if j in (5, 4, 3, 2, 1, 0):
                    b.new_wave()
                have_child = any(_nonzero(e) for e in lin_f + ang_f)
                if have_child and use_pack_bwd and bw_pack is not None:
                    bw_tile_p, lfc, afc, prev_tag = bw_pack
                    t4 = trig_raw[j + 1]
                    Rf = rot_fix[j + 1]
                    zlf = b.packed_pair_rot(
                        None, None, t4, -1, b.scratch(), b.scratch(), False,
                        A_ready=(bw_tile_p[:, 0:2 * F],) + lfc)
                    zaf = b.packed_pair_rot(
                        None, None, t4, -1, b.scratch(), b.scratch(), False,
                        A_ready=(bw_tile_p[:, 2 * F:4 * F],) + afc)
                    b.free_wide.setdefault(4, []).append(prev_tag)
                    RzLf = [zlf[0], zlf[1], lin_f[2]]
                    RzAf = [zaf[0], zaf[1], ang_f[2]]
                    Rc_lf = b.matvec_const(Rf, RzLf)
                    Rc_lf = [b.snap(e, f"rclf{j}", scratch_ok=True)
                             if len(e.terms) > 2 and (j > 0 or i < 2) else e
                             for i, e in enumerate(Rc_lf)]
                    Rc_af = b.matvec_const(Rf, RzAf)
                    child_ang = b.vadd(b.cross_const(trans[j + 1], Rc_lf), Rc_af)
                    child_lin = Rc_lf
                elif have_child and use_brot:
                    Rf = rot_fix[j + 1]
                    t4c = trig_raw[j + 1]
                    _brot2 = os.environ.get("K_QR2", "1") == "1"
                    _brfn = b.pair_rot_bwd2 if _brot2 else b.pair_rot_bwd
                    lfp, afp = _brfn(
                        (lin_f[0], lin_f[1]), (ang_f[0], ang_f[1]),
                        t4c[:, 0:F], t4c[:, F:2 * F], f"br{j}",
                        dve_seed=(j <= 1 and use_dveseed))
                    RzLf = [lfp[0], lfp[1], lin_f[2]]
                    RzAf = [afp[0], afp[1], ang_f[2]]
                    Rc_lf = b.matvec_const(Rf, RzLf)
                    Rc_lf = [b.snap(e, f"rclf{j}", scratch_ok=True)
                             if len(e.terms) > 2 and (j > 0 or i < 2) else e
                             for i, e in enumerate(Rc_lf)]
                    Rc_af = b.matvec_const(Rf, RzAf)
                    child_ang = b.vadd(b.cross_const(trans[j + 1], Rc_lf), Rc_af)
                    child_lin = Rc_lf
                elif have_child:
                    cs, ss = trig[j + 1]
                    Rf = rot_fix[j + 1]
                    # xy-rotation outputs fan out 3x through the rot_fix
                    # matvec: snapping them saves (t-1)(f-1) chain slots.
                    RzLf = [
                        b.snap(b.lin((1.0, b.mul(cs, lin_f[0])),
                                     (-1.0, b.mul(ss, lin_f[1]))),
                               f"rzlf{j}0", scratch_ok=True),
                        b.snap(b.lin((1.0, b.mul(ss, lin_f[0])),
                                     (1.0, b.mul(cs, lin_f[1]))),
                               f"rzlf{j}1", scratch_ok=True),
                        lin_f[2],
                    ]
                    Rc_lf = b.matvec_const(Rf, RzLf)
                    Rc_lf = [b.snap(e, f"rclf{j}", scratch_ok=True)
                             if len(e.terms) > 2 and (j > 0 or i < 2) else e
                             for i, e in enumerate(Rc_lf)]
                    RzAf = [
                        b.snap(b.lin((1.0, b.mul(cs, ang_f[0])),
                                     (-1.0, b.mul(ss, ang_f[1]))),
                               f"rzaf{j}0", scratch_ok=True),
                        b.snap(b.lin((1.0, b.mul(ss, ang_f[0])),
                                     (1.0, b.mul(cs, ang_f[1]))),
                               f"rzaf{j}1", scratch_ok=True),
                        ang_f[2],
                    ]
                    Rc_af = b.matvec_const(Rf, RzAf)
                    child_ang = b.vadd(b.cross_const(trans[j + 1], Rc_lf), Rc_af)
                    child_lin = Rc_lf
                else:
                    child_ang = [ZERO, ZERO, ZERO]
                    child_lin = [ZERO, ZERO, ZERO]

                # free previous joint's planes only after the child rotation
                # has consumed the raw lf/af term planes
                if j != ND - 2:
                    for vec in states[j + 1]:
                        b.free_expr_vec(vec)
                    if j + 1 in fwd_ztags:
                        for tag in fwd_ztags.pop(j + 1):
                            b.free_wide.setdefault(2, []).append(tag)

                w, v, dw, dv = states[j]
                m = float(mass[j])
                mc = m * com[j]
                cxm = np.array([
                    [0.0, -com[j][2], com[j][1]],
                    [com[j][2], 0.0, -com[j][0]],
                    [-com[j][1], com[j][0], 0.0],
                ])
                Isp = inertia[j] + m * (cxm @ cxm.T)

                if j == 0 and not _nonzero(w[0]) and not _nonzero(w[1]) \
                        and not any(_nonzero(e) for e in v):
                    # Base joint: only ang_f.z feeds torque_0 and nothing
                    # consumes lin_f_0/ang_f_0 further down. With w=(0,0,qd0)
                    # and v=0, tmp_a.z = (w x IcV_a).z + (v x IcV_l).z = 0,
                    # so af_z = (Isp dw + mc x dv).z + child_ang.z only.
                    af2 = b.lin(
                        (Isp[2, 0], dw[0]), (Isp[2, 1], dw[1]),
                        (Isp[2, 2], dw[2]),
                        (mc[0], dv[1]), (-mc[1], dv[0]),
                        (1.0, child_ang[2]))
                    write_out(0, b.lin((1.0, af2), (damping[0], qd_pl[0])))
                    b.joint_boundary()
                    continue

                IcA_l = b.vsub([b.lin((m, dv[i])) for i in range(3)],
                               b.cross_const(mc, dw))
                IcA_a = b.vadd(b.matvec_const(Isp, dw), b.cross_const(mc, dv))
                IcV_l = b.vsub([b.lin((m, v[i])) for i in range(3)],
                               b.cross_const(mc, w))
                IcV_a = b.vadd(b.matvec_const(Isp, w), b.cross_const(mc, v))
                if use_xpack:
                    # Packed crosses: operands land EXACTLY (coef 1, const
                    # folded) in cyclically-duplicated [x0|x1|x2|x0|x1]
                    # tiles; each cross is then 2 wide products + 1 wide
                    # subtract instead of 6 narrow products + per-component
                    # chain slots.
                    tail = j <= 1 and use_dveseed
                    cw, cw_tag = b.cyc_tile(w, f"cw{j}", dve_seed=tail)
                    cv, cv_tag = b.cyc_tile(v, f"cv{j}", dve_seed=tail)
                    cva, cva_tag = b.cyc_tile(IcV_a, f"cva{j}", dve_seed=tail)
                    cvl, cvl_tag = b.cyc_tile(IcV_l, f"cvl{j}", dve_seed=tail)
                    C1 = b.cross_packed(cw, cva)
                    C2 = b.cross_packed(cv, cvl)
                    C3 = b.cross_packed(cw, cvl)
                    nc.vector.tensor_tensor(C1, C1, C2, OP.add)
                    b.n_tt += 1

                    def _slices(t):
                        out = []
                        for k in range(3):
                            ap = t[:, k * F:(k + 1) * F]
                            b.pers_ids.add(id(ap))
                            e = Expr([(1.0, ap)])
                            e._mat = (1.0, ap)
                            out.append(e)
                        return out

                    tmp_a = _slices(C1)
                    tmp_l = _slices(C3)
                    for tag in (cw_tag, cv_tag, cva_tag, cvl_tag):
                        b.free_wide.setdefault(5, []).append(tag)
                else:
                    tmp_a = b.vadd(b.cross_ee(w, IcV_a), b.cross_ee(v, IcV_l))
                    tmp_l = b.cross_ee(w, IcV_l)
                lf_new = b.vadd(IcA_l, tmp_l, child_lin)
                af_new = b.vadd(IcA_a, tmp_a, child_ang)
                if use_pack_bwd and j > 0:
                    bw_tile, bw_tag = b.persistent_wide(4, f"bw{j}")
                    lf0 = b.snap_to(lf_new[0], bw_tile[:, 0:F])
                    lf1 = b.snap_to(lf_new[1], bw_tile[:, F:2 * F])
                    af0 = b.snap_to(af_new[0], bw_tile[:, 2 * F:3 * F])
                    af1 = b.snap_to(af_new[1], bw_tile[:, 3 * F:4 * F])
                    lf2 = b.snap(lf_new[2], f"lf{j}2")
                    af2 = b.snap(af_new[2], f"af{j}2")
                    lin_f = [lf0, lf1, lf2]
                    ang_f = [af0, af1, af2]
                    bw_pack = (bw_tile,
                               (lf0._mat[0], lf1._mat[0],
                                lf0.const, lf1.const),
                               (af0._mat[0], af1._mat[0],
                                af0.const, af1.const), bw_tag)
                elif use_brot:
                    # carry lf/af xy raw: the next joint's packed rotation
                    # materializes them directly into its operand tile
                    lin_f = [lf_new[0], lf_new[1],
                             b.snap(lf_new[2], f"lf{j}2")]
                    ang_f = [af_new[0], af_new[1],
                             b.snap(af_new[2], f"af{j}2")]
                    bw_pack = None
                else:
                    lin_f = b.snap_vec(lf_new, f"lf{j}_")
                    ang_f = b.snap_vec(af_new, f"af{j}_")
                    bw_pack = None
                write_out(j, b.lin((1.0, ang_f[2]), (damping[j], qd_pl[j])))
                b.joint_boundary()

            nc.sync.dma_start(dram_view(tq_d), out_t[:, :])

            stats = dict(stt=b.n_stt, tt=b.n_tt, act=b.n_act, copy=b.n_copy,
                         mm=b.n_mm, nw=[len(w) for w in b.wave_coefs],
                         busy={k: round(v / 1000.0, 1)
                               for k, v in b.busy.items()},
                         pers=b.pers_idx, max_joint_allocs=b.max_joint_allocs,
                         ring=b.ring_size,
                         mat_hist=dict(sorted(getattr(b, "mat_hist", {}).items())))
            stats["wcoefs"] = [list(w) for w in b.wave_coefs]
            stats["wmax"] = wmax if use_pe else 0
            stats["n_waves"] = n_waves

    nc.compile()
    return nc, stats


_CACHE = {}


def _get_module(params):
    import hashlib
    key = b"".join(np.ascontiguousarray(np.asarray(params[k], np.float32)).tobytes()
                   for k in ("trans", "rot_fix", "mass", "com", "inertia",
                             "damping"))
    h = hashlib.sha1(key).hexdigest()
    if h not in _CACHE:
        _CACHE[h] = build_module(params)
    return _CACHE[h]


def run(q, qd, qdd_des, trans, rot_fix, mass, com, inertia, damping,
        trace=False):
    q = np.asarray(q)
    qd = np.asarray(qd)
    qdd_des = np.asarray(qdd_des)
    assert q.shape == (B_TOTAL, ND), f"unexpected q shape {q.shape}"
    assert qd.shape == (B_TOTAL, ND) and qdd_des.shape == (B_TOTAL, ND)
    params = dict(trans=trans, rot_fix=rot_fix, mass=mass, com=com,
                  inertia=inertia, damping=damping)
    nc, stats = _get_module(params)
    wconst = None
    if stats.get("wmax"):
        whalf = stats["wmax"] // 2
        nw = stats["n_waves"]
        wconst = np.zeros((P, nw * whalf * P), np.float32)
        eye = np.eye(P, dtype=np.float32)
        for wv, coefs in enumerate(stats["wcoefs"]):
            for s, cval in enumerate(coefs):
                a = (wv * whalf + s) * P
                wconst[:, a:a + P] = np.float32(cval) * eye
    in_maps = []
    for c in range(N_CORES):
        sl = slice(c * BC, (c + 1) * BC)
        m = {
            "q": np.ascontiguousarray(q[sl], np.float32),
            "qd": np.ascontiguousarray(qd[sl], np.float32),
            "qdd_des": np.ascontiguousarray(qdd_des[sl], np.float32),
        }
        if wconst is not None:
            m["wconst"] = wconst
        in_maps.append(m)
    res = bass_utils.run_bass_kernel_spmd(
        nc, in_maps, core_ids=list(range(N_CORES)), trace=trace)
    out = np.concatenate([res.results[c]["torque"] for c in range(N_CORES)],
                         axis=0)
    return out.astype(np.float32), res, stats


def kernel(q, qd, qdd_des, trans, rot_fix, mass, com, inertia, damping):
    out, _, _ = run(q, qd, qdd_des, trans, rot_fix, mass, com, inertia,
                    damping, trace=False)
    return out

